# revision 11
# baseline (speedup 1.0000x reference)
"""Trainium2 Bass kernel for SegmentationAugmentation (3D affine grid_sample, trilinear, border).

Contract: kernel(input_g, label_g, transform) -> (aug_inp f32 [8,1,128,128,128],
                                                  aug_lab bool [8,1,128,128,128])

Math (swapaxes folded into index bookkeeping; all spatial dims 128):

  out[b,c,i,j,k] = trilinear sample of input_g[b,c,:,:,:] at
      p-axis: U(i,j) = clip(64*(a00*xn(i)+a01*xn(j)+a03)+63.5, 0, 127)
      q-axis: V(i,j) = clip(64*(a10*xn(i)+a11*xn(j)+a13)+63.5, 0, 127)
      r-axis: W(k)   = clip(64*(a22*xn(k)+a23)+63.5, 0, 127)
  with xn(t) = (2t+1)/128 - 1, theta = transform[:3].  Relies on the
  generator's z-rotation structure (theta[0:2,2]==0, theta[2,0:2]==0); a
  pure-host fallback handles arbitrary transforms.

Device pipeline, data parallel over batch (core b handles batch b; each core
processes BOTH its image and label volume in one fused f16 program):

  Phase 1 (dense): load pre-transposed f16 volume chunks, z-interp on DVE via
  run-segmented staircase slices (f16, 2x perf mode), then DMA the z-interped
  rows into an interleaved pair layout in DRAM:
     ZPI[r = p*128+q] = [Z0(p,q) | Z0(p+1,q) | Z1(p,q) | Z1(p+1,q)]  (1 KiB)
  (Z0 = image, Z1 = label; each row is 128 f16 k-values.)

  Phase 2 (gather): for each output point (i,j), ONE dma_gather descriptor of
  2 KiB at entry r=(p0*128+q0) fetches entries r,r+1 = all four bilinear
  corners of BOTH volumes.  DVE combines with k-replicated f16 weight tiles
  (streamed from DRAM per call so every operand keeps innermost stride 1 and
  2-byte dtype -> DVE 2x perf mode); one 512B-descriptor DMA per call writes
  the interleaved f16 outputs of both volumes.

Host converts the f16 outputs to f32 / bool; label voxels within FIX_EPS of
0.5 are recomputed in the reference's exact f32 arithmetic order.
"""
import numpy as np

N = 128
NROWS = N * N            # 16384 (p,q) rows per volume
NIDX = 1024              # gather indices (output points) per dma_gather call
GPC = NIDX // 128        # 8 element groups per partition per call
NCALLS = NROWS // NIDX   # 16 gather calls per rep
COLS = NIDX // 16        # idx table columns per call
ELEM = 1024              # gathered f16 elems per descriptor (= 2 ZPI entries)
ESTEP = 512              # f16 elems per ZPI entry (gather elem_step)
CH = 16                  # 128-row groups per load chunk
NGRP = NROWS // N // CH  # 8 chunks per volume
ZG = 129                 # zt groups (128 data + 1 zero pad)
FIX_EPS = 8e-3           # |label-0.5| below this -> exact host recompute

_CACHE = {}


def _mkap(pairs):
    import bass_rust
    return bass_rust.VecI64Pair([tuple(p) for p in pairs])


def _host_tables(theta):
    """All transform-derived tables, computed in float64 from f32 theta."""
    th = theta.astype(np.float64)
    t = np.arange(N, dtype=np.float64)
    xn = (2.0 * t + 1.0) / N - 1.0

    U = np.clip(64.0 * (th[0, 0] * xn[:, None] + th[0, 1] * xn[None, :] + th[0, 3]) + 63.5, 0.0, 127.0)
    V = np.clip(64.0 * (th[1, 0] * xn[:, None] + th[1, 1] * xn[None, :] + th[1, 3]) + 63.5, 0.0, 127.0)
    W = np.clip(64.0 * (th[2, 2] * xn + th[2, 3]) + 63.5, 0.0, 127.0)

    p0 = np.floor(U).astype(np.int64)
    q0 = np.floor(V).astype(np.int64)
    r0 = np.floor(W).astype(np.int64)
    fu = (U - p0).astype(np.float32)
    fv = (V - q0).astype(np.float32)
    fw = (W - r0).astype(np.float32)
    r1 = np.minimum(r0 + 1, N - 1)

    idxA = (p0 * 128 + q0).astype(np.int16)          # [i,j] ZPI entry index
    w00 = ((1 - fu) * (1 - fv)).astype(np.float32)
    w10 = (fu * (1 - fv)).astype(np.float32)
    w01 = ((1 - fu) * fv).astype(np.float32)
    w11 = (fu * fv).astype(np.float32)

    # z-run decomposition: maximal segments where both r0 and r1 step by a
    # constant d in {-1,0,1}
    runs = []
    k = 0
    while k < N:
        step = 0
        if k + 1 < N:
            d = int(r0[k + 1] - r0[k])
            if d == int(r1[k + 1] - r1[k]) and d in (-1, 0, 1):
                step = d
        ln = 1
        while (k + ln < N
               and int(r0[k + ln] - r0[k]) == step * ln
               and int(r1[k + ln] - r1[k]) == step * ln):
            ln += 1
        runs.append((k, ln, int(r0[k]), int(r1[k]), step))
        k += ln

    return dict(idxA=idxA, w00=w00, w01=w01, w10=w10, w11=w11, fw=fw, runs=runs)


def _pack_idxs(idx_flat):
    """int16 dma_gather index layout: element i at [i%16, i//16], replicated to 128 partitions."""
    t = idx_flat.reshape(-1, 16).T.astype(np.int16)  # [16, n/16]
    return np.ascontiguousarray(np.tile(t, (8, 1)))  # [128, n/16]


def _chunk_plan(tables):
    """Stream chunks (2048 ZPI entries each) in the order matching the calls'
    p-band progression; per call, how many streamed chunks it needs."""
    idxA = tables["idxA"].reshape(-1).astype(np.int64)
    # call c covers points c*NIDX..(c+1)*NIDX-1; entries r and r+1 needed
    need = []
    for c in range(NCALLS):
        rs = idxA[c * NIDX:(c + 1) * NIDX]
        need.append((int(rs.min()) // (CH * 128), (int(rs.max()) + 1) // (CH * 128)))
    first_lo, _ = need[0]
    last_lo, _ = need[-1]
    descending = first_lo >= last_lo
    order = list(range(NGRP - 1, -1, -1)) if descending else list(range(NGRP))
    pos = {g: i for i, g in enumerate(order)}
    nch = [max(pos[min(lo, NGRP - 1)], pos[min(hi, NGRP - 1)]) + 1 for lo, hi in need]
    return order, nch


def _build_program(tables, reps=1):
    """Raw-Bass (explicit semaphore) program; see module docstring for the
    pipeline.  All cross-engine waits are standalone wait_ge instructions.

    Engine streams:
      sync   (SP HWDGE):  const/volume loads, per-chunk ZPI stream writes
      scalar (ACT):       h1-slot entry assembly copies; weight-tile loads and
                          interleaved output writes (HWDGE)
      vector (DVE):       z-interp into ZS entry layout, 4-corner combine
      gpsimd (SWDGE):     one dma_gather per 1024 output points, fired as
                          soon as the chunks its points touch are streamed
    """
    import concourse.bass as bass
    from concourse import bacc, mybir

    runs = tables["runs"]
    f16 = mybir.dt.float16
    i16 = mybir.dt.int16

    nc = bacc.Bacc("TRN2", target_bir_lowering=False, debug=False, num_devices=8)

    vol_in = [nc.dram_tensor(f"vol{v}", [128, NROWS], f16, kind="ExternalInput") for v in range(2)]
    idx_dram = nc.dram_tensor("idxA", [128, NROWS // 16], i16, kind="ExternalInput")
    wtile = nc.dram_tensor("wtile", [NCALLS, 128, 4 * GPC * N], f16, kind="ExternalInput")
    fwrep = nc.dram_tensor("fwrep", [2, 128, 128], f16, kind="ExternalInput")
    out_i = nc.dram_tensor("outI", [NROWS, 256], f16, kind="ExternalOutput")
    zpi = nc.dram_tensor("zpi", [NROWS + 1, ESTEP], f16, kind="Internal")

    AP = bass.AP

    WD = 4 * GPC * N  # packed weight tile width (4096)
    idx_t = nc.alloc_sbuf_tensor("idx_t", [128, NROWS // 16], i16)
    fw_t = [nc.alloc_sbuf_tensor(f"fw{c}_t", [128, 128], f16) for c in range(2)]
    wt_sb = [nc.alloc_sbuf_tensor(f"wt_{s}", [128, WD], f16) for s in range(4)]
    vt16 = [nc.alloc_sbuf_tensor(f"vt16_{s}", [128, CH * N], f16) for s in range(2)]
    zs = [nc.alloc_sbuf_tensor(f"zs{s}", [128, CH * ESTEP], f16) for s in range(2)]
    ztmp = nc.alloc_sbuf_tensor("ztmp", [128, CH * N], f16)
    At = [nc.alloc_sbuf_tensor(f"At{s}", [128, GPC * ELEM], f16) for s in range(4)]
    mt = [nc.alloc_sbuf_tensor(f"m{s}", [128, GPC * N], f16) for s in range(8)]
    accb = [nc.alloc_sbuf_tensor(f"accb{s}", [128, GPC * 256], f16) for s in range(2)]

    nrows_ap = NROWS  # gather element at entry r reads entries r, r+1; r <= 16383
    NC_ = NCALLS
    ZSW = CH * ESTEP  # 8192
    chunk_order, nch = _chunk_plan(tables)
    # the h1 cross-chunk copy sources chunk g+1, which must already be in the
    # other ZS buffer -> chunks must stream top-down
    assert chunk_order == list(range(NGRP - 1, -1, -1)), chunk_order

    from contextlib import ExitStack
    with ExitStack() as _sctx:
        block = _sctx.enter_context(nc.Block())
        s_idx = _sctx.enter_context(nc.semaphore("s_idx"))
        s_wf = _sctx.enter_context(nc.semaphore("s_wf"))
        s_mz = _sctx.enter_context(nc.semaphore("s_mz"))
        s_l = [_sctx.enter_context(nc.semaphore(f"s_l{p}")) for p in range(2)]
        s_wl = _sctx.enter_context(nc.semaphore("s_wl"))
        s_z = _sctx.enter_context(nc.semaphore("s_z"))
        s_a = _sctx.enter_context(nc.semaphore("s_a"))
        s_zw = _sctx.enter_context(nc.semaphore("s_zw"))
        s_g = [_sctx.enter_context(nc.semaphore(f"s_g{p}")) for p in range(4)]
        s_c = _sctx.enter_context(nc.semaphore("s_c"))
        s_o = [_sctx.enter_context(nc.semaphore(f"s_o{p}")) for p in range(2)]
        s_v = _sctx.enter_context(nc.semaphore("s_v"))

        @block.sync
        def _(sync):
            sync.dma_start(idx_t.ap(), idx_dram.ap()).then_inc(s_idx, 16)
            for c in range(2):
                sync.dma_start(fw_t[c].ap(), AP(fwrep, c * 128 * 128, [[128, 128], [1, 128]])).then_inc(s_wf, 16)
            # one-time: zero ZPI entry 16384 (read by gathers at r=16383)
            sync.wait_ge(s_mz, 1)
            sync.dma_start(
                AP(zpi, NROWS * ESTEP, [[128, 4], [1, 128]]),
                AP(mt[0], 0, [[GPC * N, 4], [1, 128]]),
            ).then_inc(s_zw, 16)
            for r in range(reps):
                for gi, g in enumerate(chunk_order):
                    for v in range(2):
                        zc = r * 16 + gi * 2 + v
                        if zc >= 2:
                            sync.wait_ge(s_z, zc - 1)  # WAR vt16 vs z-interp
                        sync.dma_start(
                            AP(vt16[zc % 2], 0, [[CH * N, 128], [1, CH * N]]),
                            AP(vol_in[v], g * CH * N, [[NROWS, 128], [1, CH * N]]),
                        ).then_inc(s_l[zc % 2], 16)
                    if gi == 0 and r >= 1:
                        sync.wait_ge(s_c, NC_ * r)  # WAR zpi vs prev-rep gathers
                    sync.wait_ge(s_a, r * 8 + gi + 1)
                    sync.dma_start(
                        AP(zpi, g * CH * 128 * ESTEP, [[ESTEP, 128], [128 * ESTEP, CH], [1, ESTEP]]),
                        AP(zs[g % 2], 0, [[ZSW, 128], [ESTEP, CH], [1, ESTEP]]),
                    ).then_inc(s_zw, 16)
            sync.wait_ge(s_o[0], 16 * ((NC_ * reps + 1) // 2))
            sync.wait_ge(s_o[1], 16 * (NC_ * reps // 2))

        @block.scalar
        def _(scalar):
            def wt_load(gc):
                c = gc % NC_
                scalar.dma_start(
                    wt_sb[gc % 4].ap(),
                    AP(wtile, c * 128 * WD, [[WD, 128], [1, WD]]),
                ).then_inc(s_wl, 16)

            for gc in range(min(4, NC_ * reps)):
                wt_load(gc)
            for r in range(reps):
                for gi, g in enumerate(chunk_order):
                    # assemble h1 slots of chunk g: entry r gets row r+128,
                    # i.e. subgroup s copies from subgroup s+1 (h0 slot)
                    scalar.wait_ge(s_z, r * 16 + 2 * gi + 2)
                    b = zs[g % 2]
                    for v in range(2):
                        scalar.copy(
                            AP(b, v * 256 + 128, [[ZSW, 128], [ESTEP, CH - 1], [1, N]]),
                            AP(b, ESTEP + v * 256, [[ZSW, 128], [ESTEP, CH - 1], [1, N]]),
                        )
                    last_ins = None
                    for v in range(2):
                        dst = AP(b, (CH - 1) * ESTEP + v * 256 + 128, [[ZSW, 128], [1, N]])
                        if g == NGRP - 1:
                            # top chunk: rows >= 16384 are zero (border clamp
                            # gives these corners zero weight; keep finite)
                            last_ins = scalar.memzero(dst)
                        else:
                            src_b = zs[(g + 1) % 2]
                            last_ins = scalar.copy(dst, AP(src_b, v * 256, [[ZSW, 128], [1, N]]))
                    last_ins.then_inc(s_a, 1)
                for c in range(NC_):
                    gc = r * NC_ + c
                    scalar.wait_ge(s_c, gc + 1)
                    if gc + 4 < NC_ * reps:
                        wt_load(gc + 4)
                    scalar.dma_start(
                        AP(out_i, c * NIDX * 256, [[256, 128], [128 * 256, GPC], [1, 256]]),
                        AP(accb[gc % 2], 0, [[GPC * 256, 128], [256, GPC], [1, 256]]),
                    ).then_inc(s_o[gc % 2], 16)

        @block.gpsimd
        def _(gpsimd):
            nreg = gpsimd.to_reg(NIDX)
            gpsimd.wait_ge(s_idx, 16)
            sv = AP(zpi, 0, [[ESTEP, nrows_ap], [1, ELEM]])
            for r in range(reps):
                for c in range(NC_):
                    gc = r * NC_ + c
                    gpsimd.wait_ge(s_zw, 16 + 128 * r + 16 * nch[c])
                    if gc >= 4:
                        gpsimd.wait_ge(s_c, gc - 3)  # WAR At vs combine
                    gpsimd.dma_gather(
                        AP(At[gc % 4], 0, [[GPC * ELEM, 128], [ELEM, GPC], [1, ELEM]]),
                        sv,
                        AP(idx_t, c * COLS, [[NROWS // 16, 128], [1, COLS]]),
                        NIDX, nreg, ELEM, elem_step=ESTEP,
                    ).then_inc(s_g[gc % 4], 16)

        @block.vector
        def _(vector):
            mult = mybir.AluOpType.mult
            VC = [0]

            def vsync(last_ins):
                # DVE pipeline does not interlock same-engine RAW hazards
                last_ins.then_inc(s_v, 1)
                VC[0] += 1
                vector.wait_ge(s_v, VC[0])

            vector.wait_ge(s_wf, 32)
            vector.memset(AP(mt[0], 0, [[GPC * N, 4], [1, 128]]), 0.0).then_inc(s_mz, 1)

            def zchunk(r, gi, g, v):
                zc = r * 16 + gi * 2 + v
                if zc >= 1:
                    vector.wait_ge(s_z, zc)  # WAR ztmp/pipeline drain
                vector.wait_ge(s_l[zc % 2], 16 * (zc // 2 + 1))
                if v == 0 and r * 8 + gi >= 2:
                    # WAR zs[g%2] vs its previous chunk's stream + h1 cross-read
                    vector.wait_ge(s_a, r * 8 + gi)
                    vector.wait_ge(s_zw, 16 + 128 * r + 16 * (gi - 1))
                s = vt16[zc % 2]
                b = zs[g % 2]
                last_ins = None
                for (ks, ln, r0s, r1s, st) in runs:
                    zdst = AP(b, v * 256 + ks, [[ZSW, 128], [ESTEP, CH], [1, ln]])
                    tdst = AP(ztmp, ks, [[CH * N, 128], [N, CH], [1, ln]])
                    v0 = AP(s, r0s, [[CH * N, 128], [N, CH], [st, ln]])
                    v1 = AP(s, r1s, [[CH * N, 128], [N, CH], [st, ln]])
                    f0 = AP(fw_t[0], ks, [[128, 128], [0, CH], [1, ln]])
                    f1 = AP(fw_t[1], ks, [[128, 128], [0, CH], [1, ln]])
                    vector.tensor_tensor(zdst, v0, f0, mult)
                    last_ins = vector.tensor_tensor(tdst, v1, f1, mult)
                vsync(last_ins)
                for (ks, ln, r0s, r1s, st) in runs:
                    zdst = AP(b, v * 256 + ks, [[ZSW, 128], [ESTEP, CH], [1, ln]])
                    tdst = AP(ztmp, ks, [[CH * N, 128], [N, CH], [1, ln]])
                    last_ins = vector.tensor_add(zdst, zdst, tdst)
                last_ins.then_inc(s_z, 1)

            def combine(r, c):
                gc = r * NC_ + c
                if gc >= 1:
                    vector.wait_ge(s_c, gc)  # WAR mt vs prev combine
                vector.wait_ge(s_g[gc % 4], 16 * (gc // 4 + 1))
                vector.wait_ge(s_wl, 16 * (gc + 1))
                if gc >= 2:
                    vector.wait_ge(s_o[gc % 2], 16 * (gc // 2))  # WAR accb
                A = At[gc % 4]
                W = wt_sb[gc % 4]
                shp = [[GPC * ELEM, 128], [ELEM, GPC], [1, N]]
                oshp = [[GPC * N, 128], [N, GPC], [1, N]]

                def wb(ci):
                    return AP(W, ci * GPC * N, [[WD, 128], [N, GPC], [1, N]])
                maps = [AP(m, 0, oshp) for m in mt]
                # corner offsets within a gathered element (f16 elems):
                #   vol v: (p0,q0)=v*256, (p1,q0)=v*256+128,
                #          (p0,q1)=v*256+512, (p1,q1)=v*256+640
                last_ins = None
                for v in range(2):
                    b = 4 * v
                    vector.tensor_tensor(maps[b + 0], AP(A, v * 256 + 0, shp), wb(0), mult)
                    vector.tensor_tensor(maps[b + 1], AP(A, v * 256 + 128, shp), wb(2), mult)
                    vector.tensor_tensor(maps[b + 2], AP(A, v * 256 + 512, shp), wb(1), mult)
                    last_ins = vector.tensor_tensor(maps[b + 3], AP(A, v * 256 + 640, shp), wb(3), mult)
                vsync(last_ins)
                for v in range(2):
                    b = 4 * v
                    vector.tensor_add(maps[b + 0], maps[b + 0], maps[b + 1])
                    last_ins = vector.tensor_add(maps[b + 2], maps[b + 2], maps[b + 3])
                vsync(last_ins)
                osh2 = [[GPC * 256, 128], [256, GPC], [1, N]]
                vector.tensor_add(AP(accb[gc % 2], 0, osh2), maps[0], maps[2])
                vector.tensor_add(AP(accb[gc % 2], 128, osh2), maps[4], maps[6]) \
                    .then_inc(s_c, 1)

            for r in range(reps):
                for gi, g in enumerate(chunk_order):
                    for v in range(2):
                        zchunk(r, gi, g, v)
                for c in range(NC_):
                    combine(r, c)

    nc.compile()
    return nc


def _exact_label_fixup(label_g, theta, lab_f, out_bool):
    """Recompute voxels of |lab_f - 0.5| < FIX_EPS in the reference's exact
    f32 arithmetic order (validated bit-exact against the jax reference)."""
    eps = np.float32(FIX_EPS)
    cand = np.abs(lab_f - np.float32(0.5)) < eps
    if not cand.any():
        return out_bool
    bb, ii, jj, kk = np.nonzero(cand.reshape(-1, N, N, N))
    v = _exact_reference_values(label_g, theta, bb, ii, jj, kk)
    out_bool.reshape(-1, N, N, N)[bb, ii, jj, kk] = v > np.float32(0.5)
    return out_bool


def _exact_reference_values(vol_g, theta, bb, ii, jj, kk):
    """Reference-order f32 trilinear values at selected voxels.

    Replicates: grid einsum (x*t0 + y*t1 + z*t2, left-assoc f32) + t3; unnorm;
    8-corner accumulation in (z,y,x) order with w=(wz*wy)*wx, out += v*w.
    """
    f32 = np.float32
    t = np.arange(N, dtype=f32)
    xn = ((f32(2.0) * t + f32(1.0)) / f32(N) - f32(1.0)).astype(f32)
    th = theta.astype(f32)

    x = xn[ii]; y = xn[jj]; z = xn[kk]

    # f32 fma via f64 (exact up to negligible double-rounding corner cases)
    def fma32(a, b, c):
        return (np.float64(a) * np.float64(b) + c.astype(np.float64)).astype(f32)

    # grid components — XLA CPU lowers the einsum as an FMA chain (verified
    # bit-exact): fma(z, t2, fma(y, t1, x*t0)) + t3
    def comp(r):
        a = fma32(y, th[r, 1], (x * th[r, 0]).astype(f32))
        a = fma32(z, th[r, 2], a)
        return (a + th[r, 3]).astype(f32)
    gx, gy, gz = comp(0), comp(1), comp(2)

    def unnorm(c):
        return np.clip(((c + f32(1.0)) * f32(N) - f32(1.0)) * f32(0.5), f32(0.0), f32(N - 1))
    ux, uy, uz = unnorm(gx), unnorm(gy), unnorm(gz)
    x0 = np.floor(ux); y0 = np.floor(uy); z0 = np.floor(uz)
    fx = (ux - x0).astype(f32); fy = (uy - y0).astype(f32); fz = (uz - z0).astype(f32)
    x0i = x0.astype(np.int64); y0i = y0.astype(np.int64); z0i = z0.astype(np.int64)
    x1i = np.minimum(x0i + 1, N - 1); y1i = np.minimum(y0i + 1, N - 1); z1i = np.minimum(z0i + 1, N - 1)

    vol = vol_g.reshape(-1, N, N, N)
    out = np.zeros(bb.shape, f32)
    one = f32(1.0)
    for zi, wz in ((z0i, (one - fz).astype(f32)), (z1i, fz)):
        for yi, wy in ((y0i, (one - fy).astype(f32)), (y1i, fy)):
            for xi, wx in ((x0i, (one - fx).astype(f32)), (x1i, fx)):
                # inp[b, c, zi, yi, xi] in transposed space == vol[b, xi, yi, zi]
                vals = vol[bb, xi, yi, zi]
                w = ((wz * wy).astype(f32) * wx).astype(f32)
                out = (out + (vals * w).astype(f32)).astype(f32)
    return out


def _host_fallback(input_g, label_g, transform):
    """Arbitrary-transform fallback: full reference computation on host."""
    bb, ii, jj, kk = np.meshgrid(np.arange(8), np.arange(N), np.arange(N), np.arange(N), indexing="ij")
    bb, ii, jj, kk = (a.reshape(-1) for a in (bb, ii, jj, kk))
    theta = transform[:3].astype(np.float32)
    aug_inp = _exact_reference_values(input_g, theta, bb, ii, jj, kk).reshape(8, 1, N, N, N)
    lab = _exact_reference_values(label_g, theta, bb, ii, jj, kk).reshape(8, 1, N, N, N)
    return aug_inp.astype(np.float32), lab > np.float32(0.5)


def _make_inputs(tables, input_g, label_g):
    idx_p = _pack_idxs(tables["idxA"].reshape(-1))
    # packed k-replicated per-call weight tiles:
    # wtile[call, j, ci*GPC*128 + slot*128 + k] = w_ci(i = call*8 + slot, j)
    wt = np.empty((NCALLS, 128, 4, GPC, N), np.float16)
    for ci, nm in enumerate(("w00", "w01", "w10", "w11")):
        x = tables[nm].T.reshape(128, NCALLS, GPC).astype(np.float16)  # [j, call, slot]
        wt[:, :, ci] = x.transpose(1, 0, 2)[:, :, :, None]
    wtile = np.ascontiguousarray(wt.reshape(NCALLS, 128, 4 * GPC * N))
    fwrep = np.stack([np.tile(1.0 - tables["fw"], (128, 1)),
                      np.tile(tables["fw"], (128, 1))]).astype(np.float16)
    in_maps = []
    for b in range(8):
        in_maps.append({
            # partition-major: vol[p, gc*128+k] = volume[row=gc*128+p, k]
            "vol0": np.ascontiguousarray(
                input_g[b, 0].reshape(128, 128, N).astype(np.float16).transpose(1, 0, 2).reshape(128, NROWS)),
            "vol1": np.ascontiguousarray(
                label_g[b, 0].reshape(128, 128, N).astype(np.float16).transpose(1, 0, 2).reshape(128, NROWS)),
            "idxA": idx_p, "wtile": wtile, "fwrep": fwrep,
        })
    return in_maps


def kernel(input_g, label_g, transform):
    input_g = np.ascontiguousarray(input_g, dtype=np.float32)
    label_g = np.ascontiguousarray(label_g, dtype=np.float32)
    transform = np.asarray(transform, dtype=np.float32)
    theta = transform[:3]

    structured = (abs(float(theta[0, 2])) < 1e-12 and abs(float(theta[1, 2])) < 1e-12
                  and abs(float(theta[2, 0])) < 1e-12 and abs(float(theta[2, 1])) < 1e-12)
    if not structured:
        return _host_fallback(input_g, label_g, transform)

    from concourse.bass_utils import run_bass_kernel_spmd

    tables = _host_tables(theta)
    key = transform.tobytes()
    if key not in _CACHE:
        _CACHE[key] = _build_program(tables)
    nc = _CACHE[key]

    in_maps = _make_inputs(tables, input_g, label_g)
    res = run_bass_kernel_spmd(nc, in_maps, core_ids=list(range(8)))

    aug_inp = np.empty((8, 1, N, N, N), np.float32)
    lab_f = np.empty((8, 1, N, N, N), np.float32)
    for b in range(8):
        oi = res.results[b]["outI"]
        aug_inp[b, 0] = oi[:, 0:128].astype(np.float32).reshape(N, N, N)
        lab_f[b, 0] = oi[:, 128:256].astype(np.float32).reshape(N, N, N)

    out_bool = lab_f > np.float32(0.5)
    out_bool = _exact_label_fixup(label_g, theta, lab_f, out_bool)
    return aug_inp, out_bool


# revision 16
# speedup vs baseline: 1.1347x; 1.1347x over previous
"""Trainium2 Bass kernel for SegmentationAugmentation (3D affine grid_sample, trilinear, border).

Contract: kernel(input_g, label_g, transform) -> (aug_inp f32 [8,1,128,128,128],
                                                  aug_lab bool [8,1,128,128,128])

Math (swapaxes folded into index bookkeeping; all spatial dims 128):

  out[b,c,i,j,k] = trilinear sample of input_g[b,c,:,:,:] at
      p-axis: U(i,j) = clip(64*(a00*xn(i)+a01*xn(j)+a03)+63.5, 0, 127)
      q-axis: V(i,j) = clip(64*(a10*xn(i)+a11*xn(j)+a13)+63.5, 0, 127)
      r-axis: W(k)   = clip(64*(a22*xn(k)+a23)+63.5, 0, 127)
  with xn(t) = (2t+1)/128 - 1, theta = transform[:3].  Relies on the
  generator's z-rotation structure (theta[0:2,2]==0, theta[2,0:2]==0); a
  pure-host fallback handles arbitrary transforms.

Device pipeline, data parallel over batch (core b handles batch b; each core
processes BOTH its image and label volume in one fused f16 program):

  Phase 1 (dense): load pre-transposed f16 volume chunks, z-interp on DVE via
  run-segmented staircase slices (f16, 2x perf mode), then DMA the z-interped
  rows into an interleaved pair layout in DRAM:
     ZPI[r = p*128+q] = [Z0(p,q) | Z0(p+1,q) | Z1(p,q) | Z1(p+1,q)]  (1 KiB)
  (Z0 = image, Z1 = label; each row is 128 f16 k-values.)

  Phase 2 (gather): for each output point (i,j), ONE dma_gather descriptor of
  2 KiB at entry r=(p0*128+q0) fetches entries r,r+1 = all four bilinear
  corners of BOTH volumes.  DVE combines with k-replicated f16 weight tiles
  (streamed from DRAM per call so every operand keeps innermost stride 1 and
  2-byte dtype -> DVE 2x perf mode); one 512B-descriptor DMA per call writes
  the interleaved f16 outputs of both volumes.

Host converts the f16 outputs to f32 / bool; label voxels within FIX_EPS of
0.5 are recomputed in the reference's exact f32 arithmetic order.
"""
import numpy as np

N = 128
NROWS = N * N            # 16384 (p,q) rows per volume
NIDX = 1024              # gather indices (output points) per dma_gather call
GPC = NIDX // 128        # 8 element groups per partition per call
NCALLS = NROWS // NIDX   # 16 gather calls per rep
COLS = NIDX // 16        # idx table columns per call
ELEM = 1024              # gathered f16 elems per descriptor (= 2 ZPI entries)
ESTEP = 512              # f16 elems per ZPI entry (gather elem_step)
CH = 16                  # 128-row groups per load chunk
NGRP = NROWS // N // CH  # 8 chunks per volume
ZG = 129                 # zt groups (128 data + 1 zero pad)
FIX_EPS = 8e-3           # |label-0.5| below this -> exact host recompute

_CACHE = {}


def _mkap(pairs):
    import bass_rust
    return bass_rust.VecI64Pair([tuple(p) for p in pairs])


def _host_tables(theta):
    """All transform-derived tables, computed in float64 from f32 theta."""
    th = theta.astype(np.float64)
    t = np.arange(N, dtype=np.float64)
    xn = (2.0 * t + 1.0) / N - 1.0

    U = np.clip(64.0 * (th[0, 0] * xn[:, None] + th[0, 1] * xn[None, :] + th[0, 3]) + 63.5, 0.0, 127.0)
    V = np.clip(64.0 * (th[1, 0] * xn[:, None] + th[1, 1] * xn[None, :] + th[1, 3]) + 63.5, 0.0, 127.0)
    W = np.clip(64.0 * (th[2, 2] * xn + th[2, 3]) + 63.5, 0.0, 127.0)

    p0 = np.floor(U).astype(np.int64)
    q0 = np.floor(V).astype(np.int64)
    r0 = np.floor(W).astype(np.int64)
    fu = (U - p0).astype(np.float32)
    fv = (V - q0).astype(np.float32)
    fw = (W - r0).astype(np.float32)
    r1 = np.minimum(r0 + 1, N - 1)

    idxA = (p0 * 128 + q0).astype(np.int16)          # [i,j] ZPI entry index
    w00 = ((1 - fu) * (1 - fv)).astype(np.float32)
    w10 = (fu * (1 - fv)).astype(np.float32)
    w01 = ((1 - fu) * fv).astype(np.float32)
    w11 = (fu * fv).astype(np.float32)

    # z-run decomposition: maximal segments where both r0 and r1 step by a
    # constant d in {-1,0,1}
    runs = []
    k = 0
    while k < N:
        step = 0
        if k + 1 < N:
            d = int(r0[k + 1] - r0[k])
            if d == int(r1[k + 1] - r1[k]) and d in (-1, 0, 1):
                step = d
        ln = 1
        while (k + ln < N
               and int(r0[k + ln] - r0[k]) == step * ln
               and int(r1[k + ln] - r1[k]) == step * ln):
            ln += 1
        runs.append((k, ln, int(r0[k]), int(r1[k]), step))
        k += ln

    return dict(idxA=idxA, w00=w00, w01=w01, w10=w10, w11=w11, fw=fw, runs=runs)


def _pack_idxs(idx_flat):
    """int16 dma_gather index layout: element i at [i%16, i//16], replicated to 128 partitions."""
    t = idx_flat.reshape(-1, 16).T.astype(np.int16)  # [16, n/16]
    return np.ascontiguousarray(np.tile(t, (8, 1)))  # [128, n/16]


def _chunk_plan(tables):
    """Stream chunks (2048 ZPI entries each) in the order matching the calls'
    p-band progression; per call, how many streamed chunks it needs."""
    idxA = tables["idxA"].reshape(-1).astype(np.int64)
    # call c covers points c*NIDX..(c+1)*NIDX-1; entries r and r+1 needed
    need = []
    for c in range(NCALLS):
        rs = idxA[c * NIDX:(c + 1) * NIDX]
        need.append((int(rs.min()) // (CH * 128), (int(rs.max()) + 1) // (CH * 128)))
    first_lo, _ = need[0]
    last_lo, _ = need[-1]
    descending = first_lo >= last_lo
    order = list(range(NGRP - 1, -1, -1)) if descending else list(range(NGRP))
    pos = {g: i for i, g in enumerate(order)}
    nch = [max(pos[min(lo, NGRP - 1)], pos[min(hi, NGRP - 1)]) + 1 for lo, hi in need]
    return order, nch


def _build_program(tables, reps=1):
    """Raw-Bass (explicit semaphore) program; see module docstring for the
    pipeline.  All cross-engine waits are standalone wait_ge instructions.

    Engine streams:
      sync   (SP HWDGE):  const/volume loads, per-chunk ZPI stream writes
      scalar (ACT):       h1-slot entry assembly copies; weight-tile loads and
                          interleaved output writes (HWDGE)
      vector (DVE):       z-interp into ZS entry layout, 4-corner combine
      gpsimd (SWDGE):     one dma_gather per 1024 output points, fired as
                          soon as the chunks its points touch are streamed
    """
    import concourse.bass as bass
    from concourse import bacc, mybir

    runs = tables["runs"]
    f16 = mybir.dt.float16
    i16 = mybir.dt.int16

    nc = bacc.Bacc("TRN2", target_bir_lowering=False, debug=False, num_devices=8)

    vol_in = [nc.dram_tensor(f"vol{v}", [128, NROWS], f16, kind="ExternalInput") for v in range(2)]
    idx_dram = nc.dram_tensor("idxA", [128, NROWS // 16], i16, kind="ExternalInput")
    wtile = nc.dram_tensor("wtile", [NCALLS, 128, 4 * GPC * N], f16, kind="ExternalInput")
    fwrep = nc.dram_tensor("fwrep", [2, 128, 128], f16, kind="ExternalInput")
    out_i = nc.dram_tensor("outI", [NROWS, 256], f16, kind="ExternalOutput")
    zpi = nc.dram_tensor("zpi", [NROWS + 1, ESTEP], f16, kind="Internal")

    AP = bass.AP

    WD = 4 * GPC * N  # packed weight tile width (4096)
    idx_t = nc.alloc_sbuf_tensor("idx_t", [128, NROWS // 16], i16)
    fw_t = [nc.alloc_sbuf_tensor(f"fw{c}_t", [128, 128], f16) for c in range(2)]
    wt_sb = [nc.alloc_sbuf_tensor(f"wt_{s}", [128, WD], f16) for s in range(4)]
    vt16 = [nc.alloc_sbuf_tensor(f"vt16_{s}", [128, CH * N], f16) for s in range(4)]
    zs = [nc.alloc_sbuf_tensor(f"zs{s}", [128, CH * ESTEP], f16) for s in range(2)]
    ztmp = nc.alloc_sbuf_tensor("ztmp", [128, CH * N], f16)
    At = [nc.alloc_sbuf_tensor(f"At{s}", [128, GPC * ELEM], f16) for s in range(4)]
    mt = [nc.alloc_sbuf_tensor(f"m{s}", [128, GPC * N], f16) for s in range(8)]
    accb = [nc.alloc_sbuf_tensor(f"accb{s}", [128, GPC * 256], f16) for s in range(2)]

    nrows_ap = NROWS  # gather element at entry r reads entries r, r+1; r <= 16383
    NC_ = NCALLS
    ZSW = CH * ESTEP  # 8192
    chunk_order, nch = _chunk_plan(tables)
    # the h1 cross-chunk copy sources chunk g+1, which must already be in the
    # other ZS buffer -> chunks must stream top-down
    assert chunk_order == list(range(NGRP - 1, -1, -1)), chunk_order

    from contextlib import ExitStack
    with ExitStack() as _sctx:
        block = _sctx.enter_context(nc.Block())
        s_idx = _sctx.enter_context(nc.semaphore("s_idx"))
        s_wf = _sctx.enter_context(nc.semaphore("s_wf"))
        s_mz = _sctx.enter_context(nc.semaphore("s_mz"))
        s_l = [_sctx.enter_context(nc.semaphore(f"s_l{p}")) for p in range(4)]
        s_wl = _sctx.enter_context(nc.semaphore("s_wl"))
        s_z = _sctx.enter_context(nc.semaphore("s_z"))
        s_a = _sctx.enter_context(nc.semaphore("s_a"))
        s_zw = _sctx.enter_context(nc.semaphore("s_zw"))
        s_g = [_sctx.enter_context(nc.semaphore(f"s_g{p}")) for p in range(4)]
        s_c = _sctx.enter_context(nc.semaphore("s_c"))
        s_o = [_sctx.enter_context(nc.semaphore(f"s_o{p}")) for p in range(2)]
        s_v = _sctx.enter_context(nc.semaphore("s_v"))

        @block.sync
        def _(sync):
            sync.dma_start(idx_t.ap(), idx_dram.ap()).then_inc(s_idx, 16)
            for c in range(2):
                sync.dma_start(fw_t[c].ap(), AP(fwrep, c * 128 * 128, [[128, 128], [1, 128]])).then_inc(s_wf, 16)
            # one-time: zero ZPI entry 16384 (read by gathers at r=16383)
            sync.wait_ge(s_mz, 1)
            sync.dma_start(
                AP(zpi, NROWS * ESTEP, [[128, 4], [1, 128]]),
                AP(mt[0], 0, [[GPC * N, 4], [1, 128]]),
            ).then_inc(s_zw, 16)
            def stream(r, gi):
                g = chunk_order[gi]
                if gi == 0 and r >= 1:
                    sync.wait_ge(s_c, NC_ * r)  # WAR zpi vs prev-rep gathers
                sync.wait_ge(s_a, r * 8 + gi + 1)
                sync.dma_start(
                    AP(zpi, g * CH * 128 * ESTEP, [[ESTEP, 128], [128 * ESTEP, CH], [1, ESTEP]]),
                    AP(zs[g % 2], 0, [[ZSW, 128], [ESTEP, CH], [1, ESTEP]]),
                ).then_inc(s_zw, 16)

            for r in range(reps):
                for gi, g in enumerate(chunk_order):
                    for v in range(2):
                        zc = r * 16 + gi * 2 + v
                        if zc >= 4:
                            sync.wait_ge(s_z, zc - 3)  # WAR vt16 vs z-interp
                        sync.dma_start(
                            AP(vt16[zc % 4], 0, [[CH * N, 128], [1, CH * N]]),
                            AP(vol_in[v], g * CH * N, [[NROWS, 128], [1, CH * N]]),
                        ).then_inc(s_l[zc % 4], 16)
                    # stream lags the loads by one chunk so load issue is
                    # never blocked behind the s_a wait
                    if gi >= 1:
                        stream(r, gi - 1)
                stream(r, NGRP - 1)
            sync.wait_ge(s_o[0], 16 * ((NC_ * reps + 1) // 2))
            sync.wait_ge(s_o[1], 16 * (NC_ * reps // 2))

        @block.scalar
        def _(scalar):
            def wt_load(gc):
                c = gc % NC_
                scalar.dma_start(
                    wt_sb[gc % 4].ap(),
                    AP(wtile, c * 128 * WD, [[WD, 128], [1, WD]]),
                ).then_inc(s_wl, 16)

            for gc in range(min(4, NC_ * reps)):
                wt_load(gc)
            for r in range(reps):
                for gi, g in enumerate(chunk_order):
                    # assemble h1 slots of chunk g: entry r gets row r+128,
                    # i.e. subgroup s copies from subgroup s+1 (h0 slot)
                    scalar.wait_ge(s_z, r * 16 + 2 * gi + 2)
                    b = zs[g % 2]
                    for v in range(2):
                        scalar.copy(
                            AP(b, v * 256 + 128, [[ZSW, 128], [ESTEP, CH - 1], [1, N]]),
                            AP(b, ESTEP + v * 256, [[ZSW, 128], [ESTEP, CH - 1], [1, N]]),
                        )
                    last_ins = None
                    for v in range(2):
                        dst = AP(b, (CH - 1) * ESTEP + v * 256 + 128, [[ZSW, 128], [1, N]])
                        if g == NGRP - 1:
                            # top chunk: rows >= 16384 are zero (border clamp
                            # gives these corners zero weight; keep finite)
                            last_ins = scalar.memzero(dst)
                        else:
                            src_b = zs[(g + 1) % 2]
                            last_ins = scalar.copy(dst, AP(src_b, v * 256, [[ZSW, 128], [1, N]]))
                    last_ins.then_inc(s_a, 1)
                for c in range(NC_):
                    gc = r * NC_ + c
                    scalar.wait_ge(s_c, gc + 1)
                    if gc + 4 < NC_ * reps:
                        wt_load(gc + 4)
                    scalar.dma_start(
                        AP(out_i, c * NIDX * 256, [[256, 128], [128 * 256, GPC], [1, 256]]),
                        AP(accb[gc % 2], 0, [[GPC * 256, 128], [256, GPC], [1, 256]]),
                    ).then_inc(s_o[gc % 2], 16)

        @block.gpsimd
        def _(gpsimd):
            nreg = gpsimd.to_reg(NIDX)
            gpsimd.wait_ge(s_idx, 16)
            sv = AP(zpi, 0, [[ESTEP, nrows_ap], [1, ELEM]])
            for r in range(reps):
                for c in range(NC_):
                    gc = r * NC_ + c
                    gpsimd.wait_ge(s_zw, 16 + 128 * r + 16 * nch[c])
                    if gc >= 4:
                        gpsimd.wait_ge(s_c, gc - 3)  # WAR At vs combine
                    gpsimd.dma_gather(
                        AP(At[gc % 4], 0, [[GPC * ELEM, 128], [ELEM, GPC], [1, ELEM]]),
                        sv,
                        AP(idx_t, c * COLS, [[NROWS // 16, 128], [1, COLS]]),
                        NIDX, nreg, ELEM, elem_step=ESTEP,
                    ).then_inc(s_g[gc % 4], 16)

        @block.vector
        def _(vector):
            mult = mybir.AluOpType.mult
            VC = [0]

            def vsync(last_ins):
                # DVE pipeline does not interlock same-engine RAW hazards
                last_ins.then_inc(s_v, 1)
                VC[0] += 1
                vector.wait_ge(s_v, VC[0])

            vector.wait_ge(s_wf, 32)
            vector.memset(AP(mt[0], 0, [[GPC * N, 4], [1, 128]]), 0.0).then_inc(s_mz, 1)

            def zchunk(r, gi, g, v):
                zc = r * 16 + gi * 2 + v
                if zc >= 1:
                    vector.wait_ge(s_z, zc)  # WAR ztmp/pipeline drain
                vector.wait_ge(s_l[zc % 4], 16 * (zc // 4 + 1))
                if v == 0 and r * 8 + gi >= 2:
                    # WAR zs[g%2] vs its previous chunk's stream + h1 cross-read
                    vector.wait_ge(s_a, r * 8 + gi)
                    vector.wait_ge(s_zw, 16 + 128 * r + 16 * (gi - 1))
                s = vt16[zc % 4]
                b = zs[g % 2]
                last_ins = None
                for (ks, ln, r0s, r1s, st) in runs:
                    zdst = AP(b, v * 256 + ks, [[ZSW, 128], [ESTEP, CH], [1, ln]])
                    tdst = AP(ztmp, ks, [[CH * N, 128], [N, CH], [1, ln]])
                    v0 = AP(s, r0s, [[CH * N, 128], [N, CH], [st, ln]])
                    v1 = AP(s, r1s, [[CH * N, 128], [N, CH], [st, ln]])
                    f0 = AP(fw_t[0], ks, [[128, 128], [0, CH], [1, ln]])
                    f1 = AP(fw_t[1], ks, [[128, 128], [0, CH], [1, ln]])
                    vector.tensor_tensor(zdst, v0, f0, mult)
                    last_ins = vector.tensor_tensor(tdst, v1, f1, mult)
                vsync(last_ins)
                for (ks, ln, r0s, r1s, st) in runs:
                    zdst = AP(b, v * 256 + ks, [[ZSW, 128], [ESTEP, CH], [1, ln]])
                    tdst = AP(ztmp, ks, [[CH * N, 128], [N, CH], [1, ln]])
                    last_ins = vector.tensor_add(zdst, zdst, tdst)
                last_ins.then_inc(s_z, 1)

            def combine(r, c):
                gc = r * NC_ + c
                if gc >= 1:
                    vector.wait_ge(s_c, gc)  # WAR mt vs prev combine
                vector.wait_ge(s_g[gc % 4], 16 * (gc // 4 + 1))
                vector.wait_ge(s_wl, 16 * (gc + 1))
                if gc >= 2:
                    vector.wait_ge(s_o[gc % 2], 16 * (gc // 2))  # WAR accb
                A = At[gc % 4]
                W = wt_sb[gc % 4]
                shp = [[GPC * ELEM, 128], [ELEM, GPC], [1, N]]
                oshp = [[GPC * N, 128], [N, GPC], [1, N]]

                def wb(ci):
                    return AP(W, ci * GPC * N, [[WD, 128], [N, GPC], [1, N]])
                maps = [AP(m, 0, oshp) for m in mt]
                # corner offsets within a gathered element (f16 elems):
                #   vol v: (p0,q0)=v*256, (p1,q0)=v*256+128,
                #          (p0,q1)=v*256+512, (p1,q1)=v*256+640
                last_ins = None
                for v in range(2):
                    b = 4 * v
                    vector.tensor_tensor(maps[b + 0], AP(A, v * 256 + 0, shp), wb(0), mult)
                    vector.tensor_tensor(maps[b + 1], AP(A, v * 256 + 128, shp), wb(2), mult)
                    vector.tensor_tensor(maps[b + 2], AP(A, v * 256 + 512, shp), wb(1), mult)
                    last_ins = vector.tensor_tensor(maps[b + 3], AP(A, v * 256 + 640, shp), wb(3), mult)
                vsync(last_ins)
                for v in range(2):
                    b = 4 * v
                    vector.tensor_add(maps[b + 0], maps[b + 0], maps[b + 1])
                    last_ins = vector.tensor_add(maps[b + 2], maps[b + 2], maps[b + 3])
                vsync(last_ins)
                osh2 = [[GPC * 256, 128], [256, GPC], [1, N]]
                vector.tensor_add(AP(accb[gc % 2], 0, osh2), maps[0], maps[2])
                vector.tensor_add(AP(accb[gc % 2], 128, osh2), maps[4], maps[6]) \
                    .then_inc(s_c, 1)

            for r in range(reps):
                for gi, g in enumerate(chunk_order):
                    for v in range(2):
                        zchunk(r, gi, g, v)
                for c in range(NC_):
                    combine(r, c)

    nc.compile()
    return nc


def _exact_label_fixup(label_g, theta, lab_f, out_bool):
    """Recompute voxels of |lab_f - 0.5| < FIX_EPS in the reference's exact
    f32 arithmetic order (validated bit-exact against the jax reference)."""
    eps = np.float32(FIX_EPS)
    cand = np.abs(lab_f - np.float32(0.5)) < eps
    if not cand.any():
        return out_bool
    bb, ii, jj, kk = np.nonzero(cand.reshape(-1, N, N, N))
    v = _exact_reference_values(label_g, theta, bb, ii, jj, kk)
    out_bool.reshape(-1, N, N, N)[bb, ii, jj, kk] = v > np.float32(0.5)
    return out_bool


def _exact_reference_values(vol_g, theta, bb, ii, jj, kk):
    """Reference-order f32 trilinear values at selected voxels.

    Replicates: grid einsum (x*t0 + y*t1 + z*t2, left-assoc f32) + t3; unnorm;
    8-corner accumulation in (z,y,x) order with w=(wz*wy)*wx, out += v*w.
    """
    f32 = np.float32
    t = np.arange(N, dtype=f32)
    xn = ((f32(2.0) * t + f32(1.0)) / f32(N) - f32(1.0)).astype(f32)
    th = theta.astype(f32)

    x = xn[ii]; y = xn[jj]; z = xn[kk]

    # f32 fma via f64 (exact up to negligible double-rounding corner cases)
    def fma32(a, b, c):
        return (np.float64(a) * np.float64(b) + c.astype(np.float64)).astype(f32)

    # grid components — XLA CPU lowers the einsum as an FMA chain (verified
    # bit-exact): fma(z, t2, fma(y, t1, x*t0)) + t3
    def comp(r):
        a = fma32(y, th[r, 1], (x * th[r, 0]).astype(f32))
        a = fma32(z, th[r, 2], a)
        return (a + th[r, 3]).astype(f32)
    gx, gy, gz = comp(0), comp(1), comp(2)

    def unnorm(c):
        return np.clip(((c + f32(1.0)) * f32(N) - f32(1.0)) * f32(0.5), f32(0.0), f32(N - 1))
    ux, uy, uz = unnorm(gx), unnorm(gy), unnorm(gz)
    x0 = np.floor(ux); y0 = np.floor(uy); z0 = np.floor(uz)
    fx = (ux - x0).astype(f32); fy = (uy - y0).astype(f32); fz = (uz - z0).astype(f32)
    x0i = x0.astype(np.int64); y0i = y0.astype(np.int64); z0i = z0.astype(np.int64)
    x1i = np.minimum(x0i + 1, N - 1); y1i = np.minimum(y0i + 1, N - 1); z1i = np.minimum(z0i + 1, N - 1)

    vol = vol_g.reshape(-1, N, N, N)
    out = np.zeros(bb.shape, f32)
    one = f32(1.0)
    for zi, wz in ((z0i, (one - fz).astype(f32)), (z1i, fz)):
        for yi, wy in ((y0i, (one - fy).astype(f32)), (y1i, fy)):
            for xi, wx in ((x0i, (one - fx).astype(f32)), (x1i, fx)):
                # inp[b, c, zi, yi, xi] in transposed space == vol[b, xi, yi, zi]
                vals = vol[bb, xi, yi, zi]
                w = ((wz * wy).astype(f32) * wx).astype(f32)
                out = (out + (vals * w).astype(f32)).astype(f32)
    return out


def _host_fallback(input_g, label_g, transform):
    """Arbitrary-transform fallback: full reference computation on host."""
    bb, ii, jj, kk = np.meshgrid(np.arange(8), np.arange(N), np.arange(N), np.arange(N), indexing="ij")
    bb, ii, jj, kk = (a.reshape(-1) for a in (bb, ii, jj, kk))
    theta = transform[:3].astype(np.float32)
    aug_inp = _exact_reference_values(input_g, theta, bb, ii, jj, kk).reshape(8, 1, N, N, N)
    lab = _exact_reference_values(label_g, theta, bb, ii, jj, kk).reshape(8, 1, N, N, N)
    return aug_inp.astype(np.float32), lab > np.float32(0.5)


def _make_inputs(tables, input_g, label_g):
    idx_p = _pack_idxs(tables["idxA"].reshape(-1))
    # packed k-replicated per-call weight tiles:
    # wtile[call, j, ci*GPC*128 + slot*128 + k] = w_ci(i = call*8 + slot, j)
    wt = np.empty((NCALLS, 128, 4, GPC, N), np.float16)
    for ci, nm in enumerate(("w00", "w01", "w10", "w11")):
        x = tables[nm].T.reshape(128, NCALLS, GPC).astype(np.float16)  # [j, call, slot]
        wt[:, :, ci] = x.transpose(1, 0, 2)[:, :, :, None]
    wtile = np.ascontiguousarray(wt.reshape(NCALLS, 128, 4 * GPC * N))
    fwrep = np.stack([np.tile(1.0 - tables["fw"], (128, 1)),
                      np.tile(tables["fw"], (128, 1))]).astype(np.float16)
    in_maps = []
    for b in range(8):
        in_maps.append({
            # partition-major: vol[p, gc*128+k] = volume[row=gc*128+p, k]
            "vol0": np.ascontiguousarray(
                input_g[b, 0].reshape(128, 128, N).astype(np.float16).transpose(1, 0, 2).reshape(128, NROWS)),
            "vol1": np.ascontiguousarray(
                label_g[b, 0].reshape(128, 128, N).astype(np.float16).transpose(1, 0, 2).reshape(128, NROWS)),
            "idxA": idx_p, "wtile": wtile, "fwrep": fwrep,
        })
    return in_maps


def kernel(input_g, label_g, transform):
    input_g = np.ascontiguousarray(input_g, dtype=np.float32)
    label_g = np.ascontiguousarray(label_g, dtype=np.float32)
    transform = np.asarray(transform, dtype=np.float32)
    theta = transform[:3]

    structured = (abs(float(theta[0, 2])) < 1e-12 and abs(float(theta[1, 2])) < 1e-12
                  and abs(float(theta[2, 0])) < 1e-12 and abs(float(theta[2, 1])) < 1e-12)
    if not structured:
        return _host_fallback(input_g, label_g, transform)

    from concourse.bass_utils import run_bass_kernel_spmd

    tables = _host_tables(theta)
    key = transform.tobytes()
    if key not in _CACHE:
        _CACHE[key] = _build_program(tables)
    nc = _CACHE[key]

    in_maps = _make_inputs(tables, input_g, label_g)
    res = run_bass_kernel_spmd(nc, in_maps, core_ids=list(range(8)))

    aug_inp = np.empty((8, 1, N, N, N), np.float32)
    lab_f = np.empty((8, 1, N, N, N), np.float32)
    for b in range(8):
        oi = res.results[b]["outI"]
        aug_inp[b, 0] = oi[:, 0:128].astype(np.float32).reshape(N, N, N)
        lab_f[b, 0] = oi[:, 128:256].astype(np.float32).reshape(N, N, N)

    out_bool = lab_f > np.float32(0.5)
    out_bool = _exact_label_fixup(label_g, theta, lab_f, out_bool)
    return aug_inp, out_bool


# revision 22
# speedup vs baseline: 1.2669x; 1.1166x over previous
"""Trainium2 Bass kernel for SegmentationAugmentation (3D affine grid_sample, trilinear, border).

Contract: kernel(input_g, label_g, transform) -> (aug_inp f32 [8,1,128,128,128],
                                                  aug_lab bool [8,1,128,128,128])

Math (swapaxes folded into index bookkeeping; all spatial dims 128):

  out[b,c,i,j,k] = trilinear sample of input_g[b,c,:,:,:] at
      p-axis: U(i,j) = clip(64*(a00*xn(i)+a01*xn(j)+a03)+63.5, 0, 127)
      q-axis: V(i,j) = clip(64*(a10*xn(i)+a11*xn(j)+a13)+63.5, 0, 127)
      r-axis: W(k)   = clip(64*(a22*xn(k)+a23)+63.5, 0, 127)
  with xn(t) = (2t+1)/128 - 1, theta = transform[:3].  Relies on the
  generator's z-rotation structure (theta[0:2,2]==0, theta[2,0:2]==0); a
  pure-host fallback handles arbitrary transforms.

Device pipeline, data parallel over batch (core b handles batch b; each core
processes BOTH its image and label volume in one fused f16 program):

  Phase 1 (dense): load pre-transposed f16 volume chunks, z-interp on DVE via
  run-segmented staircase slices (f16, 2x perf mode), then DMA the z-interped
  rows into an interleaved pair layout in DRAM:
     ZPI[r = p*128+q] = [Z0(p,q) | Z0(p+1,q) | Z1(p,q) | Z1(p+1,q)]  (1 KiB)
  (Z0 = image, Z1 = label; each row is 128 f16 k-values.)

  Phase 2 (gather): for each output point (i,j), ONE dma_gather descriptor of
  2 KiB at entry r=(p0*128+q0) fetches entries r,r+1 = all four bilinear
  corners of BOTH volumes.  DVE combines with k-replicated f16 weight tiles
  (streamed from DRAM per call so every operand keeps innermost stride 1 and
  2-byte dtype -> DVE 2x perf mode); one 512B-descriptor DMA per call writes
  the interleaved f16 outputs of both volumes.

Host converts the f16 outputs to f32 / bool; label voxels within FIX_EPS of
0.5 are recomputed in the reference's exact f32 arithmetic order.
"""
import numpy as np

N = 128
NROWS = N * N            # 16384 (p,q) rows per volume
NIDX = 1024              # gather indices (output points) per dma_gather call
GPC = NIDX // 128        # 8 element groups per partition per call
NCALLS = NROWS // NIDX   # 16 gather calls per rep
COLS = NIDX // 16        # idx table columns per call
ELEM = 1024              # gathered f16 elems per descriptor (= 2 ZPI entries)
ESTEP = 512              # f16 elems per ZPI entry (gather elem_step)
CH = 16                  # 128-row groups per load chunk
NGRP = NROWS // N // CH  # 8 chunks per volume
ZG = 129                 # zt groups (128 data + 1 zero pad)
FIX_EPS = 8e-3           # |label-0.5| below this -> exact host recompute

_CACHE = {}


def _mkap(pairs):
    import bass_rust
    return bass_rust.VecI64Pair([tuple(p) for p in pairs])


def _host_tables(theta):
    """All transform-derived tables, computed in float64 from f32 theta."""
    th = theta.astype(np.float64)
    t = np.arange(N, dtype=np.float64)
    xn = (2.0 * t + 1.0) / N - 1.0

    U = np.clip(64.0 * (th[0, 0] * xn[:, None] + th[0, 1] * xn[None, :] + th[0, 3]) + 63.5, 0.0, 127.0)
    V = np.clip(64.0 * (th[1, 0] * xn[:, None] + th[1, 1] * xn[None, :] + th[1, 3]) + 63.5, 0.0, 127.0)
    W = np.clip(64.0 * (th[2, 2] * xn + th[2, 3]) + 63.5, 0.0, 127.0)

    p0 = np.floor(U).astype(np.int64)
    q0 = np.floor(V).astype(np.int64)
    r0 = np.floor(W).astype(np.int64)
    fu = (U - p0).astype(np.float32)
    fv = (V - q0).astype(np.float32)
    fw = (W - r0).astype(np.float32)
    r1 = np.minimum(r0 + 1, N - 1)

    idxA = (p0 * 128 + q0).astype(np.int16)          # [i,j] ZPI entry index
    w00 = ((1 - fu) * (1 - fv)).astype(np.float32)
    w10 = (fu * (1 - fv)).astype(np.float32)
    w01 = ((1 - fu) * fv).astype(np.float32)
    w11 = (fu * fv).astype(np.float32)

    # z-run decomposition: maximal segments where both r0 and r1 step by a
    # constant d in {-1,0,1}
    runs = []
    k = 0
    while k < N:
        step = 0
        if k + 1 < N:
            d = int(r0[k + 1] - r0[k])
            if d == int(r1[k + 1] - r1[k]) and d in (-1, 0, 1):
                step = d
        ln = 1
        while (k + ln < N
               and int(r0[k + ln] - r0[k]) == step * ln
               and int(r1[k + ln] - r1[k]) == step * ln):
            ln += 1
        runs.append((k, ln, int(r0[k]), int(r1[k]), step))
        k += ln

    return dict(idxA=idxA, w00=w00, w01=w01, w10=w10, w11=w11, fw=fw, runs=runs)


def _pack_idxs(idx_flat):
    """int16 dma_gather index layout: element i at [i%16, i//16], replicated to 128 partitions."""
    t = idx_flat.reshape(-1, 16).T.astype(np.int16)  # [16, n/16]
    return np.ascontiguousarray(np.tile(t, (8, 1)))  # [128, n/16]


def _chunk_plan(tables):
    """Stream chunks (2048 ZPI entries each) in the order matching the calls'
    p-band progression; per call, how many streamed chunks it needs."""
    idxA = tables["idxA"].reshape(-1).astype(np.int64)
    # call c covers points c*NIDX..(c+1)*NIDX-1; entries r and r+1 needed
    need = []
    for c in range(NCALLS):
        rs = idxA[c * NIDX:(c + 1) * NIDX]
        need.append((int(rs.min()) // (CH * 128), (int(rs.max()) + 1) // (CH * 128)))
    first_lo, _ = need[0]
    last_lo, _ = need[-1]
    descending = first_lo >= last_lo
    order = list(range(NGRP - 1, -1, -1)) if descending else list(range(NGRP))
    pos = {g: i for i, g in enumerate(order)}
    nch = [max(pos[min(lo, NGRP - 1)], pos[min(hi, NGRP - 1)]) + 1 for lo, hi in need]
    return order, nch


def _build_program(tables, reps=1):
    """Raw-Bass (explicit semaphore) program; see module docstring for the
    pipeline.  All cross-engine waits are standalone wait_ge instructions.

    Engine streams:
      sync   (SP HWDGE):  const/volume loads, per-chunk ZPI stream writes
      scalar (ACT):       h1-slot entry assembly copies; weight-tile loads and
                          interleaved output writes (HWDGE)
      vector (DVE):       z-interp into ZS entry layout, 4-corner combine
      gpsimd (SWDGE):     one dma_gather per 1024 output points, fired as
                          soon as the chunks its points touch are streamed
    """
    import concourse.bass as bass
    from concourse import bacc, mybir

    runs = tables["runs"]
    f16 = mybir.dt.float16
    i16 = mybir.dt.int16

    nc = bacc.Bacc("TRN2", target_bir_lowering=False, debug=False, num_devices=8)

    vol_in = [nc.dram_tensor(f"vol{v}", [128, NROWS], f16, kind="ExternalInput") for v in range(2)]
    idx_dram = nc.dram_tensor("idxA", [128, NROWS // 16], i16, kind="ExternalInput")
    wtile = nc.dram_tensor("wtile", [NCALLS, 128, 4 * GPC * N], f16, kind="ExternalInput")
    fwrep = nc.dram_tensor("fwrep", [2, 128, 128], f16, kind="ExternalInput")
    out_i = nc.dram_tensor("outI", [NROWS, 256], f16, kind="ExternalOutput")
    zpi = nc.dram_tensor("zpi", [NROWS + 1, ESTEP], f16, kind="Internal")

    AP = bass.AP

    WD = 4 * GPC * N  # packed weight tile width (4096)
    idx_t = nc.alloc_sbuf_tensor("idx_t", [128, NROWS // 16], i16)
    fw_t = [nc.alloc_sbuf_tensor(f"fw{c}_t", [128, 128], f16) for c in range(2)]
    wt_sb = [nc.alloc_sbuf_tensor(f"wt_{s}", [128, WD], f16) for s in range(4)]
    vt16 = [nc.alloc_sbuf_tensor(f"vt16_{s}", [128, CH * N], f16) for s in range(4)]
    zs = [nc.alloc_sbuf_tensor(f"zs{s}", [128, CH * ESTEP], f16) for s in range(3)]
    ztmp = nc.alloc_sbuf_tensor("ztmp", [128, CH * N], f16)
    At = [nc.alloc_sbuf_tensor(f"At{s}", [128, GPC * ELEM], f16) for s in range(4)]
    mt = [nc.alloc_sbuf_tensor(f"m{s}", [128, GPC * N], f16) for s in range(8)]
    accb = [nc.alloc_sbuf_tensor(f"accb{s}", [128, GPC * 256], f16) for s in range(2)]

    nrows_ap = NROWS  # gather element at entry r reads entries r, r+1; r <= 16383
    NC_ = NCALLS
    ZSW = CH * ESTEP  # 8192
    chunk_order, nch = _chunk_plan(tables)
    # the h1 cross-chunk copy sources chunk g+1, which must already be in the
    # other ZS buffer -> chunks must stream top-down
    assert chunk_order == list(range(NGRP - 1, -1, -1)), chunk_order

    from contextlib import ExitStack
    with ExitStack() as _sctx:
        block = _sctx.enter_context(nc.Block())
        s_idx = _sctx.enter_context(nc.semaphore("s_idx"))
        s_wf = _sctx.enter_context(nc.semaphore("s_wf"))
        s_mz = _sctx.enter_context(nc.semaphore("s_mz"))
        s_l = [_sctx.enter_context(nc.semaphore(f"s_l{p}")) for p in range(4)]
        s_wl = _sctx.enter_context(nc.semaphore("s_wl"))
        s_z = _sctx.enter_context(nc.semaphore("s_z"))
        s_a = _sctx.enter_context(nc.semaphore("s_a"))
        s_zw = _sctx.enter_context(nc.semaphore("s_zw"))
        s_g = [_sctx.enter_context(nc.semaphore(f"s_g{p}")) for p in range(4)]
        s_c = _sctx.enter_context(nc.semaphore("s_c"))
        s_o = [_sctx.enter_context(nc.semaphore(f"s_o{p}")) for p in range(2)]
        s_v = _sctx.enter_context(nc.semaphore("s_v"))

        @block.sync
        def _(sync):
            def stream(r, gi):
                g = chunk_order[gi]
                gcg = r * 8 + gi
                if gi == 0 and r >= 1:
                    sync.wait_ge(s_c, NC_ * r)  # WAR zpi vs prev-rep gathers
                sync.wait_ge(s_a, gcg + 1)
                sync.dma_start(
                    AP(zpi, g * CH * 128 * ESTEP, [[ESTEP, 128], [128 * ESTEP, CH], [1, ESTEP]]),
                    AP(zs[gcg % 3], 0, [[ZSW, 128], [ESTEP, CH], [1, ESTEP]]),
                ).then_inc(s_zw, 16)

            for v in range(2):  # chunk 0 loads first: unblock DVE asap
                zc = v
                sync.dma_start(
                    AP(vt16[zc % 4], 0, [[CH * N, 128], [1, CH * N]]),
                    AP(vol_in[v], chunk_order[0] * CH * N, [[NROWS, 128], [1, CH * N]]),
                ).then_inc(s_l[zc % 4], 16)
            sync.dma_start(idx_t.ap(), idx_dram.ap()).then_inc(s_idx, 16)
            for c in range(2):
                sync.dma_start(fw_t[c].ap(), AP(fwrep, c * 128 * 128, [[128, 128], [1, 128]])).then_inc(s_wf, 16)
            # one-time: zero ZPI entry 16384 (read by gathers at r=16383)
            sync.wait_ge(s_mz, 1)
            sync.dma_start(
                AP(zpi, NROWS * ESTEP, [[128, 4], [1, 128]]),
                AP(mt[0], 0, [[GPC * N, 4], [1, 128]]),
            ).then_inc(s_zw, 16)
            for r in range(reps):
                for gi, g in enumerate(chunk_order):
                    for v in range(2):
                        zc = r * 16 + gi * 2 + v
                        if r == 0 and gi == 0:
                            continue  # preloaded above
                        if zc >= 4:
                            sync.wait_ge(s_z, zc - 3)  # WAR vt16 vs z-interp
                        sync.dma_start(
                            AP(vt16[zc % 4], 0, [[CH * N, 128], [1, CH * N]]),
                            AP(vol_in[v], g * CH * N, [[NROWS, 128], [1, CH * N]]),
                        ).then_inc(s_l[zc % 4], 16)
                    # stream lags the loads by two chunks so load issue is
                    # never blocked behind the s_a wait
                    if gi >= 2:
                        stream(r, gi - 2)
                stream(r, NGRP - 2)
                stream(r, NGRP - 1)
            sync.wait_ge(s_o[0], 16 * ((NC_ * reps + 1) // 2))
            sync.wait_ge(s_o[1], 16 * (NC_ * reps // 2))

        @block.scalar
        def _(scalar):
            def wt_load(gc):
                c = gc % NC_
                scalar.dma_start(
                    wt_sb[gc % 4].ap(),
                    AP(wtile, c * 128 * WD, [[WD, 128], [1, WD]]),
                ).then_inc(s_wl, 16)

            for gc in range(min(4, NC_ * reps)):
                wt_load(gc)
            for r in range(reps):
                for gi, g in enumerate(chunk_order):
                    # assemble h1 slots of chunk g: entry r gets row r+128,
                    # i.e. subgroup s copies from subgroup s+1 (h0 slot)
                    gcg = r * 8 + gi
                    scalar.wait_ge(s_z, r * 16 + 2 * gi + 2)
                    b = zs[gcg % 3]
                    for v in range(2):
                        scalar.copy(
                            AP(b, v * 256 + 128, [[ZSW, 128], [ESTEP, CH - 1], [1, N]]),
                            AP(b, ESTEP + v * 256, [[ZSW, 128], [ESTEP, CH - 1], [1, N]]),
                        )
                    last_ins = None
                    for v in range(2):
                        dst = AP(b, (CH - 1) * ESTEP + v * 256 + 128, [[ZSW, 128], [1, N]])
                        if g == NGRP - 1:
                            # top chunk: rows >= 16384 are zero (border clamp
                            # gives these corners zero weight; keep finite)
                            last_ins = scalar.memzero(dst)
                        else:
                            src_b = zs[(gcg - 1) % 3]
                            last_ins = scalar.copy(dst, AP(src_b, v * 256, [[ZSW, 128], [1, N]]))
                    last_ins.then_inc(s_a, 1)
                for c in range(NC_):
                    gc = r * NC_ + c
                    scalar.wait_ge(s_c, gc + 1)
                    if gc + 4 < NC_ * reps:
                        wt_load(gc + 4)
                    scalar.dma_start(
                        AP(out_i, c * NIDX * 256, [[256, 128], [128 * 256, GPC], [1, 256]]),
                        AP(accb[gc % 2], 0, [[GPC * 256, 128], [256, GPC], [1, 256]]),
                    ).then_inc(s_o[gc % 2], 16)

        @block.gpsimd
        def _(gpsimd):
            nreg = gpsimd.to_reg(NIDX)
            gpsimd.wait_ge(s_idx, 16)
            sv = AP(zpi, 0, [[ESTEP, nrows_ap], [1, ELEM]])
            for r in range(reps):
                for c in range(NC_):
                    gc = r * NC_ + c
                    gpsimd.wait_ge(s_zw, 16 + 128 * r + 16 * nch[c])
                    if gc >= 4:
                        gpsimd.wait_ge(s_c, gc - 3)  # WAR At vs combine
                    gpsimd.dma_gather(
                        AP(At[gc % 4], 0, [[GPC * ELEM, 128], [ELEM, GPC], [1, ELEM]]),
                        sv,
                        AP(idx_t, c * COLS, [[NROWS // 16, 128], [1, COLS]]),
                        NIDX, nreg, ELEM, elem_step=ESTEP,
                    ).then_inc(s_g[gc % 4], 16)

        @block.vector
        def _(vector):
            mult = mybir.AluOpType.mult
            VC = [0]

            def vsync(last_ins):
                # DVE pipeline does not interlock same-engine RAW hazards
                last_ins.then_inc(s_v, 1)
                VC[0] += 1
                vector.wait_ge(s_v, VC[0])

            vector.wait_ge(s_wf, 32)
            vector.memset(AP(mt[0], 0, [[GPC * N, 4], [1, 128]]), 0.0).then_inc(s_mz, 1)

            def zchunk(r, gi, g, v):
                zc = r * 16 + gi * 2 + v
                gcg = r * 8 + gi
                if zc >= 1:
                    vector.wait_ge(s_z, zc)  # WAR ztmp/pipeline drain
                vector.wait_ge(s_l[zc % 4], 16 * (zc // 4 + 1))
                if v == 0 and gcg >= 3:
                    # WAR zs[gcg%3] vs chunk gcg-3's stream + chunk gcg-2's
                    # h1 cross-read
                    vector.wait_ge(s_a, gcg - 1)
                    vector.wait_ge(s_zw, 16 + 16 * (gcg - 2))
                s = vt16[zc % 4]
                b = zs[gcg % 3]
                last_ins = None
                for (ks, ln, r0s, r1s, st) in runs:
                    zdst = AP(b, v * 256 + ks, [[ZSW, 128], [ESTEP, CH], [1, ln]])
                    tdst = AP(ztmp, ks, [[CH * N, 128], [N, CH], [1, ln]])
                    v0 = AP(s, r0s, [[CH * N, 128], [N, CH], [st, ln]])
                    v1 = AP(s, r1s, [[CH * N, 128], [N, CH], [st, ln]])
                    f0 = AP(fw_t[0], ks, [[128, 128], [0, CH], [1, ln]])
                    f1 = AP(fw_t[1], ks, [[128, 128], [0, CH], [1, ln]])
                    vector.tensor_tensor(zdst, v0, f0, mult)
                    last_ins = vector.tensor_tensor(tdst, v1, f1, mult)
                vsync(last_ins)
                for (ks, ln, r0s, r1s, st) in runs:
                    zdst = AP(b, v * 256 + ks, [[ZSW, 128], [ESTEP, CH], [1, ln]])
                    tdst = AP(ztmp, ks, [[CH * N, 128], [N, CH], [1, ln]])
                    last_ins = vector.tensor_add(zdst, zdst, tdst)
                last_ins.then_inc(s_z, 1)

            def combine(r, c):
                gc = r * NC_ + c
                if gc >= 1:
                    vector.wait_ge(s_c, gc)  # WAR mt vs prev combine
                vector.wait_ge(s_g[gc % 4], 16 * (gc // 4 + 1))
                vector.wait_ge(s_wl, 16 * (gc + 1))
                if gc >= 2:
                    vector.wait_ge(s_o[gc % 2], 16 * (gc // 2))  # WAR accb
                A = At[gc % 4]
                W = wt_sb[gc % 4]
                shp = [[GPC * ELEM, 128], [ELEM, GPC], [1, N]]
                oshp = [[GPC * N, 128], [N, GPC], [1, N]]

                def wb(ci):
                    return AP(W, ci * GPC * N, [[WD, 128], [N, GPC], [1, N]])
                maps = [AP(m, 0, oshp) for m in mt]
                # corner offsets within a gathered element (f16 elems):
                #   vol v: (p0,q0)=v*256, (p1,q0)=v*256+128,
                #          (p0,q1)=v*256+512, (p1,q1)=v*256+640
                last_ins = None
                for v in range(2):
                    b = 4 * v
                    vector.tensor_tensor(maps[b + 0], AP(A, v * 256 + 0, shp), wb(0), mult)
                    vector.tensor_tensor(maps[b + 1], AP(A, v * 256 + 128, shp), wb(2), mult)
                    vector.tensor_tensor(maps[b + 2], AP(A, v * 256 + 512, shp), wb(1), mult)
                    last_ins = vector.tensor_tensor(maps[b + 3], AP(A, v * 256 + 640, shp), wb(3), mult)
                vsync(last_ins)
                for v in range(2):
                    b = 4 * v
                    vector.tensor_add(maps[b + 0], maps[b + 0], maps[b + 1])
                    last_ins = vector.tensor_add(maps[b + 2], maps[b + 2], maps[b + 3])
                vsync(last_ins)
                osh2 = [[GPC * 256, 128], [256, GPC], [1, N]]
                vector.tensor_add(AP(accb[gc % 2], 0, osh2), maps[0], maps[2])
                vector.tensor_add(AP(accb[gc % 2], 128, osh2), maps[4], maps[6]) \
                    .then_inc(s_c, 1)

            # interleave the first combines into the z tail: their gathers
            # land while later chunks are still z-interping
            ileave = {(NGRP - 2, 1): [0], (NGRP - 1, 0): [1], (NGRP - 1, 1): [2]}
            for r in range(reps):
                for gi, g in enumerate(chunk_order):
                    for v in range(2):
                        zchunk(r, gi, g, v)
                        for c in ileave.get((gi, v), []):
                            combine(r, c)
                for c in range(3, NC_):
                    combine(r, c)

    nc.compile()
    return nc


def _exact_label_fixup(label_g, theta, lab_f, out_bool):
    """Recompute voxels of |lab_f - 0.5| < FIX_EPS in the reference's exact
    f32 arithmetic order (validated bit-exact against the jax reference)."""
    eps = np.float32(FIX_EPS)
    cand = np.abs(lab_f - np.float32(0.5)) < eps
    if not cand.any():
        return out_bool
    bb, ii, jj, kk = np.nonzero(cand.reshape(-1, N, N, N))
    v = _exact_reference_values(label_g, theta, bb, ii, jj, kk)
    out_bool.reshape(-1, N, N, N)[bb, ii, jj, kk] = v > np.float32(0.5)
    return out_bool


def _exact_reference_values(vol_g, theta, bb, ii, jj, kk):
    """Reference-order f32 trilinear values at selected voxels.

    Replicates: grid einsum (x*t0 + y*t1 + z*t2, left-assoc f32) + t3; unnorm;
    8-corner accumulation in (z,y,x) order with w=(wz*wy)*wx, out += v*w.
    """
    f32 = np.float32
    t = np.arange(N, dtype=f32)
    xn = ((f32(2.0) * t + f32(1.0)) / f32(N) - f32(1.0)).astype(f32)
    th = theta.astype(f32)

    x = xn[ii]; y = xn[jj]; z = xn[kk]

    # f32 fma via f64 (exact up to negligible double-rounding corner cases)
    def fma32(a, b, c):
        return (np.float64(a) * np.float64(b) + c.astype(np.float64)).astype(f32)

    # grid components — XLA CPU lowers the einsum as an FMA chain (verified
    # bit-exact): fma(z, t2, fma(y, t1, x*t0)) + t3
    def comp(r):
        a = fma32(y, th[r, 1], (x * th[r, 0]).astype(f32))
        a = fma32(z, th[r, 2], a)
        return (a + th[r, 3]).astype(f32)
    gx, gy, gz = comp(0), comp(1), comp(2)

    def unnorm(c):
        return np.clip(((c + f32(1.0)) * f32(N) - f32(1.0)) * f32(0.5), f32(0.0), f32(N - 1))
    ux, uy, uz = unnorm(gx), unnorm(gy), unnorm(gz)
    x0 = np.floor(ux); y0 = np.floor(uy); z0 = np.floor(uz)
    fx = (ux - x0).astype(f32); fy = (uy - y0).astype(f32); fz = (uz - z0).astype(f32)
    x0i = x0.astype(np.int64); y0i = y0.astype(np.int64); z0i = z0.astype(np.int64)
    x1i = np.minimum(x0i + 1, N - 1); y1i = np.minimum(y0i + 1, N - 1); z1i = np.minimum(z0i + 1, N - 1)

    vol = vol_g.reshape(-1, N, N, N)
    out = np.zeros(bb.shape, f32)
    one = f32(1.0)
    for zi, wz in ((z0i, (one - fz).astype(f32)), (z1i, fz)):
        for yi, wy in ((y0i, (one - fy).astype(f32)), (y1i, fy)):
            for xi, wx in ((x0i, (one - fx).astype(f32)), (x1i, fx)):
                # inp[b, c, zi, yi, xi] in transposed space == vol[b, xi, yi, zi]
                vals = vol[bb, xi, yi, zi]
                w = ((wz * wy).astype(f32) * wx).astype(f32)
                out = (out + (vals * w).astype(f32)).astype(f32)
    return out


def _host_fallback(input_g, label_g, transform):
    """Arbitrary-transform fallback: full reference computation on host."""
    bb, ii, jj, kk = np.meshgrid(np.arange(8), np.arange(N), np.arange(N), np.arange(N), indexing="ij")
    bb, ii, jj, kk = (a.reshape(-1) for a in (bb, ii, jj, kk))
    theta = transform[:3].astype(np.float32)
    aug_inp = _exact_reference_values(input_g, theta, bb, ii, jj, kk).reshape(8, 1, N, N, N)
    lab = _exact_reference_values(label_g, theta, bb, ii, jj, kk).reshape(8, 1, N, N, N)
    return aug_inp.astype(np.float32), lab > np.float32(0.5)


def _make_inputs(tables, input_g, label_g):
    idx_p = _pack_idxs(tables["idxA"].reshape(-1))
    # packed k-replicated per-call weight tiles:
    # wtile[call, j, ci*GPC*128 + slot*128 + k] = w_ci(i = call*8 + slot, j)
    wt = np.empty((NCALLS, 128, 4, GPC, N), np.float16)
    for ci, nm in enumerate(("w00", "w01", "w10", "w11")):
        x = tables[nm].T.reshape(128, NCALLS, GPC).astype(np.float16)  # [j, call, slot]
        wt[:, :, ci] = x.transpose(1, 0, 2)[:, :, :, None]
    wtile = np.ascontiguousarray(wt.reshape(NCALLS, 128, 4 * GPC * N))
    fwrep = np.stack([np.tile(1.0 - tables["fw"], (128, 1)),
                      np.tile(tables["fw"], (128, 1))]).astype(np.float16)
    in_maps = []
    for b in range(8):
        in_maps.append({
            # partition-major: vol[p, gc*128+k] = volume[row=gc*128+p, k]
            "vol0": np.ascontiguousarray(
                input_g[b, 0].reshape(128, 128, N).astype(np.float16).transpose(1, 0, 2).reshape(128, NROWS)),
            "vol1": np.ascontiguousarray(
                label_g[b, 0].reshape(128, 128, N).astype(np.float16).transpose(1, 0, 2).reshape(128, NROWS)),
            "idxA": idx_p, "wtile": wtile, "fwrep": fwrep,
        })
    return in_maps


def kernel(input_g, label_g, transform):
    input_g = np.ascontiguousarray(input_g, dtype=np.float32)
    label_g = np.ascontiguousarray(label_g, dtype=np.float32)
    transform = np.asarray(transform, dtype=np.float32)
    theta = transform[:3]

    structured = (abs(float(theta[0, 2])) < 1e-12 and abs(float(theta[1, 2])) < 1e-12
                  and abs(float(theta[2, 0])) < 1e-12 and abs(float(theta[2, 1])) < 1e-12)
    if not structured:
        return _host_fallback(input_g, label_g, transform)

    from concourse.bass_utils import run_bass_kernel_spmd

    tables = _host_tables(theta)
    key = transform.tobytes()
    if key not in _CACHE:
        _CACHE[key] = _build_program(tables)
    nc = _CACHE[key]

    in_maps = _make_inputs(tables, input_g, label_g)
    res = run_bass_kernel_spmd(nc, in_maps, core_ids=list(range(8)))

    aug_inp = np.empty((8, 1, N, N, N), np.float32)
    lab_f = np.empty((8, 1, N, N, N), np.float32)
    for b in range(8):
        oi = res.results[b]["outI"]
        aug_inp[b, 0] = oi[:, 0:128].astype(np.float32).reshape(N, N, N)
        lab_f[b, 0] = oi[:, 128:256].astype(np.float32).reshape(N, N, N)

    out_bool = lab_f > np.float32(0.5)
    out_bool = _exact_label_fixup(label_g, theta, lab_f, out_bool)
    return aug_inp, out_bool


# revision 34
# speedup vs baseline: 1.3671x; 1.0791x over previous
"""Trainium2 Bass kernel for SegmentationAugmentation (3D affine grid_sample, trilinear, border).

Contract: kernel(input_g, label_g, transform) -> (aug_inp f32 [8,1,128,128,128],
                                                  aug_lab bool [8,1,128,128,128])

Math (swapaxes folded into index bookkeeping; all spatial dims 128):

  out[b,c,i,j,k] = trilinear sample of input_g[b,c,:,:,:] at
      p-axis: U(i,j) = clip(64*(a00*xn(i)+a01*xn(j)+a03)+63.5, 0, 127)
      q-axis: V(i,j) = clip(64*(a10*xn(i)+a11*xn(j)+a13)+63.5, 0, 127)
      r-axis: W(k)   = clip(64*(a22*xn(k)+a23)+63.5, 0, 127)
  with xn(t) = (2t+1)/128 - 1, theta = transform[:3].  Relies on the
  generator's z-rotation structure (theta[0:2,2]==0, theta[2,0:2]==0); a
  pure-host fallback handles arbitrary transforms.

Device pipeline, data parallel over batch (core b handles batch b; each core
processes BOTH its image and label volume in one fused f16 program):

  Phase 1 (dense): load pre-transposed f16 volume chunks, z-interp on DVE via
  run-segmented staircase slices (f16, 2x perf mode), then DMA the z-interped
  rows into an interleaved pair layout in DRAM:
     ZPI[r = p*128+q] = [Z0(p,q) | Z0(p+1,q) | Z1(p,q) | Z1(p+1,q)]  (1 KiB)
  (Z0 = image, Z1 = label; each row is 128 f16 k-values.)

  Phase 2 (gather): for each output point (i,j), ONE dma_gather descriptor of
  2 KiB at entry r=(p0*128+q0) fetches entries r,r+1 = all four bilinear
  corners of BOTH volumes.  DVE combines with k-replicated f16 weight tiles
  (streamed from DRAM per call so every operand keeps innermost stride 1 and
  2-byte dtype -> DVE 2x perf mode); one 512B-descriptor DMA per call writes
  the interleaved f16 outputs of both volumes.

Host converts the f16 outputs to f32 / bool; label voxels within FIX_EPS of
0.5 are recomputed in the reference's exact f32 arithmetic order.
"""
import numpy as np

N = 128
NROWS = N * N            # 16384 (p,q) rows per volume
NIDX = 1024              # gather indices (output points) per dma_gather call
GPC = NIDX // 128        # 8 element groups per partition per call
NCALLS = NROWS // NIDX   # 16 gather calls per rep
COLS = NIDX // 16        # idx table columns per call
ELEM = 1024              # gathered f16 elems per descriptor (= 2 ZPI entries)
ESTEP = 512              # f16 elems per ZPI entry (gather elem_step)
CH = 16                  # 128-row groups per load chunk
NGRP = NROWS // N // CH  # 8 chunks per volume
ZG = 129                 # zt groups (128 data + 1 zero pad)
FIX_EPS = 8e-3           # |label-0.5| below this -> exact host recompute

_CACHE = {}


def _mkap(pairs):
    import bass_rust
    return bass_rust.VecI64Pair([tuple(p) for p in pairs])


def _host_tables(theta):
    """All transform-derived tables, computed in float64 from f32 theta."""
    th = theta.astype(np.float64)
    t = np.arange(N, dtype=np.float64)
    xn = (2.0 * t + 1.0) / N - 1.0

    U = np.clip(64.0 * (th[0, 0] * xn[:, None] + th[0, 1] * xn[None, :] + th[0, 3]) + 63.5, 0.0, 127.0)
    V = np.clip(64.0 * (th[1, 0] * xn[:, None] + th[1, 1] * xn[None, :] + th[1, 3]) + 63.5, 0.0, 127.0)
    W = np.clip(64.0 * (th[2, 2] * xn + th[2, 3]) + 63.5, 0.0, 127.0)

    p0 = np.floor(U).astype(np.int64)
    q0 = np.floor(V).astype(np.int64)
    r0 = np.floor(W).astype(np.int64)
    fu = (U - p0).astype(np.float32)
    fv = (V - q0).astype(np.float32)
    fw = (W - r0).astype(np.float32)
    r1 = np.minimum(r0 + 1, N - 1)

    idxA = (p0 * 128 + q0).astype(np.int16)          # [i,j] ZPI entry index
    w00 = ((1 - fu) * (1 - fv)).astype(np.float32)
    w10 = (fu * (1 - fv)).astype(np.float32)
    w01 = ((1 - fu) * fv).astype(np.float32)
    w11 = (fu * fv).astype(np.float32)

    # z-run decomposition: maximal segments where both r0 and r1 step by a
    # constant d in {-1,0,1}
    runs = []
    k = 0
    while k < N:
        step = 0
        if k + 1 < N:
            d = int(r0[k + 1] - r0[k])
            if d == int(r1[k + 1] - r1[k]) and d in (-1, 0, 1):
                step = d
        ln = 1
        while (k + ln < N
               and int(r0[k + ln] - r0[k]) == step * ln
               and int(r1[k + ln] - r1[k]) == step * ln):
            ln += 1
        runs.append((k, ln, int(r0[k]), int(r1[k]), step))
        k += ln

    return dict(idxA=idxA, w00=w00, w01=w01, w10=w10, w11=w11, fw=fw, runs=runs)


def _pack_idxs(idx_flat):
    """int16 dma_gather index layout: element i at [i%16, i//16], replicated to 128 partitions."""
    t = idx_flat.reshape(-1, 16).T.astype(np.int16)  # [16, n/16]
    return np.ascontiguousarray(np.tile(t, (8, 1)))  # [128, n/16]


def _chunk_plan(tables):
    """Stream chunks (2048 ZPI entries each) in the order matching the calls'
    p-band progression; per call, how many streamed chunks it needs."""
    idxA = tables["idxA"].reshape(-1).astype(np.int64)
    # call c covers points c*NIDX..(c+1)*NIDX-1; entries r and r+1 needed
    need = []
    for c in range(NCALLS):
        rs = idxA[c * NIDX:(c + 1) * NIDX]
        need.append((int(rs.min()) // (CH * 128), (int(rs.max()) + 1) // (CH * 128)))
    first_lo, _ = need[0]
    last_lo, _ = need[-1]
    descending = first_lo >= last_lo
    order = list(range(NGRP - 1, -1, -1)) if descending else list(range(NGRP))
    pos = {g: i for i, g in enumerate(order)}
    nch = [max(pos[min(lo, NGRP - 1)], pos[min(hi, NGRP - 1)]) + 1 for lo, hi in need]
    return order, nch


def _build_program(tables, reps=1):
    """Raw-Bass (explicit semaphore) program; see module docstring for the
    pipeline.  All cross-engine waits are standalone wait_ge instructions.

    Engine streams:
      sync   (SP HWDGE):  const/volume loads, per-chunk ZPI stream writes
      scalar (ACT):       h1-slot entry assembly copies; weight-tile loads and
                          interleaved output writes (HWDGE)
      vector (DVE):       z-interp into ZS entry layout, 4-corner combine
      gpsimd (SWDGE):     one dma_gather per 1024 output points, fired as
                          soon as the chunks its points touch are streamed
    """
    import concourse.bass as bass
    from concourse import bacc, mybir

    runs = tables["runs"]
    f16 = mybir.dt.float16
    i16 = mybir.dt.int16

    nc = bacc.Bacc("TRN2", target_bir_lowering=False, debug=False, num_devices=8)

    vol_in = [nc.dram_tensor(f"vol{v}", [128, NROWS], f16, kind="ExternalInput") for v in range(2)]
    idx_dram = nc.dram_tensor("idxA", [128, NROWS // 16], i16, kind="ExternalInput")
    wtile = nc.dram_tensor("wtile", [NCALLS, 128, 4 * GPC * N], f16, kind="ExternalInput")
    fwrep = nc.dram_tensor("fwrep", [2, 128, 128], f16, kind="ExternalInput")
    out_i = nc.dram_tensor("outI", [NROWS, 256], f16, kind="ExternalOutput")
    zpi = nc.dram_tensor("zpi", [NROWS + 1, ESTEP], f16, kind="Internal")

    AP = bass.AP

    WD = 4 * GPC * N  # packed weight tile width (4096)
    idx_t = nc.alloc_sbuf_tensor("idx_t", [128, NROWS // 16], i16)
    fw_t = [nc.alloc_sbuf_tensor(f"fw{c}_t", [128, 128], f16) for c in range(2)]
    wt_sb = [nc.alloc_sbuf_tensor(f"wt_{s}", [128, WD], f16) for s in range(4)]
    vt16 = [nc.alloc_sbuf_tensor(f"vt16_{s}", [128, CH * N], f16) for s in range(4)]
    zs = [nc.alloc_sbuf_tensor(f"zs{s}", [128, CH * ESTEP], f16) for s in range(3)]
    ztmp = nc.alloc_sbuf_tensor("ztmp", [128, CH * N], f16)
    At = [nc.alloc_sbuf_tensor(f"At{s}", [128, GPC * ELEM], f16) for s in range(4)]
    mt = [nc.alloc_sbuf_tensor(f"m{s}", [128, GPC * N], f16) for s in range(8)]
    accb = [nc.alloc_sbuf_tensor(f"accb{s}", [128, GPC * 256], f16) for s in range(4)]

    nrows_ap = NROWS  # gather element at entry r reads entries r, r+1; r <= 16383
    NC_ = NCALLS
    ZSW = CH * ESTEP  # 8192
    chunk_order, nch = _chunk_plan(tables)
    # the h1 cross-chunk copy sources chunk g+1, which must already be in the
    # other ZS buffer -> chunks must stream top-down
    assert chunk_order == list(range(NGRP - 1, -1, -1)), chunk_order

    from contextlib import ExitStack
    with ExitStack() as _sctx:
        block = _sctx.enter_context(nc.Block())
        s_idx = _sctx.enter_context(nc.semaphore("s_idx"))
        s_wf = _sctx.enter_context(nc.semaphore("s_wf"))
        s_mz = _sctx.enter_context(nc.semaphore("s_mz"))
        s_l = [_sctx.enter_context(nc.semaphore(f"s_l{p}")) for p in range(4)]
        s_wl = [_sctx.enter_context(nc.semaphore(f"s_wl{p}")) for p in range(4)]
        s_z = _sctx.enter_context(nc.semaphore("s_z"))
        s_a = _sctx.enter_context(nc.semaphore("s_a"))
        s_zw = [_sctx.enter_context(nc.semaphore(f"s_zw{p}")) for p in range(NGRP)]
        s_zz = _sctx.enter_context(nc.semaphore("s_zz"))
        s_g = [_sctx.enter_context(nc.semaphore(f"s_g{p}")) for p in range(4)]
        s_c = _sctx.enter_context(nc.semaphore("s_c"))
        s_o = [_sctx.enter_context(nc.semaphore(f"s_o{p}")) for p in range(4)]
        s_v = _sctx.enter_context(nc.semaphore("s_v"))

        @block.sync
        def _(sync):
            for v in range(2):  # chunk 0 loads first: unblock DVE asap
                sync.dma_start(
                    AP(vt16[v], 0, [[CH * N, 128], [1, CH * N]]),
                    AP(vol_in[v], chunk_order[0] * CH * N, [[NROWS, 128], [1, CH * N]]),
                ).then_inc(s_l[v], 16)
            sync.dma_start(idx_t.ap(), idx_dram.ap()).then_inc(s_idx, 16)
            for c in range(2):
                sync.dma_start(fw_t[c].ap(), AP(fwrep, c * 128 * 128, [[128, 128], [1, 128]])).then_inc(s_wf, 16)
            # one-time: zero ZPI entry 16384 (read by gathers at r=16383)
            sync.wait_ge(s_mz, 1)
            sync.dma_start(
                AP(zpi, NROWS * ESTEP, [[128, 4], [1, 128]]),
                AP(mt[0], 0, [[GPC * N, 4], [1, 128]]),
            ).then_inc(s_zz, 16)
            for r in range(reps):
                for gi, g in enumerate(chunk_order):
                    for v in range(2):
                        zc = r * 16 + gi * 2 + v
                        if r == 0 and gi == 0:
                            continue  # preloaded above
                        if zc >= 4:
                            sync.wait_ge(s_z, zc - 3)  # WAR vt16 vs z-interp
                        sync.dma_start(
                            AP(vt16[zc % 4], 0, [[CH * N, 128], [1, CH * N]]),
                            AP(vol_in[v], g * CH * N, [[NROWS, 128], [1, CH * N]]),
                        ).then_inc(s_l[zc % 4], 16)
            for p in range(4):
                sync.wait_ge(s_o[p], 16 * ((NC_ * reps - p + 3) // 4))

        @block.scalar
        def _(scalar):
            def wt_load(gc):
                c = gc % NC_
                scalar.dma_start(
                    wt_sb[gc % 4].ap(),
                    AP(wtile, c * 128 * WD, [[WD, 128], [1, WD]]),
                ).then_inc(s_wl[gc % 4], 16)

            for gc in range(min(4, NC_ * reps)):
                wt_load(gc)
            for r in range(reps):
                for gi, g in enumerate(chunk_order):
                    # assemble h1 slots of chunk g: entry r gets row r+128,
                    # i.e. subgroup s copies from subgroup s+1 (h0 slot)
                    gcg = r * 8 + gi
                    scalar.wait_ge(s_z, r * 16 + 2 * gi + 2)
                    b = zs[gcg % 3]
                    for v in range(2):
                        scalar.copy(
                            AP(b, v * 256 + 128, [[ZSW, 128], [ESTEP, CH - 1], [1, N]]),
                            AP(b, ESTEP + v * 256, [[ZSW, 128], [ESTEP, CH - 1], [1, N]]),
                        )
                    last_ins = None
                    for v in range(2):
                        dst = AP(b, (CH - 1) * ESTEP + v * 256 + 128, [[ZSW, 128], [1, N]])
                        if g == NGRP - 1:
                            # top chunk: rows >= 16384 are zero (border clamp
                            # gives these corners zero weight; keep finite)
                            last_ins = scalar.memzero(dst)
                        else:
                            src_b = zs[(gcg - 1) % 3]
                            last_ins = scalar.copy(dst, AP(src_b, v * 256, [[ZSW, 128], [1, N]]))
                    last_ins.then_inc(s_a, 1)
                    # stream this chunk's assembled entries to ZPI; the ACT
                    # sequencer runs ahead of the engine pipeline, so fully
                    # drain the copies before the DMA reads the buffer
                    scalar.drain()
                    scalar.wait_ge(s_a, gcg + 1)
                    if gi == 0 and r >= 1:
                        scalar.wait_ge(s_c, NC_ * r)  # WAR zpi vs prev-rep gathers
                    scalar.dma_start(
                        AP(zpi, g * CH * 128 * ESTEP, [[ESTEP, 128], [128 * ESTEP, CH], [1, ESTEP]]),
                        AP(b, 0, [[ZSW, 128], [ESTEP, CH], [1, ESTEP]]),
                    ).then_inc(s_zw[gi], 16)
                for c in range(NC_):
                    gc = r * NC_ + c
                    scalar.wait_ge(s_c, gc + 1)
                    scalar.dma_start(
                        AP(out_i, c * NIDX * 256, [[256, 128], [128 * 256, GPC], [1, 256]]),
                        AP(accb[gc % 4], 0, [[GPC * 256, 128], [256, GPC], [1, 256]]),
                    ).then_inc(s_o[gc % 4], 16)
                    if gc + 4 < NC_ * reps:
                        wt_load(gc + 4)

        @block.gpsimd
        def _(gpsimd):
            nreg = gpsimd.to_reg(NIDX)
            gpsimd.wait_ge(s_idx, 16)
            sv = AP(zpi, 0, [[ESTEP, nrows_ap], [1, ELEM]])
            for r in range(reps):
                for c in range(NC_):
                    gc = r * NC_ + c
                    if gc == 0:
                        gpsimd.wait_ge(s_zz, 16)
                    for pos in range(nch[c]):
                        gpsimd.wait_ge(s_zw[pos], 16 * (r + 1))
                    if gc >= 4:
                        gpsimd.wait_ge(s_c, gc - 3)  # WAR At vs combine
                    gpsimd.dma_gather(
                        AP(At[gc % 4], 0, [[GPC * ELEM, 128], [ELEM, GPC], [1, ELEM]]),
                        sv,
                        AP(idx_t, c * COLS, [[NROWS // 16, 128], [1, COLS]]),
                        NIDX, nreg, ELEM, elem_step=ESTEP,
                    ).then_inc(s_g[gc % 4], 16)

        @block.vector
        def _(vector):
            mult = mybir.AluOpType.mult
            VC = [0]

            def vsync(last_ins):
                # DVE pipeline does not interlock same-engine RAW hazards
                last_ins.then_inc(s_v, 1)
                VC[0] += 1
                vector.wait_ge(s_v, VC[0])

            vector.wait_ge(s_wf, 32)
            vector.memset(AP(mt[0], 0, [[GPC * N, 4], [1, 128]]), 0.0).then_inc(s_mz, 1)

            def zchunk(r, gi, g, v):
                zc = r * 16 + gi * 2 + v
                gcg = r * 8 + gi
                if zc >= 1:
                    vector.wait_ge(s_z, zc)  # WAR ztmp/pipeline drain
                vector.wait_ge(s_l[zc % 4], 16 * (zc // 4 + 1))
                if v == 0 and gcg >= 3:
                    # WAR zs[gcg%3] vs chunk gcg-3's stream + chunk gcg-2's
                    # h1 cross-read
                    vector.wait_ge(s_a, gcg - 1)
                    vector.wait_ge(s_zw[(gcg - 3) % NGRP], 16 * ((gcg - 3) // NGRP + 1))
                s = vt16[zc % 4]
                b = zs[gcg % 3]
                last_ins = None
                for (ks, ln, r0s, r1s, st) in runs:
                    zdst = AP(b, v * 256 + ks, [[ZSW, 128], [ESTEP, CH], [1, ln]])
                    tdst = AP(ztmp, ks, [[CH * N, 128], [N, CH], [1, ln]])
                    v0 = AP(s, r0s, [[CH * N, 128], [N, CH], [st, ln]])
                    v1 = AP(s, r1s, [[CH * N, 128], [N, CH], [st, ln]])
                    f0 = AP(fw_t[0], ks, [[128, 128], [0, CH], [1, ln]])
                    f1 = AP(fw_t[1], ks, [[128, 128], [0, CH], [1, ln]])
                    vector.tensor_tensor(zdst, v0, f0, mult)
                    last_ins = vector.tensor_tensor(tdst, v1, f1, mult)
                vsync(last_ins)
                for (ks, ln, r0s, r1s, st) in runs:
                    zdst = AP(b, v * 256 + ks, [[ZSW, 128], [ESTEP, CH], [1, ln]])
                    tdst = AP(ztmp, ks, [[CH * N, 128], [N, CH], [1, ln]])
                    last_ins = vector.tensor_add(zdst, zdst, tdst)
                last_ins.then_inc(s_z, 1)

            def combine(r, c):
                gc = r * NC_ + c
                if gc >= 1:
                    vector.wait_ge(s_c, gc)  # WAR mt vs prev combine
                vector.wait_ge(s_g[gc % 4], 16 * (gc // 4 + 1))
                vector.wait_ge(s_wl[gc % 4], 16 * (gc // 4 + 1))
                if gc >= 4:
                    vector.wait_ge(s_o[gc % 4], 16 * (gc // 4))  # WAR accb
                A = At[gc % 4]
                W = wt_sb[gc % 4]
                shp = [[GPC * ELEM, 128], [ELEM, GPC], [1, N]]
                oshp = [[GPC * N, 128], [N, GPC], [1, N]]

                def wb(ci):
                    return AP(W, ci * GPC * N, [[WD, 128], [N, GPC], [1, N]])
                maps = [AP(m, 0, oshp) for m in mt]
                # corner offsets within a gathered element (f16 elems):
                #   vol v: (p0,q0)=v*256, (p1,q0)=v*256+128,
                #          (p0,q1)=v*256+512, (p1,q1)=v*256+640
                last_ins = None
                for v in range(2):
                    b = 4 * v
                    vector.tensor_tensor(maps[b + 0], AP(A, v * 256 + 0, shp), wb(0), mult)
                    vector.tensor_tensor(maps[b + 1], AP(A, v * 256 + 128, shp), wb(2), mult)
                    vector.tensor_tensor(maps[b + 2], AP(A, v * 256 + 512, shp), wb(1), mult)
                    last_ins = vector.tensor_tensor(maps[b + 3], AP(A, v * 256 + 640, shp), wb(3), mult)
                vsync(last_ins)
                for v in range(2):
                    b = 4 * v
                    vector.tensor_add(maps[b + 0], maps[b + 0], maps[b + 1])
                    last_ins = vector.tensor_add(maps[b + 2], maps[b + 2], maps[b + 3])
                vsync(last_ins)
                osh2 = [[GPC * 256, 128], [256, GPC], [1, N]]
                vector.tensor_add(AP(accb[gc % 4], 0, osh2), maps[0], maps[2])
                vector.tensor_add(AP(accb[gc % 4], 128, osh2), maps[4], maps[6]) \
                    .then_inc(s_c, 1)

            # interleave the first combines into the z tail: their gathers
            # land while later chunks are still z-interping
            ileave = {(NGRP - 2, 1): [0], (NGRP - 1, 0): [1]}
            for r in range(reps):
                for gi, g in enumerate(chunk_order):
                    for v in range(2):
                        zchunk(r, gi, g, v)
                        for c in ileave.get((gi, v), []):
                            combine(r, c)
                for c in range(2, NC_):
                    combine(r, c)

    nc.compile()
    return nc


def _exact_label_fixup(label_g, theta, lab_f, out_bool):
    """Recompute voxels of |lab_f - 0.5| < FIX_EPS in the reference's exact
    f32 arithmetic order (validated bit-exact against the jax reference)."""
    eps = np.float32(FIX_EPS)
    cand = np.abs(lab_f - np.float32(0.5)) < eps
    if not cand.any():
        return out_bool
    bb, ii, jj, kk = np.nonzero(cand.reshape(-1, N, N, N))
    v = _exact_reference_values(label_g, theta, bb, ii, jj, kk)
    out_bool.reshape(-1, N, N, N)[bb, ii, jj, kk] = v > np.float32(0.5)
    return out_bool


def _exact_reference_values(vol_g, theta, bb, ii, jj, kk):
    """Reference-order f32 trilinear values at selected voxels.

    Replicates: grid einsum (x*t0 + y*t1 + z*t2, left-assoc f32) + t3; unnorm;
    8-corner accumulation in (z,y,x) order with w=(wz*wy)*wx, out += v*w.
    """
    f32 = np.float32
    t = np.arange(N, dtype=f32)
    xn = ((f32(2.0) * t + f32(1.0)) / f32(N) - f32(1.0)).astype(f32)
    th = theta.astype(f32)

    x = xn[ii]; y = xn[jj]; z = xn[kk]

    # f32 fma via f64 (exact up to negligible double-rounding corner cases)
    def fma32(a, b, c):
        return (np.float64(a) * np.float64(b) + c.astype(np.float64)).astype(f32)

    # grid components — XLA CPU lowers the einsum as an FMA chain (verified
    # bit-exact): fma(z, t2, fma(y, t1, x*t0)) + t3
    def comp(r):
        a = fma32(y, th[r, 1], (x * th[r, 0]).astype(f32))
        a = fma32(z, th[r, 2], a)
        return (a + th[r, 3]).astype(f32)
    gx, gy, gz = comp(0), comp(1), comp(2)

    def unnorm(c):
        return np.clip(((c + f32(1.0)) * f32(N) - f32(1.0)) * f32(0.5), f32(0.0), f32(N - 1))
    ux, uy, uz = unnorm(gx), unnorm(gy), unnorm(gz)
    x0 = np.floor(ux); y0 = np.floor(uy); z0 = np.floor(uz)
    fx = (ux - x0).astype(f32); fy = (uy - y0).astype(f32); fz = (uz - z0).astype(f32)
    x0i = x0.astype(np.int64); y0i = y0.astype(np.int64); z0i = z0.astype(np.int64)
    x1i = np.minimum(x0i + 1, N - 1); y1i = np.minimum(y0i + 1, N - 1); z1i = np.minimum(z0i + 1, N - 1)

    vol = vol_g.reshape(-1, N, N, N)
    out = np.zeros(bb.shape, f32)
    one = f32(1.0)
    for zi, wz in ((z0i, (one - fz).astype(f32)), (z1i, fz)):
        for yi, wy in ((y0i, (one - fy).astype(f32)), (y1i, fy)):
            for xi, wx in ((x0i, (one - fx).astype(f32)), (x1i, fx)):
                # inp[b, c, zi, yi, xi] in transposed space == vol[b, xi, yi, zi]
                vals = vol[bb, xi, yi, zi]
                w = ((wz * wy).astype(f32) * wx).astype(f32)
                out = (out + (vals * w).astype(f32)).astype(f32)
    return out


def _host_fallback(input_g, label_g, transform):
    """Arbitrary-transform fallback: full reference computation on host."""
    bb, ii, jj, kk = np.meshgrid(np.arange(8), np.arange(N), np.arange(N), np.arange(N), indexing="ij")
    bb, ii, jj, kk = (a.reshape(-1) for a in (bb, ii, jj, kk))
    theta = transform[:3].astype(np.float32)
    aug_inp = _exact_reference_values(input_g, theta, bb, ii, jj, kk).reshape(8, 1, N, N, N)
    lab = _exact_reference_values(label_g, theta, bb, ii, jj, kk).reshape(8, 1, N, N, N)
    return aug_inp.astype(np.float32), lab > np.float32(0.5)


def _make_inputs(tables, input_g, label_g):
    idx_p = _pack_idxs(tables["idxA"].reshape(-1))
    # packed k-replicated per-call weight tiles:
    # wtile[call, j, ci*GPC*128 + slot*128 + k] = w_ci(i = call*8 + slot, j)
    wt = np.empty((NCALLS, 128, 4, GPC, N), np.float16)
    for ci, nm in enumerate(("w00", "w01", "w10", "w11")):
        x = tables[nm].T.reshape(128, NCALLS, GPC).astype(np.float16)  # [j, call, slot]
        wt[:, :, ci] = x.transpose(1, 0, 2)[:, :, :, None]
    wtile = np.ascontiguousarray(wt.reshape(NCALLS, 128, 4 * GPC * N))
    fwrep = np.stack([np.tile(1.0 - tables["fw"], (128, 1)),
                      np.tile(tables["fw"], (128, 1))]).astype(np.float16)
    in_maps = []
    for b in range(8):
        in_maps.append({
            # partition-major: vol[p, gc*128+k] = volume[row=gc*128+p, k]
            "vol0": np.ascontiguousarray(
                input_g[b, 0].reshape(128, 128, N).astype(np.float16).transpose(1, 0, 2).reshape(128, NROWS)),
            "vol1": np.ascontiguousarray(
                label_g[b, 0].reshape(128, 128, N).astype(np.float16).transpose(1, 0, 2).reshape(128, NROWS)),
            "idxA": idx_p, "wtile": wtile, "fwrep": fwrep,
        })
    return in_maps


def kernel(input_g, label_g, transform):
    input_g = np.ascontiguousarray(input_g, dtype=np.float32)
    label_g = np.ascontiguousarray(label_g, dtype=np.float32)
    transform = np.asarray(transform, dtype=np.float32)
    theta = transform[:3]

    structured = (abs(float(theta[0, 2])) < 1e-12 and abs(float(theta[1, 2])) < 1e-12
                  and abs(float(theta[2, 0])) < 1e-12 and abs(float(theta[2, 1])) < 1e-12)
    if not structured:
        return _host_fallback(input_g, label_g, transform)

    from concourse.bass_utils import run_bass_kernel_spmd

    tables = _host_tables(theta)
    key = transform.tobytes()
    if key not in _CACHE:
        _CACHE[key] = _build_program(tables)
    nc = _CACHE[key]

    in_maps = _make_inputs(tables, input_g, label_g)
    res = run_bass_kernel_spmd(nc, in_maps, core_ids=list(range(8)))

    aug_inp = np.empty((8, 1, N, N, N), np.float32)
    lab_f = np.empty((8, 1, N, N, N), np.float32)
    for b in range(8):
        oi = res.results[b]["outI"]
        aug_inp[b, 0] = oi[:, 0:128].astype(np.float32).reshape(N, N, N)
        lab_f[b, 0] = oi[:, 128:256].astype(np.float32).reshape(N, N, N)

    out_bool = lab_f > np.float32(0.5)
    out_bool = _exact_label_fixup(label_g, theta, lab_f, out_bool)
    return aug_inp, out_bool


# revision 35
# speedup vs baseline: 1.5969x; 1.1681x over previous
"""Trainium2 Bass kernel for SegmentationAugmentation (3D affine grid_sample, trilinear, border).

Contract: kernel(input_g, label_g, transform) -> (aug_inp f32 [8,1,128,128,128],
                                                  aug_lab bool [8,1,128,128,128])

Math (swapaxes folded into index bookkeeping; all spatial dims 128):

  out[b,c,i,j,k] = trilinear sample of input_g[b,c,:,:,:] at
      p-axis: U(i,j) = clip(64*(a00*xn(i)+a01*xn(j)+a03)+63.5, 0, 127)
      q-axis: V(i,j) = clip(64*(a10*xn(i)+a11*xn(j)+a13)+63.5, 0, 127)
      r-axis: W(k)   = clip(64*(a22*xn(k)+a23)+63.5, 0, 127)
  with xn(t) = (2t+1)/128 - 1, theta = transform[:3].  Relies on the
  generator's z-rotation structure (theta[0:2,2]==0, theta[2,0:2]==0); a
  pure-host fallback handles arbitrary transforms.

Device pipeline, data parallel over batch (core b handles batch b; each core
processes BOTH its image and label volume in one fused f16 program):

  Phase 1 (dense): load pre-transposed f16 volume chunks, z-interp on DVE via
  run-segmented staircase slices (f16, 2x perf mode), then DMA the z-interped
  rows into an interleaved pair layout in DRAM:
     ZPI[r = p*128+q] = [Z0(p,q) | Z0(p+1,q) | Z1(p,q) | Z1(p+1,q)]  (1 KiB)
  (Z0 = image, Z1 = label; each row is 128 f16 k-values.)

  Phase 2 (gather): for each output point (i,j), ONE dma_gather descriptor of
  2 KiB at entry r=(p0*128+q0) fetches entries r,r+1 = all four bilinear
  corners of BOTH volumes.  DVE combines with k-replicated f16 weight tiles
  (streamed from DRAM per call so every operand keeps innermost stride 1 and
  2-byte dtype -> DVE 2x perf mode); one 512B-descriptor DMA per call writes
  the interleaved f16 outputs of both volumes.

Host converts the f16 outputs to f32 / bool; label voxels within FIX_EPS of
0.5 are recomputed in the reference's exact f32 arithmetic order.
"""
import numpy as np

N = 128
NROWS = N * N            # 16384 (p,q) rows per volume
NIDX = 1024              # gather indices (output points) per dma_gather call
GPC = NIDX // 128        # 8 element groups per partition per call
NCALLS = NROWS // NIDX   # 16 gather calls per rep
COLS = NIDX // 16        # idx table columns per call
ELEM = 1024              # gathered f16 elems per descriptor (= 2 ZPI entries)
ESTEP = 512              # f16 elems per ZPI entry (gather elem_step)
CH = 16                  # 128-row groups per load chunk
NGRP = NROWS // N // CH  # 8 chunks per volume
ZG = 129                 # zt groups (128 data + 1 zero pad)
FIX_EPS = 8e-3           # |label-0.5| below this -> exact host recompute

_CACHE = {}


def _mkap(pairs):
    import bass_rust
    return bass_rust.VecI64Pair([tuple(p) for p in pairs])


def _host_tables(theta):
    """All transform-derived tables, computed in float64 from f32 theta."""
    th = theta.astype(np.float64)
    t = np.arange(N, dtype=np.float64)
    xn = (2.0 * t + 1.0) / N - 1.0

    U = np.clip(64.0 * (th[0, 0] * xn[:, None] + th[0, 1] * xn[None, :] + th[0, 3]) + 63.5, 0.0, 127.0)
    V = np.clip(64.0 * (th[1, 0] * xn[:, None] + th[1, 1] * xn[None, :] + th[1, 3]) + 63.5, 0.0, 127.0)
    W = np.clip(64.0 * (th[2, 2] * xn + th[2, 3]) + 63.5, 0.0, 127.0)

    p0 = np.floor(U).astype(np.int64)
    q0 = np.floor(V).astype(np.int64)
    r0 = np.floor(W).astype(np.int64)
    fu = (U - p0).astype(np.float32)
    fv = (V - q0).astype(np.float32)
    fw = (W - r0).astype(np.float32)
    r1 = np.minimum(r0 + 1, N - 1)

    idxA = (p0 * 128 + q0).astype(np.int16)          # [i,j] ZPI entry index
    w00 = ((1 - fu) * (1 - fv)).astype(np.float32)
    w10 = (fu * (1 - fv)).astype(np.float32)
    w01 = ((1 - fu) * fv).astype(np.float32)
    w11 = (fu * fv).astype(np.float32)

    # z-run decomposition: maximal segments where both r0 and r1 step by a
    # constant d in {-1,0,1}
    runs = []
    k = 0
    while k < N:
        step = 0
        if k + 1 < N:
            d = int(r0[k + 1] - r0[k])
            if d == int(r1[k + 1] - r1[k]) and d in (-1, 0, 1):
                step = d
        ln = 1
        while (k + ln < N
               and int(r0[k + ln] - r0[k]) == step * ln
               and int(r1[k + ln] - r1[k]) == step * ln):
            ln += 1
        runs.append((k, ln, int(r0[k]), int(r1[k]), step))
        k += ln

    return dict(idxA=idxA, w00=w00, w01=w01, w10=w10, w11=w11, fw=fw, runs=runs)


def _pack_idxs(idx_flat):
    """int16 dma_gather index layout: element i at [i%16, i//16], replicated to 128 partitions."""
    t = idx_flat.reshape(-1, 16).T.astype(np.int16)  # [16, n/16]
    return np.ascontiguousarray(np.tile(t, (8, 1)))  # [128, n/16]


def _chunk_plan(tables):
    """Stream chunks (2048 ZPI entries each) in the order matching the calls'
    p-band progression; per call, how many streamed chunks it needs."""
    idxA = tables["idxA"].reshape(-1).astype(np.int64)
    # call c covers points c*NIDX..(c+1)*NIDX-1; entries r and r+1 needed
    need = []
    for c in range(NCALLS):
        rs = idxA[c * NIDX:(c + 1) * NIDX]
        need.append((int(rs.min()) // (CH * 128), (int(rs.max()) + 1) // (CH * 128)))
    first_lo, _ = need[0]
    last_lo, _ = need[-1]
    descending = first_lo >= last_lo
    order = list(range(NGRP - 1, -1, -1)) if descending else list(range(NGRP))
    pos = {g: i for i, g in enumerate(order)}
    nch = [max(pos[min(lo, NGRP - 1)], pos[min(hi, NGRP - 1)]) + 1 for lo, hi in need]
    return order, nch


def _build_program(tables, reps=1):
    """Raw-Bass (explicit semaphore) program; see module docstring for the
    pipeline.  All cross-engine waits are standalone wait_ge instructions.

    Engine streams:
      sync   (SP HWDGE):  const/volume loads, per-chunk ZPI stream writes
      scalar (ACT):       h1-slot entry assembly copies; weight-tile loads and
                          interleaved output writes (HWDGE)
      vector (DVE):       z-interp into ZS entry layout, 4-corner combine
      gpsimd (SWDGE):     one dma_gather per 1024 output points, fired as
                          soon as the chunks its points touch are streamed
    """
    import concourse.bass as bass
    from concourse import bacc, mybir

    runs = tables["runs"]
    f16 = mybir.dt.float16
    i16 = mybir.dt.int16

    nc = bacc.Bacc("TRN2", target_bir_lowering=False, debug=False, num_devices=8)

    vol_in = [nc.dram_tensor(f"vol{v}", [128, NROWS], f16, kind="ExternalInput") for v in range(2)]
    idx_dram = nc.dram_tensor("idxA", [128, NROWS // 16], i16, kind="ExternalInput")
    wtile = nc.dram_tensor("wtile", [NCALLS, 128, 4 * GPC * N], f16, kind="ExternalInput")
    fwrep = nc.dram_tensor("fwrep", [2, 128, 128], f16, kind="ExternalInput")
    out_i = nc.dram_tensor("outI", [NROWS, 256], f16, kind="ExternalOutput")
    zpi = nc.dram_tensor("zpi", [NROWS + 1, ESTEP], f16, kind="Internal")

    AP = bass.AP

    WD = 4 * GPC * N  # packed weight tile width (4096)
    idx_t = nc.alloc_sbuf_tensor("idx_t", [128, NROWS // 16], i16)
    fw_t = [nc.alloc_sbuf_tensor(f"fw{c}_t", [128, 128], f16) for c in range(2)]
    wt_sb = [nc.alloc_sbuf_tensor(f"wt_{s}", [128, WD], f16) for s in range(4)]
    vt16 = [nc.alloc_sbuf_tensor(f"vt16_{s}", [128, CH * N], f16) for s in range(4)]
    zs = [nc.alloc_sbuf_tensor(f"zs{s}", [128, CH * ESTEP], f16) for s in range(3)]
    ztmp = nc.alloc_sbuf_tensor("ztmp", [128, CH * N], f16)
    At = [nc.alloc_sbuf_tensor(f"At{s}", [128, GPC * ELEM], f16) for s in range(4)]
    mt = [nc.alloc_sbuf_tensor(f"m{s}", [128, GPC * N], f16) for s in range(8)]
    accb = [nc.alloc_sbuf_tensor(f"accb{s}", [128, GPC * 256], f16) for s in range(4)]

    nrows_ap = NROWS  # gather element at entry r reads entries r, r+1; r <= 16383
    NC_ = NCALLS
    ZSW = CH * ESTEP  # 8192
    chunk_order, nch = _chunk_plan(tables)
    # the h1 cross-chunk copy sources chunk g+1, which must already be in the
    # other ZS buffer -> chunks must stream top-down
    assert chunk_order == list(range(NGRP - 1, -1, -1)), chunk_order

    from contextlib import ExitStack
    with ExitStack() as _sctx:
        block = _sctx.enter_context(nc.Block())
        s_idx = _sctx.enter_context(nc.semaphore("s_idx"))
        s_wf = _sctx.enter_context(nc.semaphore("s_wf"))
        s_mz = _sctx.enter_context(nc.semaphore("s_mz"))
        s_l = [_sctx.enter_context(nc.semaphore(f"s_l{p}")) for p in range(4)]
        s_wl = [_sctx.enter_context(nc.semaphore(f"s_wl{p}")) for p in range(4)]
        s_z = _sctx.enter_context(nc.semaphore("s_z"))
        s_a = _sctx.enter_context(nc.semaphore("s_a"))
        s_zw = [_sctx.enter_context(nc.semaphore(f"s_zw{p}")) for p in range(NGRP)]
        s_zz = _sctx.enter_context(nc.semaphore("s_zz"))
        s_g = [_sctx.enter_context(nc.semaphore(f"s_g{p}")) for p in range(4)]
        s_c = _sctx.enter_context(nc.semaphore("s_c"))
        s_o = [_sctx.enter_context(nc.semaphore(f"s_o{p}")) for p in range(4)]
        s_v = _sctx.enter_context(nc.semaphore("s_v"))

        @block.sync
        def _(sync):
            for v in range(2):  # chunk 0 loads first: unblock DVE asap
                sync.dma_start(
                    AP(vt16[v], 0, [[CH * N, 128], [1, CH * N]]),
                    AP(vol_in[v], chunk_order[0] * CH * N, [[NROWS, 128], [1, CH * N]]),
                ).then_inc(s_l[v], 16)
            sync.dma_start(idx_t.ap(), idx_dram.ap()).then_inc(s_idx, 16)
            for c in range(2):
                sync.dma_start(fw_t[c].ap(), AP(fwrep, c * 128 * 128, [[128, 128], [1, 128]])).then_inc(s_wf, 16)
            # one-time: zero ZPI entry 16384 (read by gathers at r=16383)
            sync.wait_ge(s_mz, 1)
            sync.dma_start(
                AP(zpi, NROWS * ESTEP, [[128, 4], [1, 128]]),
                AP(mt[0], 0, [[GPC * N, 4], [1, 128]]),
            ).then_inc(s_zz, 16)
            def wt_load(gc):
                c = gc % NC_
                sync.dma_start(
                    wt_sb[gc % 4].ap(),
                    AP(wtile, c * 128 * WD, [[WD, 128], [1, WD]]),
                ).then_inc(s_wl[gc % 4], 16)

            for gc in range(min(4, NC_ * reps)):
                wt_load(gc)
            for r in range(reps):
                for gi, g in enumerate(chunk_order):
                    for v in range(2):
                        zc = r * 16 + gi * 2 + v
                        if r == 0 and gi == 0:
                            continue  # preloaded above
                        if zc >= 4:
                            sync.wait_ge(s_z, zc - 3)  # WAR vt16 vs z-interp
                        sync.dma_start(
                            AP(vt16[zc % 4], 0, [[CH * N, 128], [1, CH * N]]),
                            AP(vol_in[v], g * CH * N, [[NROWS, 128], [1, CH * N]]),
                        ).then_inc(s_l[zc % 4], 16)
                for c in range(NC_):
                    gc = r * NC_ + c
                    sync.wait_ge(s_c, gc + 1)
                    sync.dma_start(
                        AP(out_i, c * NIDX * 256, [[256, 128], [128 * 256, GPC], [1, 256]]),
                        AP(accb[gc % 4], 0, [[GPC * 256, 128], [256, GPC], [1, 256]]),
                    ).then_inc(s_o[gc % 4], 16)
                    if gc + 4 < NC_ * reps:
                        wt_load(gc + 4)
            for p in range(4):
                sync.wait_ge(s_o[p], 16 * ((NC_ * reps - p + 3) // 4))

        @block.scalar
        def _(scalar):
            for r in range(reps):
                for gi, g in enumerate(chunk_order):
                    # assemble h1 slots of chunk g: entry r gets row r+128,
                    # i.e. subgroup s copies from subgroup s+1 (h0 slot)
                    gcg = r * 8 + gi
                    scalar.wait_ge(s_z, r * 16 + 2 * gi + 2)
                    b = zs[gcg % 3]
                    for v in range(2):
                        scalar.copy(
                            AP(b, v * 256 + 128, [[ZSW, 128], [ESTEP, CH - 1], [1, N]]),
                            AP(b, ESTEP + v * 256, [[ZSW, 128], [ESTEP, CH - 1], [1, N]]),
                        )
                    last_ins = None
                    for v in range(2):
                        dst = AP(b, (CH - 1) * ESTEP + v * 256 + 128, [[ZSW, 128], [1, N]])
                        if g == NGRP - 1:
                            # top chunk: rows >= 16384 are zero (border clamp
                            # gives these corners zero weight; keep finite)
                            last_ins = scalar.memzero(dst)
                        else:
                            src_b = zs[(gcg - 1) % 3]
                            last_ins = scalar.copy(dst, AP(src_b, v * 256, [[ZSW, 128], [1, N]]))
                    last_ins.then_inc(s_a, 1)
                    # stream this chunk's assembled entries to ZPI; the ACT
                    # sequencer runs ahead of the engine pipeline, so fully
                    # drain the copies before the DMA reads the buffer
                    scalar.drain()
                    scalar.wait_ge(s_a, gcg + 1)
                    if gi == 0 and r >= 1:
                        scalar.wait_ge(s_c, NC_ * r)  # WAR zpi vs prev-rep gathers
                    scalar.dma_start(
                        AP(zpi, g * CH * 128 * ESTEP, [[ESTEP, 128], [128 * ESTEP, CH], [1, ESTEP]]),
                        AP(b, 0, [[ZSW, 128], [ESTEP, CH], [1, ESTEP]]),
                    ).then_inc(s_zw[gi], 16)

        @block.gpsimd
        def _(gpsimd):
            nreg = gpsimd.to_reg(NIDX)
            gpsimd.wait_ge(s_idx, 16)
            sv = AP(zpi, 0, [[ESTEP, nrows_ap], [1, ELEM]])
            for r in range(reps):
                for c in range(NC_):
                    gc = r * NC_ + c
                    if gc == 0:
                        gpsimd.wait_ge(s_zz, 16)
                    for pos in range(nch[c]):
                        gpsimd.wait_ge(s_zw[pos], 16 * (r + 1))
                    if gc >= 4:
                        gpsimd.wait_ge(s_c, gc - 3)  # WAR At vs combine
                    gpsimd.dma_gather(
                        AP(At[gc % 4], 0, [[GPC * ELEM, 128], [ELEM, GPC], [1, ELEM]]),
                        sv,
                        AP(idx_t, c * COLS, [[NROWS // 16, 128], [1, COLS]]),
                        NIDX, nreg, ELEM, elem_step=ESTEP,
                    ).then_inc(s_g[gc % 4], 16)

        @block.vector
        def _(vector):
            mult = mybir.AluOpType.mult
            VC = [0]

            def vsync(last_ins):
                # DVE pipeline does not interlock same-engine RAW hazards
                last_ins.then_inc(s_v, 1)
                VC[0] += 1
                vector.wait_ge(s_v, VC[0])

            vector.wait_ge(s_wf, 32)
            vector.memset(AP(mt[0], 0, [[GPC * N, 4], [1, 128]]), 0.0).then_inc(s_mz, 1)

            def zchunk(r, gi, g, v):
                zc = r * 16 + gi * 2 + v
                gcg = r * 8 + gi
                if zc >= 1:
                    vector.wait_ge(s_z, zc)  # WAR ztmp/pipeline drain
                vector.wait_ge(s_l[zc % 4], 16 * (zc // 4 + 1))
                if v == 0 and gcg >= 3:
                    # WAR zs[gcg%3] vs chunk gcg-3's stream + chunk gcg-2's
                    # h1 cross-read
                    vector.wait_ge(s_a, gcg - 1)
                    vector.wait_ge(s_zw[(gcg - 3) % NGRP], 16 * ((gcg - 3) // NGRP + 1))
                s = vt16[zc % 4]
                b = zs[gcg % 3]
                last_ins = None
                for (ks, ln, r0s, r1s, st) in runs:
                    zdst = AP(b, v * 256 + ks, [[ZSW, 128], [ESTEP, CH], [1, ln]])
                    tdst = AP(ztmp, ks, [[CH * N, 128], [N, CH], [1, ln]])
                    v0 = AP(s, r0s, [[CH * N, 128], [N, CH], [st, ln]])
                    v1 = AP(s, r1s, [[CH * N, 128], [N, CH], [st, ln]])
                    f0 = AP(fw_t[0], ks, [[128, 128], [0, CH], [1, ln]])
                    f1 = AP(fw_t[1], ks, [[128, 128], [0, CH], [1, ln]])
                    vector.tensor_tensor(zdst, v0, f0, mult)
                    last_ins = vector.tensor_tensor(tdst, v1, f1, mult)
                vsync(last_ins)
                for (ks, ln, r0s, r1s, st) in runs:
                    zdst = AP(b, v * 256 + ks, [[ZSW, 128], [ESTEP, CH], [1, ln]])
                    tdst = AP(ztmp, ks, [[CH * N, 128], [N, CH], [1, ln]])
                    last_ins = vector.tensor_add(zdst, zdst, tdst)
                last_ins.then_inc(s_z, 1)

            def combine(r, c):
                gc = r * NC_ + c
                if gc >= 1:
                    vector.wait_ge(s_c, gc)  # WAR mt vs prev combine
                vector.wait_ge(s_g[gc % 4], 16 * (gc // 4 + 1))
                vector.wait_ge(s_wl[gc % 4], 16 * (gc // 4 + 1))
                if gc >= 4:
                    vector.wait_ge(s_o[gc % 4], 16 * (gc // 4))  # WAR accb
                A = At[gc % 4]
                W = wt_sb[gc % 4]
                shp = [[GPC * ELEM, 128], [ELEM, GPC], [1, N]]
                oshp = [[GPC * N, 128], [N, GPC], [1, N]]

                def wb(ci):
                    return AP(W, ci * GPC * N, [[WD, 128], [N, GPC], [1, N]])
                maps = [AP(m, 0, oshp) for m in mt]
                # corner offsets within a gathered element (f16 elems):
                #   vol v: (p0,q0)=v*256, (p1,q0)=v*256+128,
                #          (p0,q1)=v*256+512, (p1,q1)=v*256+640
                last_ins = None
                for v in range(2):
                    b = 4 * v
                    vector.tensor_tensor(maps[b + 0], AP(A, v * 256 + 0, shp), wb(0), mult)
                    vector.tensor_tensor(maps[b + 1], AP(A, v * 256 + 128, shp), wb(2), mult)
                    vector.tensor_tensor(maps[b + 2], AP(A, v * 256 + 512, shp), wb(1), mult)
                    last_ins = vector.tensor_tensor(maps[b + 3], AP(A, v * 256 + 640, shp), wb(3), mult)
                vsync(last_ins)
                for v in range(2):
                    b = 4 * v
                    vector.tensor_add(maps[b + 0], maps[b + 0], maps[b + 1])
                    last_ins = vector.tensor_add(maps[b + 2], maps[b + 2], maps[b + 3])
                vsync(last_ins)
                osh2 = [[GPC * 256, 128], [256, GPC], [1, N]]
                vector.tensor_add(AP(accb[gc % 4], 0, osh2), maps[0], maps[2])
                vector.tensor_add(AP(accb[gc % 4], 128, osh2), maps[4], maps[6]) \
                    .then_inc(s_c, 1)

            # interleave the first combines into the z tail: their gathers
            # land while later chunks are still z-interping
            ileave = {(NGRP - 2, 1): [0], (NGRP - 1, 0): [1]}
            for r in range(reps):
                for gi, g in enumerate(chunk_order):
                    for v in range(2):
                        zchunk(r, gi, g, v)
                        for c in ileave.get((gi, v), []):
                            combine(r, c)
                for c in range(2, NC_):
                    combine(r, c)

    nc.compile()
    return nc


def _exact_label_fixup(label_g, theta, lab_f, out_bool):
    """Recompute voxels of |lab_f - 0.5| < FIX_EPS in the reference's exact
    f32 arithmetic order (validated bit-exact against the jax reference)."""
    eps = np.float32(FIX_EPS)
    cand = np.abs(lab_f - np.float32(0.5)) < eps
    if not cand.any():
        return out_bool
    bb, ii, jj, kk = np.nonzero(cand.reshape(-1, N, N, N))
    v = _exact_reference_values(label_g, theta, bb, ii, jj, kk)
    out_bool.reshape(-1, N, N, N)[bb, ii, jj, kk] = v > np.float32(0.5)
    return out_bool


def _exact_reference_values(vol_g, theta, bb, ii, jj, kk):
    """Reference-order f32 trilinear values at selected voxels.

    Replicates: grid einsum (x*t0 + y*t1 + z*t2, left-assoc f32) + t3; unnorm;
    8-corner accumulation in (z,y,x) order with w=(wz*wy)*wx, out += v*w.
    """
    f32 = np.float32
    t = np.arange(N, dtype=f32)
    xn = ((f32(2.0) * t + f32(1.0)) / f32(N) - f32(1.0)).astype(f32)
    th = theta.astype(f32)

    x = xn[ii]; y = xn[jj]; z = xn[kk]

    # f32 fma via f64 (exact up to negligible double-rounding corner cases)
    def fma32(a, b, c):
        return (np.float64(a) * np.float64(b) + c.astype(np.float64)).astype(f32)

    # grid components — XLA CPU lowers the einsum as an FMA chain (verified
    # bit-exact): fma(z, t2, fma(y, t1, x*t0)) + t3
    def comp(r):
        a = fma32(y, th[r, 1], (x * th[r, 0]).astype(f32))
        a = fma32(z, th[r, 2], a)
        return (a + th[r, 3]).astype(f32)
    gx, gy, gz = comp(0), comp(1), comp(2)

    def unnorm(c):
        return np.clip(((c + f32(1.0)) * f32(N) - f32(1.0)) * f32(0.5), f32(0.0), f32(N - 1))
    ux, uy, uz = unnorm(gx), unnorm(gy), unnorm(gz)
    x0 = np.floor(ux); y0 = np.floor(uy); z0 = np.floor(uz)
    fx = (ux - x0).astype(f32); fy = (uy - y0).astype(f32); fz = (uz - z0).astype(f32)
    x0i = x0.astype(np.int64); y0i = y0.astype(np.int64); z0i = z0.astype(np.int64)
    x1i = np.minimum(x0i + 1, N - 1); y1i = np.minimum(y0i + 1, N - 1); z1i = np.minimum(z0i + 1, N - 1)

    vol = vol_g.reshape(-1, N, N, N)
    out = np.zeros(bb.shape, f32)
    one = f32(1.0)
    for zi, wz in ((z0i, (one - fz).astype(f32)), (z1i, fz)):
        for yi, wy in ((y0i, (one - fy).astype(f32)), (y1i, fy)):
            for xi, wx in ((x0i, (one - fx).astype(f32)), (x1i, fx)):
                # inp[b, c, zi, yi, xi] in transposed space == vol[b, xi, yi, zi]
                vals = vol[bb, xi, yi, zi]
                w = ((wz * wy).astype(f32) * wx).astype(f32)
                out = (out + (vals * w).astype(f32)).astype(f32)
    return out


def _host_fallback(input_g, label_g, transform):
    """Arbitrary-transform fallback: full reference computation on host."""
    bb, ii, jj, kk = np.meshgrid(np.arange(8), np.arange(N), np.arange(N), np.arange(N), indexing="ij")
    bb, ii, jj, kk = (a.reshape(-1) for a in (bb, ii, jj, kk))
    theta = transform[:3].astype(np.float32)
    aug_inp = _exact_reference_values(input_g, theta, bb, ii, jj, kk).reshape(8, 1, N, N, N)
    lab = _exact_reference_values(label_g, theta, bb, ii, jj, kk).reshape(8, 1, N, N, N)
    return aug_inp.astype(np.float32), lab > np.float32(0.5)


def _make_inputs(tables, input_g, label_g):
    idx_p = _pack_idxs(tables["idxA"].reshape(-1))
    # packed k-replicated per-call weight tiles:
    # wtile[call, j, ci*GPC*128 + slot*128 + k] = w_ci(i = call*8 + slot, j)
    wt = np.empty((NCALLS, 128, 4, GPC, N), np.float16)
    for ci, nm in enumerate(("w00", "w01", "w10", "w11")):
        x = tables[nm].T.reshape(128, NCALLS, GPC).astype(np.float16)  # [j, call, slot]
        wt[:, :, ci] = x.transpose(1, 0, 2)[:, :, :, None]
    wtile = np.ascontiguousarray(wt.reshape(NCALLS, 128, 4 * GPC * N))
    fwrep = np.stack([np.tile(1.0 - tables["fw"], (128, 1)),
                      np.tile(tables["fw"], (128, 1))]).astype(np.float16)
    in_maps = []
    for b in range(8):
        in_maps.append({
            # partition-major: vol[p, gc*128+k] = volume[row=gc*128+p, k]
            "vol0": np.ascontiguousarray(
                input_g[b, 0].reshape(128, 128, N).astype(np.float16).transpose(1, 0, 2).reshape(128, NROWS)),
            "vol1": np.ascontiguousarray(
                label_g[b, 0].reshape(128, 128, N).astype(np.float16).transpose(1, 0, 2).reshape(128, NROWS)),
            "idxA": idx_p, "wtile": wtile, "fwrep": fwrep,
        })
    return in_maps


def kernel(input_g, label_g, transform):
    input_g = np.ascontiguousarray(input_g, dtype=np.float32)
    label_g = np.ascontiguousarray(label_g, dtype=np.float32)
    transform = np.asarray(transform, dtype=np.float32)
    theta = transform[:3]

    structured = (abs(float(theta[0, 2])) < 1e-12 and abs(float(theta[1, 2])) < 1e-12
                  and abs(float(theta[2, 0])) < 1e-12 and abs(float(theta[2, 1])) < 1e-12)
    if not structured:
        return _host_fallback(input_g, label_g, transform)

    from concourse.bass_utils import run_bass_kernel_spmd

    tables = _host_tables(theta)
    key = transform.tobytes()
    if key not in _CACHE:
        _CACHE[key] = _build_program(tables)
    nc = _CACHE[key]

    in_maps = _make_inputs(tables, input_g, label_g)
    res = run_bass_kernel_spmd(nc, in_maps, core_ids=list(range(8)))

    aug_inp = np.empty((8, 1, N, N, N), np.float32)
    lab_f = np.empty((8, 1, N, N, N), np.float32)
    for b in range(8):
        oi = res.results[b]["outI"]
        aug_inp[b, 0] = oi[:, 0:128].astype(np.float32).reshape(N, N, N)
        lab_f[b, 0] = oi[:, 128:256].astype(np.float32).reshape(N, N, N)

    out_bool = lab_f > np.float32(0.5)
    out_bool = _exact_label_fixup(label_g, theta, lab_f, out_bool)
    return aug_inp, out_bool


# revision 39
# speedup vs baseline: 1.6068x; 1.0062x over previous
"""Trainium2 Bass kernel for SegmentationAugmentation (3D affine grid_sample, trilinear, border).

Contract: kernel(input_g, label_g, transform) -> (aug_inp f32 [8,1,128,128,128],
                                                  aug_lab bool [8,1,128,128,128])

Math (swapaxes folded into index bookkeeping; all spatial dims 128):

  out[b,c,i,j,k] = trilinear sample of input_g[b,c,:,:,:] at
      p-axis: U(i,j) = clip(64*(a00*xn(i)+a01*xn(j)+a03)+63.5, 0, 127)
      q-axis: V(i,j) = clip(64*(a10*xn(i)+a11*xn(j)+a13)+63.5, 0, 127)
      r-axis: W(k)   = clip(64*(a22*xn(k)+a23)+63.5, 0, 127)
  with xn(t) = (2t+1)/128 - 1, theta = transform[:3].  Relies on the
  generator's z-rotation structure (theta[0:2,2]==0, theta[2,0:2]==0); a
  pure-host fallback handles arbitrary transforms.

Device pipeline, data parallel over batch (core b handles batch b; each core
processes BOTH its image and label volume in one fused f16 program):

  Phase 1 (dense): load pre-transposed f16 volume chunks, z-interp on DVE via
  run-segmented staircase slices (f16, 2x perf mode), then DMA the z-interped
  rows into an interleaved pair layout in DRAM:
     ZPI[r = p*128+q] = [Z0(p,q) | Z0(p+1,q) | Z1(p,q) | Z1(p+1,q)]  (1 KiB)
  (Z0 = image, Z1 = label; each row is 128 f16 k-values.)

  Phase 2 (gather): for each output point (i,j), ONE dma_gather descriptor of
  2 KiB at entry r=(p0*128+q0) fetches entries r,r+1 = all four bilinear
  corners of BOTH volumes.  DVE combines with k-replicated f16 weight tiles
  (streamed from DRAM per call so every operand keeps innermost stride 1 and
  2-byte dtype -> DVE 2x perf mode); one 512B-descriptor DMA per call writes
  the interleaved f16 outputs of both volumes.

Host converts the f16 outputs to f32 / bool; label voxels within FIX_EPS of
0.5 are recomputed in the reference's exact f32 arithmetic order.
"""
import numpy as np

N = 128
NROWS = N * N            # 16384 (p,q) rows per volume
NIDX = 1024              # gather indices (output points) per dma_gather call
GPC = NIDX // 128        # 8 element groups per partition per call
NCALLS = NROWS // NIDX   # 16 gather calls per rep
COLS = NIDX // 16        # idx table columns per call
ELEM = 1024              # gathered f16 elems per descriptor (= 2 ZPI entries)
ESTEP = 512              # f16 elems per ZPI entry (gather elem_step)
CH = 16                  # 128-row groups per load chunk
NGRP = NROWS // N // CH  # 8 chunks per volume
ZG = 129                 # zt groups (128 data + 1 zero pad)
FIX_EPS = 8e-3           # |label-0.5| below this -> exact host recompute

_CACHE = {}


def _mkap(pairs):
    import bass_rust
    return bass_rust.VecI64Pair([tuple(p) for p in pairs])


def _host_tables(theta):
    """All transform-derived tables, computed in float64 from f32 theta."""
    th = theta.astype(np.float64)
    t = np.arange(N, dtype=np.float64)
    xn = (2.0 * t + 1.0) / N - 1.0

    U = np.clip(64.0 * (th[0, 0] * xn[:, None] + th[0, 1] * xn[None, :] + th[0, 3]) + 63.5, 0.0, 127.0)
    V = np.clip(64.0 * (th[1, 0] * xn[:, None] + th[1, 1] * xn[None, :] + th[1, 3]) + 63.5, 0.0, 127.0)
    W = np.clip(64.0 * (th[2, 2] * xn + th[2, 3]) + 63.5, 0.0, 127.0)

    p0 = np.floor(U).astype(np.int64)
    q0 = np.floor(V).astype(np.int64)
    r0 = np.floor(W).astype(np.int64)
    fu = (U - p0).astype(np.float32)
    fv = (V - q0).astype(np.float32)
    fw = (W - r0).astype(np.float32)
    r1 = np.minimum(r0 + 1, N - 1)

    idxA = (p0 * 128 + q0).astype(np.int16)          # [i,j] ZPI entry index
    w00 = ((1 - fu) * (1 - fv)).astype(np.float32)
    w10 = (fu * (1 - fv)).astype(np.float32)
    w01 = ((1 - fu) * fv).astype(np.float32)
    w11 = (fu * fv).astype(np.float32)

    # z-run decomposition: maximal segments where both r0 and r1 step by a
    # constant d in {-1,0,1}
    runs = []
    k = 0
    while k < N:
        step = 0
        if k + 1 < N:
            d = int(r0[k + 1] - r0[k])
            if d == int(r1[k + 1] - r1[k]) and d in (-1, 0, 1):
                step = d
        ln = 1
        while (k + ln < N
               and int(r0[k + ln] - r0[k]) == step * ln
               and int(r1[k + ln] - r1[k]) == step * ln):
            ln += 1
        runs.append((k, ln, int(r0[k]), int(r1[k]), step))
        k += ln

    return dict(idxA=idxA, w00=w00, w01=w01, w10=w10, w11=w11, fw=fw, runs=runs)


def _pack_idxs(idx_flat):
    """int16 dma_gather index layout: element i at [i%16, i//16], replicated to 128 partitions."""
    t = idx_flat.reshape(-1, 16).T.astype(np.int16)  # [16, n/16]
    return np.ascontiguousarray(np.tile(t, (8, 1)))  # [128, n/16]


def _chunk_plan(tables):
    """Stream chunks (2048 ZPI entries each) in the order matching the calls'
    p-band progression; per call, how many streamed chunks it needs."""
    idxA = tables["idxA"].reshape(-1).astype(np.int64)
    # call c covers points c*NIDX..(c+1)*NIDX-1; entries r and r+1 needed
    need = []
    for c in range(NCALLS):
        rs = idxA[c * NIDX:(c + 1) * NIDX]
        need.append((int(rs.min()) // (CH * 128), (int(rs.max()) + 1) // (CH * 128)))
    first_lo, _ = need[0]
    last_lo, _ = need[-1]
    descending = first_lo >= last_lo
    order = list(range(NGRP - 1, -1, -1)) if descending else list(range(NGRP))
    pos = {g: i for i, g in enumerate(order)}
    nch = [max(pos[min(lo, NGRP - 1)], pos[min(hi, NGRP - 1)]) + 1 for lo, hi in need]
    return order, nch


def _build_program(tables, reps=1):
    """Raw-Bass (explicit semaphore) program; see module docstring for the
    pipeline.  All cross-engine waits are standalone wait_ge instructions.

    Engine streams:
      sync   (SP HWDGE):  const/volume loads, per-chunk ZPI stream writes
      scalar (ACT):       h1-slot entry assembly copies; weight-tile loads and
                          interleaved output writes (HWDGE)
      vector (DVE):       z-interp into ZS entry layout, 4-corner combine
      gpsimd (SWDGE):     one dma_gather per 1024 output points, fired as
                          soon as the chunks its points touch are streamed
    """
    import concourse.bass as bass
    from concourse import bacc, mybir

    runs = tables["runs"]
    f16 = mybir.dt.float16
    i16 = mybir.dt.int16

    nc = bacc.Bacc("TRN2", target_bir_lowering=False, debug=False, num_devices=8)

    vol_in = [nc.dram_tensor(f"vol{v}", [128, NROWS], f16, kind="ExternalInput") for v in range(2)]
    idx_dram = nc.dram_tensor("idxA", [128, NROWS // 16], i16, kind="ExternalInput")
    wts4 = nc.dram_tensor("wts4", [4, 128, 128], f16, kind="ExternalInput")
    fwrep = nc.dram_tensor("fwrep", [2, 128, 128], f16, kind="ExternalInput")
    out_i = nc.dram_tensor("outI", [NROWS, 256], f16, kind="ExternalOutput")
    zpi = nc.dram_tensor("zpi", [NROWS + 1, ESTEP], f16, kind="Internal")

    AP = bass.AP

    WD = 4 * GPC * N  # packed weight tile width (4096)
    idx_t = nc.alloc_sbuf_tensor("idx_t", [128, NROWS // 16], i16)
    fw_t = [nc.alloc_sbuf_tensor(f"fw{c}_t", [128, 128], f16) for c in range(2)]
    w4_t = [nc.alloc_sbuf_tensor(f"w4_{c}", [128, 128], f16) for c in range(4)]
    wt_sb = [nc.alloc_sbuf_tensor(f"wt_{s}", [128, WD], f16) for s in range(3)]
    vt16 = [nc.alloc_sbuf_tensor(f"vt16_{s}", [128, CH * N], f16) for s in range(4)]
    zs = [nc.alloc_sbuf_tensor(f"zs{s}", [128, CH * ESTEP], f16) for s in range(3)]
    ztmp = nc.alloc_sbuf_tensor("ztmp", [128, CH * N], f16)
    At = [nc.alloc_sbuf_tensor(f"At{s}", [128, GPC * ELEM], f16) for s in range(4)]
    mt = [nc.alloc_sbuf_tensor(f"m{s}", [128, GPC * N], f16) for s in range(8)]
    accb = [nc.alloc_sbuf_tensor(f"accb{s}", [128, GPC * 256], f16) for s in range(4)]

    nrows_ap = NROWS  # gather element at entry r reads entries r, r+1; r <= 16383
    NC_ = NCALLS
    ZSW = CH * ESTEP  # 8192
    chunk_order, nch = _chunk_plan(tables)
    # the h1 cross-chunk copy sources chunk g+1, which must already be in the
    # other ZS buffer -> chunks must stream top-down
    assert chunk_order == list(range(NGRP - 1, -1, -1)), chunk_order

    from contextlib import ExitStack
    with ExitStack() as _sctx:
        block = _sctx.enter_context(nc.Block())
        s_idx = _sctx.enter_context(nc.semaphore("s_idx"))
        s_wf = _sctx.enter_context(nc.semaphore("s_wf"))
        s_mz = _sctx.enter_context(nc.semaphore("s_mz"))
        s_l = [_sctx.enter_context(nc.semaphore(f"s_l{p}")) for p in range(4)]
        s_wl = [_sctx.enter_context(nc.semaphore(f"s_wl{p}")) for p in range(3)]
        s_z = _sctx.enter_context(nc.semaphore("s_z"))
        s_a = _sctx.enter_context(nc.semaphore("s_a"))
        s_zw = [_sctx.enter_context(nc.semaphore(f"s_zw{p}")) for p in range(NGRP)]
        s_zz = _sctx.enter_context(nc.semaphore("s_zz"))
        s_g = [_sctx.enter_context(nc.semaphore(f"s_g{p}")) for p in range(4)]
        s_c = _sctx.enter_context(nc.semaphore("s_c"))
        s_o = [_sctx.enter_context(nc.semaphore(f"s_o{p}")) for p in range(4)]
        s_v = _sctx.enter_context(nc.semaphore("s_v"))

        @block.sync
        def _(sync):
            for v in range(2):  # chunk 0 loads first: unblock DVE asap
                sync.dma_start(
                    AP(vt16[v], 0, [[CH * N, 128], [1, CH * N]]),
                    AP(vol_in[v], chunk_order[0] * CH * N, [[NROWS, 128], [1, CH * N]]),
                ).then_inc(s_l[v], 16)
            sync.dma_start(idx_t.ap(), idx_dram.ap()).then_inc(s_idx, 16)
            for c in range(2):
                sync.dma_start(fw_t[c].ap(), AP(fwrep, c * 128 * 128, [[128, 128], [1, 128]])).then_inc(s_wf, 16)
            for c in range(4):
                sync.dma_start(w4_t[c].ap(), AP(wts4, c * 128 * 128, [[128, 128], [1, 128]])).then_inc(s_wf, 16)
            # one-time: zero ZPI entry 16384 (read by gathers at r=16383)
            sync.wait_ge(s_mz, 1)
            sync.dma_start(
                AP(zpi, NROWS * ESTEP, [[128, 4], [1, 128]]),
                AP(mt[0], 0, [[GPC * N, 4], [1, 128]]),
            ).then_inc(s_zz, 16)
            for r in range(reps):
                for gi, g in enumerate(chunk_order):
                    for v in range(2):
                        zc = r * 16 + gi * 2 + v
                        if r == 0 and gi == 0:
                            continue  # preloaded above
                        if zc >= 4:
                            sync.wait_ge(s_z, zc - 3)  # WAR vt16 vs z-interp
                        sync.dma_start(
                            AP(vt16[zc % 4], 0, [[CH * N, 128], [1, CH * N]]),
                            AP(vol_in[v], g * CH * N, [[NROWS, 128], [1, CH * N]]),
                        ).then_inc(s_l[zc % 4], 16)
                for c in range(NC_):
                    gc = r * NC_ + c
                    sync.wait_ge(s_c, gc + 1)
                    sync.dma_start(
                        AP(out_i, c * NIDX * 256, [[256, 128], [128 * 256, GPC], [1, 256]]),
                        AP(accb[gc % 4], 0, [[GPC * 256, 128], [256, GPC], [1, 256]]),
                    ).then_inc(s_o[gc % 4], 16)
            for p in range(4):
                sync.wait_ge(s_o[p], 16 * ((NC_ * reps - p + 3) // 4))

        @block.scalar
        def _(scalar):
            def wt_gen(gc):
                # build the k-replicated weight tile for call gc%NC_ on-chip:
                # wt[j, ci*1024 + slot*128 + k] = w_ci(call*8+slot, j)
                c = gc % NC_
                for ci in range(4):
                    scalar.copy(
                        AP(wt_sb[gc % 3], ci * GPC * N, [[WD, 128], [N, GPC], [1, N]]),
                        AP(w4_t[ci], c * GPC, [[128, 128], [1, GPC], [0, N]]),
                    )
                # ACT write visibility to DVE: drain the pipeline before the
                # semaphore rises
                scalar.drain()
                scalar.sem_inc(s_wl[gc % 3], 16)

            scalar.wait_ge(s_wf, 96)
            for gc in range(min(3, NC_ * reps)):
                wt_gen(gc)
            for r in range(reps):
                for gi, g in enumerate(chunk_order):
                    # assemble h1 slots of chunk g: entry r gets row r+128,
                    # i.e. subgroup s copies from subgroup s+1 (h0 slot)
                    gcg = r * 8 + gi
                    scalar.wait_ge(s_z, r * 16 + 2 * gi + 2)
                    b = zs[gcg % 3]
                    for v in range(2):
                        scalar.copy(
                            AP(b, v * 256 + 128, [[ZSW, 128], [ESTEP, CH - 1], [1, N]]),
                            AP(b, ESTEP + v * 256, [[ZSW, 128], [ESTEP, CH - 1], [1, N]]),
                        )
                    last_ins = None
                    for v in range(2):
                        dst = AP(b, (CH - 1) * ESTEP + v * 256 + 128, [[ZSW, 128], [1, N]])
                        if g == NGRP - 1:
                            # top chunk: rows >= 16384 are zero (border clamp
                            # gives these corners zero weight; keep finite)
                            last_ins = scalar.memzero(dst)
                        else:
                            src_b = zs[(gcg - 1) % 3]
                            last_ins = scalar.copy(dst, AP(src_b, v * 256, [[ZSW, 128], [1, N]]))
                    last_ins.then_inc(s_a, 1)
                    # stream this chunk's assembled entries to ZPI; the ACT
                    # sequencer runs ahead of the engine pipeline, so fully
                    # drain the copies before the DMA reads the buffer
                    scalar.drain()
                    scalar.wait_ge(s_a, gcg + 1)
                    if gi == 0 and r >= 1:
                        scalar.wait_ge(s_c, NC_ * r)  # WAR zpi vs prev-rep gathers
                    scalar.dma_start(
                        AP(zpi, g * CH * 128 * ESTEP, [[ESTEP, 128], [128 * ESTEP, CH], [1, ESTEP]]),
                        AP(b, 0, [[ZSW, 128], [ESTEP, CH], [1, ESTEP]]),
                    ).then_inc(s_zw[gi], 16)
                for c in range(NC_):
                    gc = r * NC_ + c
                    if gc < 3:
                        continue  # pre-generated
                    scalar.wait_ge(s_c, gc - 2)  # WAR wt_sb vs combine
                    wt_gen(gc)

        @block.gpsimd
        def _(gpsimd):
            nreg = gpsimd.to_reg(NIDX)
            gpsimd.wait_ge(s_idx, 16)
            sv = AP(zpi, 0, [[ESTEP, nrows_ap], [1, ELEM]])
            for r in range(reps):
                for c in range(NC_):
                    gc = r * NC_ + c
                    if gc == 0:
                        gpsimd.wait_ge(s_zz, 16)
                    for pos in range(nch[c]):
                        gpsimd.wait_ge(s_zw[pos], 16 * (r + 1))
                    if gc >= 4:
                        gpsimd.wait_ge(s_c, gc - 3)  # WAR At vs combine
                    gpsimd.dma_gather(
                        AP(At[gc % 4], 0, [[GPC * ELEM, 128], [ELEM, GPC], [1, ELEM]]),
                        sv,
                        AP(idx_t, c * COLS, [[NROWS // 16, 128], [1, COLS]]),
                        NIDX, nreg, ELEM, elem_step=ESTEP,
                    ).then_inc(s_g[gc % 4], 16)

        @block.vector
        def _(vector):
            mult = mybir.AluOpType.mult
            VC = [0]

            def vsync(last_ins):
                # DVE pipeline does not interlock same-engine RAW hazards
                last_ins.then_inc(s_v, 1)
                VC[0] += 1
                vector.wait_ge(s_v, VC[0])

            vector.wait_ge(s_wf, 96)
            vector.memset(AP(mt[0], 0, [[GPC * N, 4], [1, 128]]), 0.0).then_inc(s_mz, 1)

            def zchunk(r, gi, g, v):
                zc = r * 16 + gi * 2 + v
                gcg = r * 8 + gi
                if zc >= 1:
                    vector.wait_ge(s_z, zc)  # WAR ztmp/pipeline drain
                vector.wait_ge(s_l[zc % 4], 16 * (zc // 4 + 1))
                if v == 0 and gcg >= 3:
                    # WAR zs[gcg%3] vs chunk gcg-3's stream + chunk gcg-2's
                    # h1 cross-read
                    vector.wait_ge(s_a, gcg - 1)
                    vector.wait_ge(s_zw[(gcg - 3) % NGRP], 16 * ((gcg - 3) // NGRP + 1))
                s = vt16[zc % 4]
                b = zs[gcg % 3]
                last_ins = None
                for (ks, ln, r0s, r1s, st) in runs:
                    zdst = AP(b, v * 256 + ks, [[ZSW, 128], [ESTEP, CH], [1, ln]])
                    tdst = AP(ztmp, ks, [[CH * N, 128], [N, CH], [1, ln]])
                    v0 = AP(s, r0s, [[CH * N, 128], [N, CH], [st, ln]])
                    v1 = AP(s, r1s, [[CH * N, 128], [N, CH], [st, ln]])
                    f0 = AP(fw_t[0], ks, [[128, 128], [0, CH], [1, ln]])
                    f1 = AP(fw_t[1], ks, [[128, 128], [0, CH], [1, ln]])
                    vector.tensor_tensor(zdst, v0, f0, mult)
                    last_ins = vector.tensor_tensor(tdst, v1, f1, mult)
                vsync(last_ins)
                for (ks, ln, r0s, r1s, st) in runs:
                    zdst = AP(b, v * 256 + ks, [[ZSW, 128], [ESTEP, CH], [1, ln]])
                    tdst = AP(ztmp, ks, [[CH * N, 128], [N, CH], [1, ln]])
                    last_ins = vector.tensor_add(zdst, zdst, tdst)
                last_ins.then_inc(s_z, 1)

            def combine(r, c):
                gc = r * NC_ + c
                if gc >= 1:
                    vector.wait_ge(s_c, gc)  # WAR mt vs prev combine
                vector.wait_ge(s_g[gc % 4], 16 * (gc // 4 + 1))
                vector.wait_ge(s_wl[gc % 3], 16 * (gc // 3 + 1))
                if gc >= 4:
                    vector.wait_ge(s_o[gc % 4], 16 * (gc // 4))  # WAR accb
                A = At[gc % 4]
                W = wt_sb[gc % 3]
                shp = [[GPC * ELEM, 128], [ELEM, GPC], [1, N]]
                oshp = [[GPC * N, 128], [N, GPC], [1, N]]

                def wb(ci):
                    return AP(W, ci * GPC * N, [[WD, 128], [N, GPC], [1, N]])
                maps = [AP(m, 0, oshp) for m in mt]
                # corner offsets within a gathered element (f16 elems):
                #   vol v: (p0,q0)=v*256, (p1,q0)=v*256+128,
                #          (p0,q1)=v*256+512, (p1,q1)=v*256+640
                last_ins = None
                for v in range(2):
                    b = 4 * v
                    vector.tensor_tensor(maps[b + 0], AP(A, v * 256 + 0, shp), wb(0), mult)
                    vector.tensor_tensor(maps[b + 1], AP(A, v * 256 + 128, shp), wb(2), mult)
                    vector.tensor_tensor(maps[b + 2], AP(A, v * 256 + 512, shp), wb(1), mult)
                    last_ins = vector.tensor_tensor(maps[b + 3], AP(A, v * 256 + 640, shp), wb(3), mult)
                vsync(last_ins)
                for v in range(2):
                    b = 4 * v
                    vector.tensor_add(maps[b + 0], maps[b + 0], maps[b + 1])
                    last_ins = vector.tensor_add(maps[b + 2], maps[b + 2], maps[b + 3])
                vsync(last_ins)
                osh2 = [[GPC * 256, 128], [256, GPC], [1, N]]
                vector.tensor_add(AP(accb[gc % 4], 0, osh2), maps[0], maps[2])
                vector.tensor_add(AP(accb[gc % 4], 128, osh2), maps[4], maps[6]) \
                    .then_inc(s_c, 1)

            # interleave the first combines into the z tail: their gathers
            # land while later chunks are still z-interping
            ileave = {(NGRP - 2, 1): [0], (NGRP - 1, 0): [1]}
            for r in range(reps):
                for gi, g in enumerate(chunk_order):
                    for v in range(2):
                        zchunk(r, gi, g, v)
                        for c in ileave.get((gi, v), []):
                            combine(r, c)
                for c in range(2, NC_):
                    combine(r, c)

    nc.compile()
    return nc


def _exact_label_fixup(label_g, theta, lab_f, out_bool):
    """Recompute voxels of |lab_f - 0.5| < FIX_EPS in the reference's exact
    f32 arithmetic order (validated bit-exact against the jax reference)."""
    eps = np.float32(FIX_EPS)
    cand = np.abs(lab_f - np.float32(0.5)) < eps
    if not cand.any():
        return out_bool
    bb, ii, jj, kk = np.nonzero(cand.reshape(-1, N, N, N))
    v = _exact_reference_values(label_g, theta, bb, ii, jj, kk)
    out_bool.reshape(-1, N, N, N)[bb, ii, jj, kk] = v > np.float32(0.5)
    return out_bool


def _exact_reference_values(vol_g, theta, bb, ii, jj, kk):
    """Reference-order f32 trilinear values at selected voxels.

    Replicates: grid einsum (x*t0 + y*t1 + z*t2, left-assoc f32) + t3; unnorm;
    8-corner accumulation in (z,y,x) order with w=(wz*wy)*wx, out += v*w.
    """
    f32 = np.float32
    t = np.arange(N, dtype=f32)
    xn = ((f32(2.0) * t + f32(1.0)) / f32(N) - f32(1.0)).astype(f32)
    th = theta.astype(f32)

    x = xn[ii]; y = xn[jj]; z = xn[kk]

    # f32 fma via f64 (exact up to negligible double-rounding corner cases)
    def fma32(a, b, c):
        return (np.float64(a) * np.float64(b) + c.astype(np.float64)).astype(f32)

    # grid components — XLA CPU lowers the einsum as an FMA chain (verified
    # bit-exact): fma(z, t2, fma(y, t1, x*t0)) + t3
    def comp(r):
        a = fma32(y, th[r, 1], (x * th[r, 0]).astype(f32))
        a = fma32(z, th[r, 2], a)
        return (a + th[r, 3]).astype(f32)
    gx, gy, gz = comp(0), comp(1), comp(2)

    def unnorm(c):
        return np.clip(((c + f32(1.0)) * f32(N) - f32(1.0)) * f32(0.5), f32(0.0), f32(N - 1))
    ux, uy, uz = unnorm(gx), unnorm(gy), unnorm(gz)
    x0 = np.floor(ux); y0 = np.floor(uy); z0 = np.floor(uz)
    fx = (ux - x0).astype(f32); fy = (uy - y0).astype(f32); fz = (uz - z0).astype(f32)
    x0i = x0.astype(np.int64); y0i = y0.astype(np.int64); z0i = z0.astype(np.int64)
    x1i = np.minimum(x0i + 1, N - 1); y1i = np.minimum(y0i + 1, N - 1); z1i = np.minimum(z0i + 1, N - 1)

    vol = vol_g.reshape(-1, N, N, N)
    out = np.zeros(bb.shape, f32)
    one = f32(1.0)
    for zi, wz in ((z0i, (one - fz).astype(f32)), (z1i, fz)):
        for yi, wy in ((y0i, (one - fy).astype(f32)), (y1i, fy)):
            for xi, wx in ((x0i, (one - fx).astype(f32)), (x1i, fx)):
                # inp[b, c, zi, yi, xi] in transposed space == vol[b, xi, yi, zi]
                vals = vol[bb, xi, yi, zi]
                w = ((wz * wy).astype(f32) * wx).astype(f32)
                out = (out + (vals * w).astype(f32)).astype(f32)
    return out


def _host_fallback(input_g, label_g, transform):
    """Arbitrary-transform fallback: full reference computation on host."""
    bb, ii, jj, kk = np.meshgrid(np.arange(8), np.arange(N), np.arange(N), np.arange(N), indexing="ij")
    bb, ii, jj, kk = (a.reshape(-1) for a in (bb, ii, jj, kk))
    theta = transform[:3].astype(np.float32)
    aug_inp = _exact_reference_values(input_g, theta, bb, ii, jj, kk).reshape(8, 1, N, N, N)
    lab = _exact_reference_values(label_g, theta, bb, ii, jj, kk).reshape(8, 1, N, N, N)
    return aug_inp.astype(np.float32), lab > np.float32(0.5)


def _make_inputs(tables, input_g, label_g):
    idx_p = _pack_idxs(tables["idxA"].reshape(-1))
    # per-corner weight tables [j, i]; k-replicated tiles are built on-device
    wts4 = np.stack([tables[nm].T.copy() for nm in ("w00", "w01", "w10", "w11")]).astype(np.float16)
    fwrep = np.stack([np.tile(1.0 - tables["fw"], (128, 1)),
                      np.tile(tables["fw"], (128, 1))]).astype(np.float16)
    in_maps = []
    for b in range(8):
        in_maps.append({
            # partition-major: vol[p, gc*128+k] = volume[row=gc*128+p, k]
            "vol0": np.ascontiguousarray(
                input_g[b, 0].reshape(128, 128, N).astype(np.float16).transpose(1, 0, 2).reshape(128, NROWS)),
            "vol1": np.ascontiguousarray(
                label_g[b, 0].reshape(128, 128, N).astype(np.float16).transpose(1, 0, 2).reshape(128, NROWS)),
            "idxA": idx_p, "wts4": wts4, "fwrep": fwrep,
        })
    return in_maps


def kernel(input_g, label_g, transform):
    input_g = np.ascontiguousarray(input_g, dtype=np.float32)
    label_g = np.ascontiguousarray(label_g, dtype=np.float32)
    transform = np.asarray(transform, dtype=np.float32)
    theta = transform[:3]

    structured = (abs(float(theta[0, 2])) < 1e-12 and abs(float(theta[1, 2])) < 1e-12
                  and abs(float(theta[2, 0])) < 1e-12 and abs(float(theta[2, 1])) < 1e-12)
    if not structured:
        return _host_fallback(input_g, label_g, transform)

    from concourse.bass_utils import run_bass_kernel_spmd

    tables = _host_tables(theta)
    key = transform.tobytes()
    if key not in _CACHE:
        _CACHE[key] = _build_program(tables)
    nc = _CACHE[key]

    in_maps = _make_inputs(tables, input_g, label_g)
    res = run_bass_kernel_spmd(nc, in_maps, core_ids=list(range(8)))

    aug_inp = np.empty((8, 1, N, N, N), np.float32)
    lab_f = np.empty((8, 1, N, N, N), np.float32)
    for b in range(8):
        oi = res.results[b]["outI"]
        aug_inp[b, 0] = oi[:, 0:128].astype(np.float32).reshape(N, N, N)
        lab_f[b, 0] = oi[:, 128:256].astype(np.float32).reshape(N, N, N)

    out_bool = lab_f > np.float32(0.5)
    out_bool = _exact_label_fixup(label_g, theta, lab_f, out_bool)
    return aug_inp, out_bool


# revision 41
# speedup vs baseline: 1.6606x; 1.0334x over previous
"""Trainium2 Bass kernel for SegmentationAugmentation (3D affine grid_sample, trilinear, border).

Contract: kernel(input_g, label_g, transform) -> (aug_inp f32 [8,1,128,128,128],
                                                  aug_lab bool [8,1,128,128,128])

Math (swapaxes folded into index bookkeeping; all spatial dims 128):

  out[b,c,i,j,k] = trilinear sample of input_g[b,c,:,:,:] at
      p-axis: U(i,j) = clip(64*(a00*xn(i)+a01*xn(j)+a03)+63.5, 0, 127)
      q-axis: V(i,j) = clip(64*(a10*xn(i)+a11*xn(j)+a13)+63.5, 0, 127)
      r-axis: W(k)   = clip(64*(a22*xn(k)+a23)+63.5, 0, 127)
  with xn(t) = (2t+1)/128 - 1, theta = transform[:3].  Relies on the
  generator's z-rotation structure (theta[0:2,2]==0, theta[2,0:2]==0); a
  pure-host fallback handles arbitrary transforms.

Device pipeline, data parallel over batch (core b handles batch b; each core
processes BOTH its image and label volume in one fused f16 program):

  Phase 1 (dense): load pre-transposed f16 volume chunks, z-interp on DVE via
  run-segmented staircase slices (f16, 2x perf mode), then DMA the z-interped
  rows into an interleaved pair layout in DRAM:
     ZPI[r = p*128+q] = [Z0(p,q) | Z0(p+1,q) | Z1(p,q) | Z1(p+1,q)]  (1 KiB)
  (Z0 = image, Z1 = label; each row is 128 f16 k-values.)

  Phase 2 (gather): for each output point (i,j), ONE dma_gather descriptor of
  2 KiB at entry r=(p0*128+q0) fetches entries r,r+1 = all four bilinear
  corners of BOTH volumes.  DVE combines with k-replicated f16 weight tiles
  (streamed from DRAM per call so every operand keeps innermost stride 1 and
  2-byte dtype -> DVE 2x perf mode); one 512B-descriptor DMA per call writes
  the interleaved f16 outputs of both volumes.

Host converts the f16 outputs to f32 / bool; label voxels within FIX_EPS of
0.5 are recomputed in the reference's exact f32 arithmetic order.
"""
import numpy as np

N = 128
NROWS = N * N            # 16384 (p,q) rows per volume
NIDX = 1024              # gather indices (output points) per dma_gather call
GPC = NIDX // 128        # 8 element groups per partition per call
NCALLS = NROWS // NIDX   # 16 gather calls per rep
COLS = NIDX // 16        # idx table columns per call
ELEM = 1024              # gathered f16 elems per descriptor (= 2 ZPI entries)
ESTEP = 512              # f16 elems per ZPI entry (gather elem_step)
CH = 16                  # 128-row groups per load chunk
NGRP = NROWS // N // CH  # 8 chunks per volume
ZG = 129                 # zt groups (128 data + 1 zero pad)
FIX_EPS = 8e-3           # |label-0.5| below this -> exact host recompute

_CACHE = {}


def _mkap(pairs):
    import bass_rust
    return bass_rust.VecI64Pair([tuple(p) for p in pairs])


def _host_tables(theta):
    """All transform-derived tables, computed in float64 from f32 theta."""
    th = theta.astype(np.float64)
    t = np.arange(N, dtype=np.float64)
    xn = (2.0 * t + 1.0) / N - 1.0

    U = np.clip(64.0 * (th[0, 0] * xn[:, None] + th[0, 1] * xn[None, :] + th[0, 3]) + 63.5, 0.0, 127.0)
    V = np.clip(64.0 * (th[1, 0] * xn[:, None] + th[1, 1] * xn[None, :] + th[1, 3]) + 63.5, 0.0, 127.0)
    W = np.clip(64.0 * (th[2, 2] * xn + th[2, 3]) + 63.5, 0.0, 127.0)

    p0 = np.floor(U).astype(np.int64)
    q0 = np.floor(V).astype(np.int64)
    r0 = np.floor(W).astype(np.int64)
    fu = (U - p0).astype(np.float32)
    fv = (V - q0).astype(np.float32)
    fw = (W - r0).astype(np.float32)
    r1 = np.minimum(r0 + 1, N - 1)

    idxA = (p0 * 128 + q0).astype(np.int16)          # [i,j] ZPI entry index
    w00 = ((1 - fu) * (1 - fv)).astype(np.float32)
    w10 = (fu * (1 - fv)).astype(np.float32)
    w01 = ((1 - fu) * fv).astype(np.float32)
    w11 = (fu * fv).astype(np.float32)

    # z-run decomposition: maximal segments where both r0 and r1 step by a
    # constant d in {-1,0,1}
    runs = []
    k = 0
    while k < N:
        step = 0
        if k + 1 < N:
            d = int(r0[k + 1] - r0[k])
            if d == int(r1[k + 1] - r1[k]) and d in (-1, 0, 1):
                step = d
        ln = 1
        while (k + ln < N
               and int(r0[k + ln] - r0[k]) == step * ln
               and int(r1[k + ln] - r1[k]) == step * ln):
            ln += 1
        runs.append((k, ln, int(r0[k]), int(r1[k]), step))
        k += ln

    return dict(idxA=idxA, w00=w00, w01=w01, w10=w10, w11=w11, fw=fw, runs=runs)


def _pack_idxs(idx_flat):
    """int16 dma_gather index layout: element i at [i%16, i//16], replicated to 128 partitions."""
    t = idx_flat.reshape(-1, 16).T.astype(np.int16)  # [16, n/16]
    return np.ascontiguousarray(np.tile(t, (8, 1)))  # [128, n/16]


def _chunk_plan(tables):
    """Stream chunks (2048 ZPI entries each) in the order matching the calls'
    p-band progression; per call, how many streamed chunks it needs."""
    idxA = tables["idxA"].reshape(-1).astype(np.int64)
    # call c covers points c*NIDX..(c+1)*NIDX-1; entries r and r+1 needed
    need = []
    for c in range(NCALLS):
        rs = idxA[c * NIDX:(c + 1) * NIDX]
        need.append((int(rs.min()) // (CH * 128), (int(rs.max()) + 1) // (CH * 128)))
    first_lo, _ = need[0]
    last_lo, _ = need[-1]
    descending = first_lo >= last_lo
    order = list(range(NGRP - 1, -1, -1)) if descending else list(range(NGRP))
    pos = {g: i for i, g in enumerate(order)}
    nch = [max(pos[min(lo, NGRP - 1)], pos[min(hi, NGRP - 1)]) + 1 for lo, hi in need]
    return order, nch


def _build_program(tables, reps=1):
    """Raw-Bass (explicit semaphore) program; see module docstring for the
    pipeline.  All cross-engine waits are standalone wait_ge instructions.

    Engine streams:
      sync   (SP HWDGE):  const/volume loads, per-chunk ZPI stream writes
      scalar (ACT):       h1-slot entry assembly copies; weight-tile loads and
                          interleaved output writes (HWDGE)
      vector (DVE):       z-interp into ZS entry layout, 4-corner combine
      gpsimd (SWDGE):     one dma_gather per 1024 output points, fired as
                          soon as the chunks its points touch are streamed
    """
    import concourse.bass as bass
    from concourse import bacc, mybir

    runs = tables["runs"]
    f16 = mybir.dt.float16
    i16 = mybir.dt.int16

    nc = bacc.Bacc("TRN2", target_bir_lowering=False, debug=False, num_devices=8)

    vol_in = [nc.dram_tensor(f"vol{v}", [128, NROWS], f16, kind="ExternalInput") for v in range(2)]
    idx_dram = nc.dram_tensor("idxA", [128, NROWS // 16], i16, kind="ExternalInput")
    wts4 = nc.dram_tensor("wts4", [4, 128, 128], f16, kind="ExternalInput")
    fwrep = nc.dram_tensor("fwrep", [2, 128, 128], f16, kind="ExternalInput")
    out_i = nc.dram_tensor("outI", [NROWS, 256], f16, kind="ExternalOutput")
    zpi = nc.dram_tensor("zpi", [NROWS + 1, ESTEP], f16, kind="Internal")

    AP = bass.AP

    WD = 4 * GPC * N  # packed weight tile width (4096)
    idx_t = nc.alloc_sbuf_tensor("idx_t", [128, NROWS // 16], i16)
    fw_t = [nc.alloc_sbuf_tensor(f"fw{c}_t", [128, 128], f16) for c in range(2)]
    w4_t = [nc.alloc_sbuf_tensor(f"w4_{c}", [128, 128], f16) for c in range(4)]
    wt_sb = [nc.alloc_sbuf_tensor(f"wt_{s}", [128, 4 * GPC * 256], f16) for s in range(2)]
    vt16 = [nc.alloc_sbuf_tensor(f"vt16_{s}", [128, CH * N], f16) for s in range(4)]
    zs = [nc.alloc_sbuf_tensor(f"zs{s}", [128, CH * ESTEP], f16) for s in range(3)]
    ztmp = nc.alloc_sbuf_tensor("ztmp", [128, CH * N], f16)
    At = [nc.alloc_sbuf_tensor(f"At{s}", [128, GPC * ELEM], f16) for s in range(4)]
    mt = [nc.alloc_sbuf_tensor(f"m{s}", [128, GPC * 256], f16) for s in range(4)]
    accb = [nc.alloc_sbuf_tensor(f"accb{s}", [128, GPC * 256], f16) for s in range(4)]

    nrows_ap = NROWS  # gather element at entry r reads entries r, r+1; r <= 16383
    NC_ = NCALLS
    ZSW = CH * ESTEP  # 8192
    chunk_order, nch = _chunk_plan(tables)
    # the h1 cross-chunk copy sources chunk g+1, which must already be in the
    # other ZS buffer -> chunks must stream top-down
    assert chunk_order == list(range(NGRP - 1, -1, -1)), chunk_order

    from contextlib import ExitStack
    with ExitStack() as _sctx:
        block = _sctx.enter_context(nc.Block())
        s_idx = _sctx.enter_context(nc.semaphore("s_idx"))
        s_wf = _sctx.enter_context(nc.semaphore("s_wf"))
        s_mz = _sctx.enter_context(nc.semaphore("s_mz"))
        s_l = [_sctx.enter_context(nc.semaphore(f"s_l{p}")) for p in range(4)]
        s_wl = [_sctx.enter_context(nc.semaphore(f"s_wl{p}")) for p in range(2)]
        s_z = _sctx.enter_context(nc.semaphore("s_z"))
        s_a = _sctx.enter_context(nc.semaphore("s_a"))
        s_zw = [_sctx.enter_context(nc.semaphore(f"s_zw{p}")) for p in range(NGRP)]
        s_zz = _sctx.enter_context(nc.semaphore("s_zz"))
        s_g = [_sctx.enter_context(nc.semaphore(f"s_g{p}")) for p in range(4)]
        s_c = _sctx.enter_context(nc.semaphore("s_c"))
        s_o = [_sctx.enter_context(nc.semaphore(f"s_o{p}")) for p in range(4)]
        s_v = _sctx.enter_context(nc.semaphore("s_v"))

        @block.sync
        def _(sync):
            for v in range(2):  # chunk 0 loads first: unblock DVE asap
                sync.dma_start(
                    AP(vt16[v], 0, [[CH * N, 128], [1, CH * N]]),
                    AP(vol_in[v], chunk_order[0] * CH * N, [[NROWS, 128], [1, CH * N]]),
                ).then_inc(s_l[v], 16)
            sync.dma_start(idx_t.ap(), idx_dram.ap()).then_inc(s_idx, 16)
            for c in range(2):
                sync.dma_start(fw_t[c].ap(), AP(fwrep, c * 128 * 128, [[128, 128], [1, 128]])).then_inc(s_wf, 16)
            for c in range(4):
                sync.dma_start(w4_t[c].ap(), AP(wts4, c * 128 * 128, [[128, 128], [1, 128]])).then_inc(s_wf, 16)
            # one-time: zero ZPI entry 16384 (read by gathers at r=16383)
            sync.wait_ge(s_mz, 1)
            sync.dma_start(
                AP(zpi, NROWS * ESTEP, [[128, 4], [1, 128]]),
                AP(mt[0], 0, [[GPC * 256, 4], [1, 128]]),
            ).then_inc(s_zz, 16)
            for r in range(reps):
                for gi, g in enumerate(chunk_order):
                    for v in range(2):
                        zc = r * 16 + gi * 2 + v
                        if r == 0 and gi == 0:
                            continue  # preloaded above
                        if zc >= 4:
                            sync.wait_ge(s_z, zc - 3)  # WAR vt16 vs z-interp
                        sync.dma_start(
                            AP(vt16[zc % 4], 0, [[CH * N, 128], [1, CH * N]]),
                            AP(vol_in[v], g * CH * N, [[NROWS, 128], [1, CH * N]]),
                        ).then_inc(s_l[zc % 4], 16)
                for c in range(NC_):
                    gc = r * NC_ + c
                    sync.wait_ge(s_c, gc + 1)
                    sync.dma_start(
                        AP(out_i, c * NIDX * 256, [[256, 128], [128 * 256, GPC], [1, 256]]),
                        AP(accb[gc % 4], 0, [[GPC * 256, 128], [256, GPC], [1, 256]]),
                    ).then_inc(s_o[gc % 4], 16)
            for p in range(4):
                sync.wait_ge(s_o[p], 16 * ((NC_ * reps - p + 3) // 4))

        @block.scalar
        def _(scalar):
            WD2 = 4 * GPC * 256

            def wt_gen(gc):
                # build the (v,k)-replicated weight tile for call gc%NC_:
                # wt[j, ci*2048 + slot*256 + vk] = w_ci(call*8+slot, j)
                c = gc % NC_
                for ci in range(4):
                    scalar.copy(
                        AP(wt_sb[gc % 2], ci * GPC * 256, [[WD2, 128], [256, GPC], [1, 256]]),
                        AP(w4_t[ci], c * GPC, [[128, 128], [1, GPC], [0, 256]]),
                    )
                # ACT write visibility to DVE: drain the pipeline before the
                # semaphore rises
                scalar.drain()
                scalar.sem_inc(s_wl[gc % 2], 16)

            scalar.wait_ge(s_wf, 96)
            for gc in range(min(2, NC_ * reps)):
                wt_gen(gc)
            for r in range(reps):
                for gi, g in enumerate(chunk_order):
                    # assemble h1 slots of chunk g: entry r gets row r+128,
                    # i.e. subgroup s copies from subgroup s+1 (h0 slot)
                    gcg = r * 8 + gi
                    scalar.wait_ge(s_z, r * 16 + 2 * gi + 2)
                    b = zs[gcg % 3]
                    scalar.copy(
                        AP(b, 256, [[ZSW, 128], [ESTEP, CH - 1], [1, 256]]),
                        AP(b, ESTEP, [[ZSW, 128], [ESTEP, CH - 1], [1, 256]]),
                    )
                    dst = AP(b, (CH - 1) * ESTEP + 256, [[ZSW, 128], [1, 256]])
                    if g == NGRP - 1:
                        # top chunk: rows >= 16384 are zero (border clamp
                        # gives these corners zero weight; keep finite)
                        last_ins = scalar.memzero(dst)
                    else:
                        src_b = zs[(gcg - 1) % 3]
                        last_ins = scalar.copy(dst, AP(src_b, 0, [[ZSW, 128], [1, 256]]))
                    last_ins.then_inc(s_a, 1)
                    # stream this chunk's assembled entries to ZPI; the ACT
                    # sequencer runs ahead of the engine pipeline, so fully
                    # drain the copies before the DMA reads the buffer
                    scalar.drain()
                    scalar.wait_ge(s_a, gcg + 1)
                    if gi == 0 and r >= 1:
                        scalar.wait_ge(s_c, NC_ * r)  # WAR zpi vs prev-rep gathers
                    scalar.dma_start(
                        AP(zpi, g * CH * 128 * ESTEP, [[ESTEP, 128], [128 * ESTEP, CH], [1, ESTEP]]),
                        AP(b, 0, [[ZSW, 128], [ESTEP, CH], [1, ESTEP]]),
                    ).then_inc(s_zw[gi], 16)
                    if gi == NGRP - 2:
                        gc2 = r * NC_ + 2
                        scalar.wait_ge(s_c, gc2 - 1)  # WAR wt_sb vs combine
                        wt_gen(gc2)
                gc3 = r * NC_ + 3
                if gc3 < NC_ * reps:
                    scalar.wait_ge(s_c, gc3 - 1)  # WAR wt_sb vs combine
                    wt_gen(gc3)
                for c in range(NC_):
                    gc2 = r * NC_ + c + 4
                    if gc2 >= NC_ * reps or gc2 % NC_ in (0, 1):
                        continue  # next rep's head tiles handled below
                    scalar.wait_ge(s_c, gc2 - 1)
                    wt_gen(gc2)
                for c in range(2):
                    gc2 = (r + 1) * NC_ + c
                    if gc2 < NC_ * reps:
                        scalar.wait_ge(s_c, gc2 - 1)
                        wt_gen(gc2)

        @block.gpsimd
        def _(gpsimd):
            nreg = gpsimd.to_reg(NIDX)
            gpsimd.wait_ge(s_idx, 16)
            sv = AP(zpi, 0, [[ESTEP, nrows_ap], [1, ELEM]])
            for r in range(reps):
                for c in range(NC_):
                    gc = r * NC_ + c
                    if gc == 0:
                        gpsimd.wait_ge(s_zz, 16)
                    for pos in range(nch[c]):
                        gpsimd.wait_ge(s_zw[pos], 16 * (r + 1))
                    if gc >= 4:
                        gpsimd.wait_ge(s_c, gc - 3)  # WAR At vs combine
                    gpsimd.dma_gather(
                        AP(At[gc % 4], 0, [[GPC * ELEM, 128], [ELEM, GPC], [1, ELEM]]),
                        sv,
                        AP(idx_t, c * COLS, [[NROWS // 16, 128], [1, COLS]]),
                        NIDX, nreg, ELEM, elem_step=ESTEP,
                    ).then_inc(s_g[gc % 4], 16)

        @block.vector
        def _(vector):
            mult = mybir.AluOpType.mult
            VC = [0]

            def vsync(last_ins):
                # DVE pipeline does not interlock same-engine RAW hazards
                last_ins.then_inc(s_v, 1)
                VC[0] += 1
                vector.wait_ge(s_v, VC[0])

            vector.wait_ge(s_wf, 96)
            vector.memset(AP(mt[0], 0, [[GPC * 256, 4], [1, 128]]), 0.0).then_inc(s_mz, 1)

            def zchunk(r, gi, g, v):
                zc = r * 16 + gi * 2 + v
                gcg = r * 8 + gi
                if zc >= 1:
                    vector.wait_ge(s_z, zc)  # WAR ztmp/pipeline drain
                vector.wait_ge(s_l[zc % 4], 16 * (zc // 4 + 1))
                if v == 0 and gcg >= 3:
                    # WAR zs[gcg%3] vs chunk gcg-3's stream + chunk gcg-2's
                    # h1 cross-read
                    vector.wait_ge(s_a, gcg - 1)
                    vector.wait_ge(s_zw[(gcg - 3) % NGRP], 16 * ((gcg - 3) // NGRP + 1))
                s = vt16[zc % 4]
                b = zs[gcg % 3]
                last_ins = None
                for (ks, ln, r0s, r1s, st) in runs:
                    zdst = AP(b, v * 128 + ks, [[ZSW, 128], [ESTEP, CH], [1, ln]])
                    tdst = AP(ztmp, ks, [[CH * N, 128], [N, CH], [1, ln]])
                    v0 = AP(s, r0s, [[CH * N, 128], [N, CH], [st, ln]])
                    v1 = AP(s, r1s, [[CH * N, 128], [N, CH], [st, ln]])
                    f0 = AP(fw_t[0], ks, [[128, 128], [0, CH], [1, ln]])
                    f1 = AP(fw_t[1], ks, [[128, 128], [0, CH], [1, ln]])
                    vector.tensor_tensor(zdst, v0, f0, mult)
                    last_ins = vector.tensor_tensor(tdst, v1, f1, mult)
                vsync(last_ins)
                for (ks, ln, r0s, r1s, st) in runs:
                    zdst = AP(b, v * 128 + ks, [[ZSW, 128], [ESTEP, CH], [1, ln]])
                    tdst = AP(ztmp, ks, [[CH * N, 128], [N, CH], [1, ln]])
                    last_ins = vector.tensor_add(zdst, zdst, tdst)
                last_ins.then_inc(s_z, 1)

            def combine(r, c):
                gc = r * NC_ + c
                if gc >= 1:
                    vector.wait_ge(s_c, gc)  # WAR mt vs prev combine
                vector.wait_ge(s_g[gc % 4], 16 * (gc // 4 + 1))
                vector.wait_ge(s_wl[gc % 2], 16 * (gc // 2 + 1))
                if gc >= 4:
                    vector.wait_ge(s_o[gc % 4], 16 * (gc // 4))  # WAR accb
                A = At[gc % 4]
                W = wt_sb[gc % 2]
                WD2 = 4 * GPC * 256
                shp = [[GPC * ELEM, 128], [ELEM, GPC], [1, 256]]
                oshp = [[GPC * 256, 128], [256, GPC], [1, 256]]

                def wb(ci):
                    return AP(W, ci * GPC * 256, [[WD2, 128], [256, GPC], [1, 256]])
                maps = [AP(m, 0, oshp) for m in mt]
                # corner offsets within a gathered element (f16 elems), both
                # volumes contiguous per corner:
                #   (p0,q0)=0, (p1,q0)=256, (p0,q1)=512, (p1,q1)=768
                vector.tensor_tensor(maps[0], AP(A, 0, shp), wb(0), mult)
                vector.tensor_tensor(maps[1], AP(A, 256, shp), wb(2), mult)
                vector.tensor_tensor(maps[2], AP(A, 512, shp), wb(1), mult)
                vsync(vector.tensor_tensor(maps[3], AP(A, 768, shp), wb(3), mult))
                vector.tensor_add(maps[0], maps[0], maps[1])
                vsync(vector.tensor_add(maps[2], maps[2], maps[3]))
                vector.tensor_add(AP(accb[gc % 4], 0, oshp), maps[0], maps[2]) \
                    .then_inc(s_c, 1)

            # interleave the first combines into the z tail: their gathers
            # land while later chunks are still z-interping
            ileave = {(NGRP - 2, 1): [0], (NGRP - 1, 0): [1]}
            for r in range(reps):
                for gi, g in enumerate(chunk_order):
                    for v in range(2):
                        zchunk(r, gi, g, v)
                        for c in ileave.get((gi, v), []):
                            combine(r, c)
                for c in range(2, NC_):
                    combine(r, c)

    nc.compile()
    return nc


def _exact_label_fixup(label_g, theta, lab_f, out_bool):
    """Recompute voxels of |lab_f - 0.5| < FIX_EPS in the reference's exact
    f32 arithmetic order (validated bit-exact against the jax reference)."""
    eps = np.float32(FIX_EPS)
    cand = np.abs(lab_f - np.float32(0.5)) < eps
    if not cand.any():
        return out_bool
    bb, ii, jj, kk = np.nonzero(cand.reshape(-1, N, N, N))
    v = _exact_reference_values(label_g, theta, bb, ii, jj, kk)
    out_bool.reshape(-1, N, N, N)[bb, ii, jj, kk] = v > np.float32(0.5)
    return out_bool


def _exact_reference_values(vol_g, theta, bb, ii, jj, kk):
    """Reference-order f32 trilinear values at selected voxels.

    Replicates: grid einsum (x*t0 + y*t1 + z*t2, left-assoc f32) + t3; unnorm;
    8-corner accumulation in (z,y,x) order with w=(wz*wy)*wx, out += v*w.
    """
    f32 = np.float32
    t = np.arange(N, dtype=f32)
    xn = ((f32(2.0) * t + f32(1.0)) / f32(N) - f32(1.0)).astype(f32)
    th = theta.astype(f32)

    x = xn[ii]; y = xn[jj]; z = xn[kk]

    # f32 fma via f64 (exact up to negligible double-rounding corner cases)
    def fma32(a, b, c):
        return (np.float64(a) * np.float64(b) + c.astype(np.float64)).astype(f32)

    # grid components — XLA CPU lowers the einsum as an FMA chain (verified
    # bit-exact): fma(z, t2, fma(y, t1, x*t0)) + t3
    def comp(r):
        a = fma32(y, th[r, 1], (x * th[r, 0]).astype(f32))
        a = fma32(z, th[r, 2], a)
        return (a + th[r, 3]).astype(f32)
    gx, gy, gz = comp(0), comp(1), comp(2)

    def unnorm(c):
        return np.clip(((c + f32(1.0)) * f32(N) - f32(1.0)) * f32(0.5), f32(0.0), f32(N - 1))
    ux, uy, uz = unnorm(gx), unnorm(gy), unnorm(gz)
    x0 = np.floor(ux); y0 = np.floor(uy); z0 = np.floor(uz)
    fx = (ux - x0).astype(f32); fy = (uy - y0).astype(f32); fz = (uz - z0).astype(f32)
    x0i = x0.astype(np.int64); y0i = y0.astype(np.int64); z0i = z0.astype(np.int64)
    x1i = np.minimum(x0i + 1, N - 1); y1i = np.minimum(y0i + 1, N - 1); z1i = np.minimum(z0i + 1, N - 1)

    vol = vol_g.reshape(-1, N, N, N)
    out = np.zeros(bb.shape, f32)
    one = f32(1.0)
    for zi, wz in ((z0i, (one - fz).astype(f32)), (z1i, fz)):
        for yi, wy in ((y0i, (one - fy).astype(f32)), (y1i, fy)):
            for xi, wx in ((x0i, (one - fx).astype(f32)), (x1i, fx)):
                # inp[b, c, zi, yi, xi] in transposed space == vol[b, xi, yi, zi]
                vals = vol[bb, xi, yi, zi]
                w = ((wz * wy).astype(f32) * wx).astype(f32)
                out = (out + (vals * w).astype(f32)).astype(f32)
    return out


def _host_fallback(input_g, label_g, transform):
    """Arbitrary-transform fallback: full reference computation on host."""
    bb, ii, jj, kk = np.meshgrid(np.arange(8), np.arange(N), np.arange(N), np.arange(N), indexing="ij")
    bb, ii, jj, kk = (a.reshape(-1) for a in (bb, ii, jj, kk))
    theta = transform[:3].astype(np.float32)
    aug_inp = _exact_reference_values(input_g, theta, bb, ii, jj, kk).reshape(8, 1, N, N, N)
    lab = _exact_reference_values(label_g, theta, bb, ii, jj, kk).reshape(8, 1, N, N, N)
    return aug_inp.astype(np.float32), lab > np.float32(0.5)


def _make_inputs(tables, input_g, label_g):
    idx_p = _pack_idxs(tables["idxA"].reshape(-1))
    # per-corner weight tables [j, i]; k-replicated tiles are built on-device
    wts4 = np.stack([tables[nm].T.copy() for nm in ("w00", "w01", "w10", "w11")]).astype(np.float16)
    fwrep = np.stack([np.tile(1.0 - tables["fw"], (128, 1)),
                      np.tile(tables["fw"], (128, 1))]).astype(np.float16)
    in_maps = []
    for b in range(8):
        in_maps.append({
            # partition-major: vol[p, gc*128+k] = volume[row=gc*128+p, k]
            "vol0": np.ascontiguousarray(
                input_g[b, 0].reshape(128, 128, N).astype(np.float16).transpose(1, 0, 2).reshape(128, NROWS)),
            "vol1": np.ascontiguousarray(
                label_g[b, 0].reshape(128, 128, N).astype(np.float16).transpose(1, 0, 2).reshape(128, NROWS)),
            "idxA": idx_p, "wts4": wts4, "fwrep": fwrep,
        })
    return in_maps


def kernel(input_g, label_g, transform):
    input_g = np.ascontiguousarray(input_g, dtype=np.float32)
    label_g = np.ascontiguousarray(label_g, dtype=np.float32)
    transform = np.asarray(transform, dtype=np.float32)
    theta = transform[:3]

    structured = (abs(float(theta[0, 2])) < 1e-12 and abs(float(theta[1, 2])) < 1e-12
                  and abs(float(theta[2, 0])) < 1e-12 and abs(float(theta[2, 1])) < 1e-12)
    if not structured:
        return _host_fallback(input_g, label_g, transform)

    from concourse.bass_utils import run_bass_kernel_spmd

    tables = _host_tables(theta)
    key = transform.tobytes()
    if key not in _CACHE:
        _CACHE[key] = _build_program(tables)
    nc = _CACHE[key]

    in_maps = _make_inputs(tables, input_g, label_g)
    res = run_bass_kernel_spmd(nc, in_maps, core_ids=list(range(8)))

    aug_inp = np.empty((8, 1, N, N, N), np.float32)
    lab_f = np.empty((8, 1, N, N, N), np.float32)
    for b in range(8):
        oi = res.results[b]["outI"]
        aug_inp[b, 0] = oi[:, 0:128].astype(np.float32).reshape(N, N, N)
        lab_f[b, 0] = oi[:, 128:256].astype(np.float32).reshape(N, N, N)

    out_bool = lab_f > np.float32(0.5)
    out_bool = _exact_label_fixup(label_g, theta, lab_f, out_bool)
    return aug_inp, out_bool


# revision 42
# speedup vs baseline: 1.6644x; 1.0023x over previous
"""Trainium2 Bass kernel for SegmentationAugmentation (3D affine grid_sample, trilinear, border).

Contract: kernel(input_g, label_g, transform) -> (aug_inp f32 [8,1,128,128,128],
                                                  aug_lab bool [8,1,128,128,128])

Math (swapaxes folded into index bookkeeping; all spatial dims 128):

  out[b,c,i,j,k] = trilinear sample of input_g[b,c,:,:,:] at
      p-axis: U(i,j) = clip(64*(a00*xn(i)+a01*xn(j)+a03)+63.5, 0, 127)
      q-axis: V(i,j) = clip(64*(a10*xn(i)+a11*xn(j)+a13)+63.5, 0, 127)
      r-axis: W(k)   = clip(64*(a22*xn(k)+a23)+63.5, 0, 127)
  with xn(t) = (2t+1)/128 - 1, theta = transform[:3].  Relies on the
  generator's z-rotation structure (theta[0:2,2]==0, theta[2,0:2]==0); a
  pure-host fallback handles arbitrary transforms.

Device pipeline, data parallel over batch (core b handles batch b; each core
processes BOTH its image and label volume in one fused f16 program):

  Phase 1 (dense): load pre-transposed f16 volume chunks, z-interp on DVE via
  run-segmented staircase slices (f16, 2x perf mode), then DMA the z-interped
  rows into an interleaved pair layout in DRAM:
     ZPI[r = p*128+q] = [Z0(p,q) | Z0(p+1,q) | Z1(p,q) | Z1(p+1,q)]  (1 KiB)
  (Z0 = image, Z1 = label; each row is 128 f16 k-values.)

  Phase 2 (gather): for each output point (i,j), ONE dma_gather descriptor of
  2 KiB at entry r=(p0*128+q0) fetches entries r,r+1 = all four bilinear
  corners of BOTH volumes.  DVE combines with k-replicated f16 weight tiles
  (streamed from DRAM per call so every operand keeps innermost stride 1 and
  2-byte dtype -> DVE 2x perf mode); one 512B-descriptor DMA per call writes
  the interleaved f16 outputs of both volumes.

Host converts the f16 outputs to f32 / bool; label voxels within FIX_EPS of
0.5 are recomputed in the reference's exact f32 arithmetic order.
"""
import numpy as np

N = 128
NROWS = N * N            # 16384 (p,q) rows per volume
NIDX = 1024              # gather indices (output points) per dma_gather call
GPC = NIDX // 128        # 8 element groups per partition per call
NCALLS = NROWS // NIDX   # 16 gather calls per rep
COLS = NIDX // 16        # idx table columns per call
ELEM = 1024              # gathered f16 elems per descriptor (= 2 ZPI entries)
ESTEP = 512              # f16 elems per ZPI entry (gather elem_step)
CH = 16                  # 128-row groups per load chunk
NGRP = NROWS // N // CH  # 8 chunks per volume
ZG = 129                 # zt groups (128 data + 1 zero pad)
FIX_EPS = 8e-3           # |label-0.5| below this -> exact host recompute

_CACHE = {}


def _mkap(pairs):
    import bass_rust
    return bass_rust.VecI64Pair([tuple(p) for p in pairs])


def _host_tables(theta):
    """All transform-derived tables, computed in float64 from f32 theta."""
    th = theta.astype(np.float64)
    t = np.arange(N, dtype=np.float64)
    xn = (2.0 * t + 1.0) / N - 1.0

    U = np.clip(64.0 * (th[0, 0] * xn[:, None] + th[0, 1] * xn[None, :] + th[0, 3]) + 63.5, 0.0, 127.0)
    V = np.clip(64.0 * (th[1, 0] * xn[:, None] + th[1, 1] * xn[None, :] + th[1, 3]) + 63.5, 0.0, 127.0)
    W = np.clip(64.0 * (th[2, 2] * xn + th[2, 3]) + 63.5, 0.0, 127.0)

    p0 = np.floor(U).astype(np.int64)
    q0 = np.floor(V).astype(np.int64)
    r0 = np.floor(W).astype(np.int64)
    fu = (U - p0).astype(np.float32)
    fv = (V - q0).astype(np.float32)
    fw = (W - r0).astype(np.float32)
    r1 = np.minimum(r0 + 1, N - 1)

    idxA = (p0 * 128 + q0).astype(np.int16)          # [i,j] ZPI entry index
    w00 = ((1 - fu) * (1 - fv)).astype(np.float32)
    w10 = (fu * (1 - fv)).astype(np.float32)
    w01 = ((1 - fu) * fv).astype(np.float32)
    w11 = (fu * fv).astype(np.float32)

    # z-run decomposition: maximal segments where both r0 and r1 step by a
    # constant d in {-1,0,1}
    runs = []
    k = 0
    while k < N:
        step = 0
        if k + 1 < N:
            d = int(r0[k + 1] - r0[k])
            if d == int(r1[k + 1] - r1[k]) and d in (-1, 0, 1):
                step = d
        ln = 1
        while (k + ln < N
               and int(r0[k + ln] - r0[k]) == step * ln
               and int(r1[k + ln] - r1[k]) == step * ln):
            ln += 1
        runs.append((k, ln, int(r0[k]), int(r1[k]), step))
        k += ln

    return dict(idxA=idxA, w00=w00, w01=w01, w10=w10, w11=w11, fw=fw, runs=runs)


def _pack_idxs(idx_flat):
    """int16 dma_gather index layout: element i at [i%16, i//16], replicated to 128 partitions."""
    t = idx_flat.reshape(-1, 16).T.astype(np.int16)  # [16, n/16]
    return np.ascontiguousarray(np.tile(t, (8, 1)))  # [128, n/16]


def _chunk_plan(tables):
    """Stream chunks (2048 ZPI entries each) in the order matching the calls'
    p-band progression; per call, how many streamed chunks it needs."""
    idxA = tables["idxA"].reshape(-1).astype(np.int64)
    # call c covers points c*NIDX..(c+1)*NIDX-1; entries r and r+1 needed
    need = []
    for c in range(NCALLS):
        rs = idxA[c * NIDX:(c + 1) * NIDX]
        need.append((int(rs.min()) // (CH * 128), (int(rs.max()) + 1) // (CH * 128)))
    first_lo, _ = need[0]
    last_lo, _ = need[-1]
    descending = first_lo >= last_lo
    order = list(range(NGRP - 1, -1, -1)) if descending else list(range(NGRP))
    pos = {g: i for i, g in enumerate(order)}
    nch = [max(pos[min(lo, NGRP - 1)], pos[min(hi, NGRP - 1)]) + 1 for lo, hi in need]
    return order, nch


def _build_program(tables, reps=1):
    """Raw-Bass (explicit semaphore) program; see module docstring for the
    pipeline.  All cross-engine waits are standalone wait_ge instructions.

    Engine streams:
      sync   (SP HWDGE):  const/volume loads, per-chunk ZPI stream writes
      scalar (ACT):       h1-slot entry assembly copies; weight-tile loads and
                          interleaved output writes (HWDGE)
      vector (DVE):       z-interp into ZS entry layout, 4-corner combine
      gpsimd (SWDGE):     one dma_gather per 1024 output points, fired as
                          soon as the chunks its points touch are streamed
    """
    import concourse.bass as bass
    from concourse import bacc, mybir

    runs = tables["runs"]
    f16 = mybir.dt.float16
    i16 = mybir.dt.int16

    nc = bacc.Bacc("TRN2", target_bir_lowering=False, debug=False, num_devices=8)

    vol_in = [nc.dram_tensor(f"vol{v}", [128, NROWS], f16, kind="ExternalInput") for v in range(2)]
    idx_dram = nc.dram_tensor("idxA", [128, NROWS // 16], i16, kind="ExternalInput")
    wts4 = nc.dram_tensor("wts4", [4, 128, 128], f16, kind="ExternalInput")
    fwrep = nc.dram_tensor("fwrep", [2, 128, 128], f16, kind="ExternalInput")
    out_i = nc.dram_tensor("outI", [NROWS, 256], f16, kind="ExternalOutput")
    zpi = nc.dram_tensor("zpi", [NROWS + 1, ESTEP], f16, kind="Internal")

    AP = bass.AP

    WD = 4 * GPC * N  # packed weight tile width (4096)
    idx_t = nc.alloc_sbuf_tensor("idx_t", [128, NROWS // 16], i16)
    fw_t = [nc.alloc_sbuf_tensor(f"fw{c}_t", [128, 128], f16) for c in range(2)]
    w4_t = [nc.alloc_sbuf_tensor(f"w4_{c}", [128, 128], f16) for c in range(4)]
    wt_sb = [nc.alloc_sbuf_tensor(f"wt_{s}", [128, WD], f16) for s in range(3)]
    vt16 = [nc.alloc_sbuf_tensor(f"vt16_{s}", [128, CH * N], f16) for s in range(4)]
    zs = [nc.alloc_sbuf_tensor(f"zs{s}", [128, CH * ESTEP], f16) for s in range(3)]
    ztmp = nc.alloc_sbuf_tensor("ztmp", [128, CH * N], f16)
    At = [nc.alloc_sbuf_tensor(f"At{s}", [128, GPC * ELEM], f16) for s in range(4)]
    mt = [nc.alloc_sbuf_tensor(f"m{s}", [128, GPC * N], f16) for s in range(8)]
    accb = [nc.alloc_sbuf_tensor(f"accb{s}", [128, GPC * 256], f16) for s in range(4)]

    nrows_ap = NROWS  # gather element at entry r reads entries r, r+1; r <= 16383
    NC_ = NCALLS
    ZSW = CH * ESTEP  # 8192
    chunk_order, nch = _chunk_plan(tables)
    # the h1 cross-chunk copy sources chunk g+1, which must already be in the
    # other ZS buffer -> chunks must stream top-down
    assert chunk_order == list(range(NGRP - 1, -1, -1)), chunk_order

    from contextlib import ExitStack
    with ExitStack() as _sctx:
        block = _sctx.enter_context(nc.Block())
        s_idx = _sctx.enter_context(nc.semaphore("s_idx"))
        s_wf = _sctx.enter_context(nc.semaphore("s_wf"))
        s_mz = _sctx.enter_context(nc.semaphore("s_mz"))
        s_l = [_sctx.enter_context(nc.semaphore(f"s_l{p}")) for p in range(4)]
        s_wl = [_sctx.enter_context(nc.semaphore(f"s_wl{p}")) for p in range(3)]
        s_z = _sctx.enter_context(nc.semaphore("s_z"))
        s_a = _sctx.enter_context(nc.semaphore("s_a"))
        s_zw = [_sctx.enter_context(nc.semaphore(f"s_zw{p}")) for p in range(NGRP)]
        s_zz = _sctx.enter_context(nc.semaphore("s_zz"))
        s_g = [_sctx.enter_context(nc.semaphore(f"s_g{p}")) for p in range(4)]
        s_c = _sctx.enter_context(nc.semaphore("s_c"))
        s_o = [_sctx.enter_context(nc.semaphore(f"s_o{p}")) for p in range(4)]
        s_v = _sctx.enter_context(nc.semaphore("s_v"))

        @block.sync
        def _(sync):
            for v in range(2):  # chunk 0 loads first: unblock DVE asap
                sync.dma_start(
                    AP(vt16[v], 0, [[CH * N, 128], [1, CH * N]]),
                    AP(vol_in[v], chunk_order[0] * CH * N, [[NROWS, 128], [1, CH * N]]),
                ).then_inc(s_l[v], 16)
            sync.dma_start(idx_t.ap(), idx_dram.ap()).then_inc(s_idx, 16)
            for c in range(2):
                sync.dma_start(fw_t[c].ap(), AP(fwrep, c * 128 * 128, [[128, 128], [1, 128]])).then_inc(s_wf, 16)
            for c in range(4):
                sync.dma_start(w4_t[c].ap(), AP(wts4, c * 128 * 128, [[128, 128], [1, 128]])).then_inc(s_wf, 16)
            # one-time: zero ZPI entry 16384 (read by gathers at r=16383)
            sync.wait_ge(s_mz, 1)
            sync.dma_start(
                AP(zpi, NROWS * ESTEP, [[128, 4], [1, 128]]),
                AP(mt[0], 0, [[GPC * N, 4], [1, 128]]),
            ).then_inc(s_zz, 16)
            for r in range(reps):
                for gi, g in enumerate(chunk_order):
                    for v in range(2):
                        zc = r * 16 + gi * 2 + v
                        if r == 0 and gi == 0:
                            continue  # preloaded above
                        if zc >= 4:
                            sync.wait_ge(s_z, zc - 3)  # WAR vt16 vs z-interp
                        sync.dma_start(
                            AP(vt16[zc % 4], 0, [[CH * N, 128], [1, CH * N]]),
                            AP(vol_in[v], g * CH * N, [[NROWS, 128], [1, CH * N]]),
                        ).then_inc(s_l[zc % 4], 16)
                for c in range(NC_):
                    gc = r * NC_ + c
                    sync.wait_ge(s_c, gc + 1)
                    sync.dma_start(
                        AP(out_i, c * NIDX * 256, [[256, 128], [128 * 256, GPC], [1, 256]]),
                        AP(accb[gc % 4], 0, [[GPC * 256, 128], [256, GPC], [1, 256]]),
                    ).then_inc(s_o[gc % 4], 16)
            for p in range(4):
                sync.wait_ge(s_o[p], 16 * ((NC_ * reps - p + 3) // 4))

        @block.scalar
        def _(scalar):
            def wt_gen(gc):
                # build the k-replicated weight tile for call gc%NC_ on-chip:
                # wt[j, ci*1024 + slot*128 + k] = w_ci(call*8+slot, j)
                c = gc % NC_
                for ci in range(4):
                    scalar.copy(
                        AP(wt_sb[gc % 3], ci * GPC * N, [[WD, 128], [N, GPC], [1, N]]),
                        AP(w4_t[ci], c * GPC, [[128, 128], [1, GPC], [0, N]]),
                    )
                # ACT write visibility to DVE: drain the pipeline before the
                # semaphore rises
                scalar.drain()
                scalar.sem_inc(s_wl[gc % 3], 16)

            scalar.wait_ge(s_wf, 96)
            for gc in range(min(3, NC_ * reps)):
                wt_gen(gc)
            for r in range(reps):
                for gi, g in enumerate(chunk_order):
                    # assemble h1 slots of chunk g: entry r gets row r+128,
                    # i.e. subgroup s copies from subgroup s+1 (h0 slot)
                    gcg = r * 8 + gi
                    scalar.wait_ge(s_z, r * 16 + 2 * gi + 2)
                    b = zs[gcg % 3]
                    for v in range(2):
                        scalar.copy(
                            AP(b, v * 256 + 128, [[ZSW, 128], [ESTEP, CH - 1], [1, N]]),
                            AP(b, ESTEP + v * 256, [[ZSW, 128], [ESTEP, CH - 1], [1, N]]),
                        )
                    last_ins = None
                    for v in range(2):
                        dst = AP(b, (CH - 1) * ESTEP + v * 256 + 128, [[ZSW, 128], [1, N]])
                        if g == NGRP - 1:
                            # top chunk: rows >= 16384 are zero (border clamp
                            # gives these corners zero weight; keep finite)
                            last_ins = scalar.memzero(dst)
                        else:
                            src_b = zs[(gcg - 1) % 3]
                            last_ins = scalar.copy(dst, AP(src_b, v * 256, [[ZSW, 128], [1, N]]))
                    last_ins.then_inc(s_a, 1)
                    # stream this chunk's assembled entries to ZPI; the ACT
                    # sequencer runs ahead of the engine pipeline, so fully
                    # drain the copies before the DMA reads the buffer
                    scalar.drain()
                    scalar.wait_ge(s_a, gcg + 1)
                    if gi == 0 and r >= 1:
                        scalar.wait_ge(s_c, NC_ * r)  # WAR zpi vs prev-rep gathers
                    scalar.dma_start(
                        AP(zpi, g * CH * 128 * ESTEP, [[ESTEP, 128], [128 * ESTEP, CH], [1, ESTEP]]),
                        AP(b, 0, [[ZSW, 128], [ESTEP, CH], [1, ESTEP]]),
                    ).then_inc(s_zw[gi], 16)
                for c in range(NC_):
                    gc = r * NC_ + c
                    if gc < 3:
                        continue  # pre-generated
                    scalar.wait_ge(s_c, gc - 2)  # WAR wt_sb vs combine
                    wt_gen(gc)

        @block.gpsimd
        def _(gpsimd):
            nreg = gpsimd.to_reg(NIDX)
            gpsimd.wait_ge(s_idx, 16)
            sv = AP(zpi, 0, [[ESTEP, nrows_ap], [1, ELEM]])
            for r in range(reps):
                for c in range(NC_):
                    gc = r * NC_ + c
                    if gc == 0:
                        gpsimd.wait_ge(s_zz, 16)
                    for pos in range(nch[c]):
                        gpsimd.wait_ge(s_zw[pos], 16 * (r + 1))
                    if gc >= 4:
                        gpsimd.wait_ge(s_c, gc - 3)  # WAR At vs combine
                    gpsimd.dma_gather(
                        AP(At[gc % 4], 0, [[GPC * ELEM, 128], [ELEM, GPC], [1, ELEM]]),
                        sv,
                        AP(idx_t, c * COLS, [[NROWS // 16, 128], [1, COLS]]),
                        NIDX, nreg, ELEM, elem_step=ESTEP,
                    ).then_inc(s_g[gc % 4], 16)

        @block.vector
        def _(vector):
            mult = mybir.AluOpType.mult
            VC = [0]

            def vsync(last_ins):
                # DVE pipeline does not interlock same-engine RAW hazards
                last_ins.then_inc(s_v, 1)
                VC[0] += 1
                vector.wait_ge(s_v, VC[0])

            vector.wait_ge(s_wf, 96)
            vector.memset(AP(mt[0], 0, [[GPC * N, 4], [1, 128]]), 0.0).then_inc(s_mz, 1)

            def zchunk(r, gi, g, v):
                zc = r * 16 + gi * 2 + v
                gcg = r * 8 + gi
                if zc >= 1:
                    vector.wait_ge(s_z, zc)  # WAR ztmp/pipeline drain
                vector.wait_ge(s_l[zc % 4], 16 * (zc // 4 + 1))
                if v == 0 and gcg >= 3:
                    # WAR zs[gcg%3] vs chunk gcg-3's stream + chunk gcg-2's
                    # h1 cross-read
                    vector.wait_ge(s_a, gcg - 1)
                    vector.wait_ge(s_zw[(gcg - 3) % NGRP], 16 * ((gcg - 3) // NGRP + 1))
                s = vt16[zc % 4]
                b = zs[gcg % 3]
                last_ins = None
                for (ks, ln, r0s, r1s, st) in runs:
                    zdst = AP(b, v * 256 + ks, [[ZSW, 128], [ESTEP, CH], [1, ln]])
                    tdst = AP(ztmp, ks, [[CH * N, 128], [N, CH], [1, ln]])
                    v0 = AP(s, r0s, [[CH * N, 128], [N, CH], [st, ln]])
                    v1 = AP(s, r1s, [[CH * N, 128], [N, CH], [st, ln]])
                    f0 = AP(fw_t[0], ks, [[128, 128], [0, CH], [1, ln]])
                    f1 = AP(fw_t[1], ks, [[128, 128], [0, CH], [1, ln]])
                    vector.tensor_tensor(zdst, v0, f0, mult)
                    last_ins = vector.tensor_tensor(tdst, v1, f1, mult)
                vsync(last_ins)
                for (ks, ln, r0s, r1s, st) in runs:
                    zdst = AP(b, v * 256 + ks, [[ZSW, 128], [ESTEP, CH], [1, ln]])
                    tdst = AP(ztmp, ks, [[CH * N, 128], [N, CH], [1, ln]])
                    last_ins = vector.tensor_add(zdst, zdst, tdst)
                last_ins.then_inc(s_z, 1)

            def combine(r, c):
                gc = r * NC_ + c
                if gc >= 1:
                    vector.wait_ge(s_c, gc)  # WAR mt vs prev combine
                vector.wait_ge(s_g[gc % 4], 16 * (gc // 4 + 1))
                vector.wait_ge(s_wl[gc % 3], 16 * (gc // 3 + 1))
                if gc >= 4:
                    vector.wait_ge(s_o[gc % 4], 16 * (gc // 4))  # WAR accb
                A = At[gc % 4]
                W = wt_sb[gc % 3]
                shp = [[GPC * ELEM, 128], [ELEM, GPC], [1, N]]
                oshp = [[GPC * N, 128], [N, GPC], [1, N]]

                def wb(ci):
                    return AP(W, ci * GPC * N, [[WD, 128], [N, GPC], [1, N]])
                maps = [AP(m, 0, oshp) for m in mt]
                # corner offsets within a gathered element (f16 elems):
                #   vol v: (p0,q0)=v*256, (p1,q0)=v*256+128,
                #          (p0,q1)=v*256+512, (p1,q1)=v*256+640
                last_ins = None
                for v in range(2):
                    b = 4 * v
                    vector.tensor_tensor(maps[b + 0], AP(A, v * 256 + 0, shp), wb(0), mult)
                    vector.tensor_tensor(maps[b + 1], AP(A, v * 256 + 128, shp), wb(2), mult)
                    vector.tensor_tensor(maps[b + 2], AP(A, v * 256 + 512, shp), wb(1), mult)
                    last_ins = vector.tensor_tensor(maps[b + 3], AP(A, v * 256 + 640, shp), wb(3), mult)
                vsync(last_ins)
                for v in range(2):
                    b = 4 * v
                    vector.tensor_add(maps[b + 0], maps[b + 0], maps[b + 1])
                    last_ins = vector.tensor_add(maps[b + 2], maps[b + 2], maps[b + 3])
                vsync(last_ins)
                osh2 = [[GPC * 256, 128], [256, GPC], [1, N]]
                vector.tensor_add(AP(accb[gc % 4], 0, osh2), maps[0], maps[2])
                vector.tensor_add(AP(accb[gc % 4], 128, osh2), maps[4], maps[6]) \
                    .then_inc(s_c, 1)

            # interleave the first combines into the z tail: their gathers
            # land while later chunks are still z-interping
            ileave = {(NGRP - 2, 1): [0], (NGRP - 1, 0): [1]}
            for r in range(reps):
                for gi, g in enumerate(chunk_order):
                    for v in range(2):
                        zchunk(r, gi, g, v)
                        for c in ileave.get((gi, v), []):
                            combine(r, c)
                for c in range(2, NC_):
                    combine(r, c)

    nc.compile()
    return nc


def _exact_label_fixup(label_g, theta, lab_f, out_bool):
    """Recompute voxels of |lab_f - 0.5| < FIX_EPS in the reference's exact
    f32 arithmetic order (validated bit-exact against the jax reference)."""
    eps = np.float32(FIX_EPS)
    cand = np.abs(lab_f - np.float32(0.5)) < eps
    if not cand.any():
        return out_bool
    bb, ii, jj, kk = np.nonzero(cand.reshape(-1, N, N, N))
    v = _exact_reference_values(label_g, theta, bb, ii, jj, kk)
    out_bool.reshape(-1, N, N, N)[bb, ii, jj, kk] = v > np.float32(0.5)
    return out_bool


def _exact_reference_values(vol_g, theta, bb, ii, jj, kk):
    """Reference-order f32 trilinear values at selected voxels.

    Replicates: grid einsum (x*t0 + y*t1 + z*t2, left-assoc f32) + t3; unnorm;
    8-corner accumulation in (z,y,x) order with w=(wz*wy)*wx, out += v*w.
    """
    f32 = np.float32
    t = np.arange(N, dtype=f32)
    xn = ((f32(2.0) * t + f32(1.0)) / f32(N) - f32(1.0)).astype(f32)
    th = theta.astype(f32)

    x = xn[ii]; y = xn[jj]; z = xn[kk]

    # f32 fma via f64 (exact up to negligible double-rounding corner cases)
    def fma32(a, b, c):
        return (np.float64(a) * np.float64(b) + c.astype(np.float64)).astype(f32)

    # grid components — XLA CPU lowers the einsum as an FMA chain (verified
    # bit-exact): fma(z, t2, fma(y, t1, x*t0)) + t3
    def comp(r):
        a = fma32(y, th[r, 1], (x * th[r, 0]).astype(f32))
        a = fma32(z, th[r, 2], a)
        return (a + th[r, 3]).astype(f32)
    gx, gy, gz = comp(0), comp(1), comp(2)

    def unnorm(c):
        return np.clip(((c + f32(1.0)) * f32(N) - f32(1.0)) * f32(0.5), f32(0.0), f32(N - 1))
    ux, uy, uz = unnorm(gx), unnorm(gy), unnorm(gz)
    x0 = np.floor(ux); y0 = np.floor(uy); z0 = np.floor(uz)
    fx = (ux - x0).astype(f32); fy = (uy - y0).astype(f32); fz = (uz - z0).astype(f32)
    x0i = x0.astype(np.int64); y0i = y0.astype(np.int64); z0i = z0.astype(np.int64)
    x1i = np.minimum(x0i + 1, N - 1); y1i = np.minimum(y0i + 1, N - 1); z1i = np.minimum(z0i + 1, N - 1)

    vol = vol_g.reshape(-1, N, N, N)
    out = np.zeros(bb.shape, f32)
    one = f32(1.0)
    for zi, wz in ((z0i, (one - fz).astype(f32)), (z1i, fz)):
        for yi, wy in ((y0i, (one - fy).astype(f32)), (y1i, fy)):
            for xi, wx in ((x0i, (one - fx).astype(f32)), (x1i, fx)):
                # inp[b, c, zi, yi, xi] in transposed space == vol[b, xi, yi, zi]
                vals = vol[bb, xi, yi, zi]
                w = ((wz * wy).astype(f32) * wx).astype(f32)
                out = (out + (vals * w).astype(f32)).astype(f32)
    return out


def _host_fallback(input_g, label_g, transform):
    """Arbitrary-transform fallback: full reference computation on host."""
    bb, ii, jj, kk = np.meshgrid(np.arange(8), np.arange(N), np.arange(N), np.arange(N), indexing="ij")
    bb, ii, jj, kk = (a.reshape(-1) for a in (bb, ii, jj, kk))
    theta = transform[:3].astype(np.float32)
    aug_inp = _exact_reference_values(input_g, theta, bb, ii, jj, kk).reshape(8, 1, N, N, N)
    lab = _exact_reference_values(label_g, theta, bb, ii, jj, kk).reshape(8, 1, N, N, N)
    return aug_inp.astype(np.float32), lab > np.float32(0.5)


def _make_inputs(tables, input_g, label_g):
    idx_p = _pack_idxs(tables["idxA"].reshape(-1))
    # per-corner weight tables [j, i]; k-replicated tiles are built on-device
    wts4 = np.stack([tables[nm].T.copy() for nm in ("w00", "w01", "w10", "w11")]).astype(np.float16)
    fwrep = np.stack([np.tile(1.0 - tables["fw"], (128, 1)),
                      np.tile(tables["fw"], (128, 1))]).astype(np.float16)
    in_maps = []
    for b in range(8):
        in_maps.append({
            # partition-major: vol[p, gc*128+k] = volume[row=gc*128+p, k]
            "vol0": np.ascontiguousarray(
                input_g[b, 0].reshape(128, 128, N).astype(np.float16).transpose(1, 0, 2).reshape(128, NROWS)),
            "vol1": np.ascontiguousarray(
                label_g[b, 0].reshape(128, 128, N).astype(np.float16).transpose(1, 0, 2).reshape(128, NROWS)),
            "idxA": idx_p, "wts4": wts4, "fwrep": fwrep,
        })
    return in_maps


def kernel(input_g, label_g, transform):
    input_g = np.ascontiguousarray(input_g, dtype=np.float32)
    label_g = np.ascontiguousarray(label_g, dtype=np.float32)
    transform = np.asarray(transform, dtype=np.float32)
    theta = transform[:3]

    structured = (abs(float(theta[0, 2])) < 1e-12 and abs(float(theta[1, 2])) < 1e-12
                  and abs(float(theta[2, 0])) < 1e-12 and abs(float(theta[2, 1])) < 1e-12)
    if not structured:
        return _host_fallback(input_g, label_g, transform)

    from concourse.bass_utils import run_bass_kernel_spmd

    tables = _host_tables(theta)
    key = transform.tobytes()
    if key not in _CACHE:
        _CACHE[key] = _build_program(tables)
    nc = _CACHE[key]

    in_maps = _make_inputs(tables, input_g, label_g)
    res = run_bass_kernel_spmd(nc, in_maps, core_ids=list(range(8)))

    aug_inp = np.empty((8, 1, N, N, N), np.float32)
    lab_f = np.empty((8, 1, N, N, N), np.float32)
    for b in range(8):
        oi = res.results[b]["outI"]
        aug_inp[b, 0] = oi[:, 0:128].astype(np.float32).reshape(N, N, N)
        lab_f[b, 0] = oi[:, 128:256].astype(np.float32).reshape(N, N, N)

    out_bool = lab_f > np.float32(0.5)
    out_bool = _exact_label_fixup(label_g, theta, lab_f, out_bool)
    return aug_inp, out_bool


# revision 44
# speedup vs baseline: 1.6991x; 1.0209x over previous
"""Trainium2 Bass kernel for SegmentationAugmentation (3D affine grid_sample, trilinear, border).

Contract: kernel(input_g, label_g, transform) -> (aug_inp f32 [8,1,128,128,128],
                                                  aug_lab bool [8,1,128,128,128])

Math (swapaxes folded into index bookkeeping; all spatial dims 128):

  out[b,c,i,j,k] = trilinear sample of input_g[b,c,:,:,:] at
      p-axis: U(i,j) = clip(64*(a00*xn(i)+a01*xn(j)+a03)+63.5, 0, 127)
      q-axis: V(i,j) = clip(64*(a10*xn(i)+a11*xn(j)+a13)+63.5, 0, 127)
      r-axis: W(k)   = clip(64*(a22*xn(k)+a23)+63.5, 0, 127)
  with xn(t) = (2t+1)/128 - 1, theta = transform[:3].  Relies on the
  generator's z-rotation structure (theta[0:2,2]==0, theta[2,0:2]==0); a
  pure-host fallback handles arbitrary transforms.

Device pipeline, data parallel over batch (core b handles batch b; each core
processes BOTH its image and label volume in one fused f16 program):

  Phase 1 (dense): load pre-transposed f16 volume chunks, z-interp on DVE via
  run-segmented staircase slices (f16, 2x perf mode), then DMA the z-interped
  rows into an interleaved pair layout in DRAM:
     ZPI[r = p*128+q] = [Z0(p,q) | Z0(p+1,q) | Z1(p,q) | Z1(p+1,q)]  (1 KiB)
  (Z0 = image, Z1 = label; each row is 128 f16 k-values.)

  Phase 2 (gather): for each output point (i,j), ONE dma_gather descriptor of
  2 KiB at entry r=(p0*128+q0) fetches entries r,r+1 = all four bilinear
  corners of BOTH volumes.  DVE combines with k-replicated f16 weight tiles
  (generated on-chip by the ACT engine per call so every operand keeps
  innermost stride 1 and 2-byte dtype -> DVE 2x perf mode); one
  512B-descriptor DMA per call writes the interleaved f16 outputs of both
  volumes.  ZPI chunks are streamed top-down and each gather fires as soon
  as the chunks its points touch are resident, overlapping the two phases.

Host converts the f16 outputs to f32 / bool; label voxels within FIX_EPS of
0.5 are recomputed in the reference's exact f32 arithmetic order.
"""
import numpy as np

N = 128
NROWS = N * N            # 16384 (p,q) rows per volume
NIDX = 1024              # gather indices (output points) per dma_gather call
GPC = NIDX // 128        # 8 element groups per partition per call
NCALLS = NROWS // NIDX   # 16 gather calls per rep
COLS = NIDX // 16        # idx table columns per call
ELEM = 1024              # gathered f16 elems per descriptor (= 2 ZPI entries)
ESTEP = 512              # f16 elems per ZPI entry (gather elem_step)
CH = 16                  # 128-row groups per load chunk
NGRP = NROWS // N // CH  # 8 chunks per volume
ZG = 129                 # zt groups (128 data + 1 zero pad)
FIX_EPS = 8e-3           # |label-0.5| below this -> exact host recompute

_CACHE = {}


def _mkap(pairs):
    import bass_rust
    return bass_rust.VecI64Pair([tuple(p) for p in pairs])


def _host_tables(theta):
    """All transform-derived tables, computed in float64 from f32 theta."""
    th = theta.astype(np.float64)
    t = np.arange(N, dtype=np.float64)
    xn = (2.0 * t + 1.0) / N - 1.0

    U = np.clip(64.0 * (th[0, 0] * xn[:, None] + th[0, 1] * xn[None, :] + th[0, 3]) + 63.5, 0.0, 127.0)
    V = np.clip(64.0 * (th[1, 0] * xn[:, None] + th[1, 1] * xn[None, :] + th[1, 3]) + 63.5, 0.0, 127.0)
    W = np.clip(64.0 * (th[2, 2] * xn + th[2, 3]) + 63.5, 0.0, 127.0)

    p0 = np.floor(U).astype(np.int64)
    q0 = np.floor(V).astype(np.int64)
    r0 = np.floor(W).astype(np.int64)
    fu = (U - p0).astype(np.float32)
    fv = (V - q0).astype(np.float32)
    fw = (W - r0).astype(np.float32)
    r1 = np.minimum(r0 + 1, N - 1)

    idxA = (p0 * 128 + q0).astype(np.int16)          # [i,j] ZPI entry index
    w00 = ((1 - fu) * (1 - fv)).astype(np.float32)
    w10 = (fu * (1 - fv)).astype(np.float32)
    w01 = ((1 - fu) * fv).astype(np.float32)
    w11 = (fu * fv).astype(np.float32)

    # z-run decomposition: maximal segments where both r0 and r1 step by a
    # constant d in {-1,0,1}
    runs = []
    k = 0
    while k < N:
        step = 0
        if k + 1 < N:
            d = int(r0[k + 1] - r0[k])
            if d == int(r1[k + 1] - r1[k]) and d in (-1, 0, 1):
                step = d
        ln = 1
        while (k + ln < N
               and int(r0[k + ln] - r0[k]) == step * ln
               and int(r1[k + ln] - r1[k]) == step * ln):
            ln += 1
        runs.append((k, ln, int(r0[k]), int(r1[k]), step))
        k += ln

    return dict(idxA=idxA, w00=w00, w01=w01, w10=w10, w11=w11, fw=fw, runs=runs)


def _pack_idxs(idx_flat):
    """int16 dma_gather index layout: element i at [i%16, i//16], replicated to 128 partitions."""
    t = idx_flat.reshape(-1, 16).T.astype(np.int16)  # [16, n/16]
    return np.ascontiguousarray(np.tile(t, (8, 1)))  # [128, n/16]


def _chunk_plan(tables):
    """Stream chunks (2048 ZPI entries each) in the order matching the calls'
    p-band progression; per call, how many streamed chunks it needs."""
    idxA = tables["idxA"].reshape(-1).astype(np.int64)
    # call c covers points c*NIDX..(c+1)*NIDX-1; entries r and r+1 needed
    need = []
    for c in range(NCALLS):
        rs = idxA[c * NIDX:(c + 1) * NIDX]
        need.append((int(rs.min()) // (CH * 128), (int(rs.max()) + 1) // (CH * 128)))
    first_lo, _ = need[0]
    last_lo, _ = need[-1]
    descending = first_lo >= last_lo
    order = list(range(NGRP - 1, -1, -1)) if descending else list(range(NGRP))
    pos = {g: i for i, g in enumerate(order)}
    nch = [max(pos[min(lo, NGRP - 1)], pos[min(hi, NGRP - 1)]) + 1 for lo, hi in need]
    return order, nch


def _build_program(tables, reps=1):
    """Raw-Bass (explicit semaphore) program; see module docstring for the
    pipeline.  All cross-engine waits are standalone wait_ge instructions.

    Engine streams:
      sync   (SP HWDGE):  const/volume loads, per-chunk ZPI stream writes
      scalar (ACT):       h1-slot entry assembly copies; weight-tile loads and
                          interleaved output writes (HWDGE)
      vector (DVE):       z-interp into ZS entry layout, 4-corner combine
      gpsimd (SWDGE):     one dma_gather per 1024 output points, fired as
                          soon as the chunks its points touch are streamed
    """
    import concourse.bass as bass
    from concourse import bacc, mybir

    runs = tables["runs"]
    f16 = mybir.dt.float16
    i16 = mybir.dt.int16

    nc = bacc.Bacc("TRN2", target_bir_lowering=False, debug=False, num_devices=8)

    vol_in = [nc.dram_tensor(f"vol{v}", [128, NROWS], f16, kind="ExternalInput") for v in range(2)]
    idx_dram = nc.dram_tensor("idxA", [128, NROWS // 16], i16, kind="ExternalInput")
    wts4 = nc.dram_tensor("wts4", [4, 128, 128], f16, kind="ExternalInput")
    fwrep = nc.dram_tensor("fwrep", [2, 128, 128], f16, kind="ExternalInput")
    out_i = nc.dram_tensor("outI", [NROWS, 256], f16, kind="ExternalOutput")
    zpi = nc.dram_tensor("zpi", [NROWS + 1, ESTEP], f16, kind="Internal")

    AP = bass.AP

    WD = 4 * GPC * N  # packed weight tile width (4096)
    idx_t = nc.alloc_sbuf_tensor("idx_t", [128, NROWS // 16], i16)
    fw_t = [nc.alloc_sbuf_tensor(f"fw{c}_t", [128, 128], f16) for c in range(2)]
    w4_t = [nc.alloc_sbuf_tensor(f"w4_{c}", [128, 128], f16) for c in range(4)]
    wt_sb = [nc.alloc_sbuf_tensor(f"wt_{s}", [128, WD], f16) for s in range(3)]
    vt16 = [nc.alloc_sbuf_tensor(f"vt16_{s}", [128, CH * N], f16) for s in range(4)]
    zs = [nc.alloc_sbuf_tensor(f"zs{s}", [128, CH * ESTEP], f16) for s in range(3)]
    ztmp = nc.alloc_sbuf_tensor("ztmp", [128, CH * N], f16)
    At = [nc.alloc_sbuf_tensor(f"At{s}", [128, GPC * ELEM], f16) for s in range(4)]
    mt = [nc.alloc_sbuf_tensor(f"m{s}", [128, GPC * N], f16) for s in range(8)]
    accb = [nc.alloc_sbuf_tensor(f"accb{s}", [128, GPC * 256], f16) for s in range(4)]

    nrows_ap = NROWS  # gather element at entry r reads entries r, r+1; r <= 16383
    NC_ = NCALLS
    ZSW = CH * ESTEP  # 8192
    chunk_order, nch = _chunk_plan(tables)
    # the h1 cross-chunk copy sources chunk g+1, which must already be in the
    # other ZS buffer -> chunks must stream top-down
    assert chunk_order == list(range(NGRP - 1, -1, -1)), chunk_order

    from contextlib import ExitStack
    with ExitStack() as _sctx:
        block = _sctx.enter_context(nc.Block())
        s_idx = _sctx.enter_context(nc.semaphore("s_idx"))
        s_wf = _sctx.enter_context(nc.semaphore("s_wf"))
        s_mz = _sctx.enter_context(nc.semaphore("s_mz"))
        s_l = [_sctx.enter_context(nc.semaphore(f"s_l{p}")) for p in range(4)]
        s_wl = [_sctx.enter_context(nc.semaphore(f"s_wl{p}")) for p in range(3)]
        s_z = _sctx.enter_context(nc.semaphore("s_z"))
        s_a = _sctx.enter_context(nc.semaphore("s_a"))
        s_zw = [_sctx.enter_context(nc.semaphore(f"s_zw{p}")) for p in range(NGRP)]
        s_zz = _sctx.enter_context(nc.semaphore("s_zz"))
        s_g = [_sctx.enter_context(nc.semaphore(f"s_g{p}")) for p in range(4)]
        s_c = _sctx.enter_context(nc.semaphore("s_c"))
        s_o = [_sctx.enter_context(nc.semaphore(f"s_o{p}")) for p in range(4)]
        s_v = _sctx.enter_context(nc.semaphore("s_v"))

        @block.sync
        def _(sync):
            for v in range(2):  # chunk 0 loads first: unblock DVE asap
                sync.dma_start(
                    AP(vt16[v], 0, [[CH * N, 128], [1, CH * N]]),
                    AP(vol_in[v], chunk_order[0] * CH * N, [[NROWS, 128], [1, CH * N]]),
                ).then_inc(s_l[v], 16)
            sync.dma_start(idx_t.ap(), idx_dram.ap()).then_inc(s_idx, 16)
            for c in range(2):
                sync.dma_start(fw_t[c].ap(), AP(fwrep, c * 128 * 128, [[128, 128], [1, 128]])).then_inc(s_wf, 16)
            for c in range(4):
                sync.dma_start(w4_t[c].ap(), AP(wts4, c * 128 * 128, [[128, 128], [1, 128]])).then_inc(s_wf, 16)
            # one-time: zero ZPI entry 16384 (read by gathers at r=16383)
            sync.wait_ge(s_mz, 1)
            sync.dma_start(
                AP(zpi, NROWS * ESTEP, [[128, 4], [1, 128]]),
                AP(mt[0], 0, [[GPC * N, 4], [1, 128]]),
            ).then_inc(s_zz, 16)
            for r in range(reps):
                for gi, g in enumerate(chunk_order):
                    for v in range(2):
                        zc = r * 16 + gi * 2 + v
                        if r == 0 and gi == 0:
                            continue  # preloaded above
                        if zc >= 4:
                            sync.wait_ge(s_z, zc - 3)  # WAR vt16 vs z-interp
                        sync.dma_start(
                            AP(vt16[zc % 4], 0, [[CH * N, 128], [1, CH * N]]),
                            AP(vol_in[v], g * CH * N, [[NROWS, 128], [1, CH * N]]),
                        ).then_inc(s_l[zc % 4], 16)
                for c in range(NC_):
                    gc = r * NC_ + c
                    sync.wait_ge(s_c, gc + 1)
                    sync.dma_start(
                        AP(out_i, c * NIDX * 256, [[256, 128], [128 * 256, GPC], [1, 256]]),
                        AP(accb[gc % 4], 0, [[GPC * 256, 128], [256, GPC], [1, 256]]),
                    ).then_inc(s_o[gc % 4], 16)
            for p in range(4):
                sync.wait_ge(s_o[p], 16 * ((NC_ * reps - p + 3) // 4))

        @block.scalar
        def _(scalar):
            def wt_gen(gc):
                # build the k-replicated weight tile for call gc%NC_ on-chip:
                # wt[j, ci*1024 + slot*128 + k] = w_ci(call*8+slot, j)
                c = gc % NC_
                for ci in range(4):
                    scalar.copy(
                        AP(wt_sb[gc % 3], ci * GPC * N, [[WD, 128], [N, GPC], [1, N]]),
                        AP(w4_t[ci], c * GPC, [[128, 128], [1, GPC], [0, N]]),
                    )
                # ACT write visibility to DVE: drain the pipeline before the
                # semaphore rises
                scalar.drain()
                scalar.sem_inc(s_wl[gc % 3], 16)

            scalar.wait_ge(s_wf, 96)
            for gc in range(min(3, NC_ * reps)):
                wt_gen(gc)
            for r in range(reps):
                for gi, g in enumerate(chunk_order):
                    # assemble h1 slots of chunk g: entry r gets row r+128,
                    # i.e. subgroup s copies from subgroup s+1 (h0 slot)
                    gcg = r * 8 + gi
                    scalar.wait_ge(s_z, r * 16 + 2 * gi + 2)
                    b = zs[gcg % 3]
                    for v in range(2):
                        scalar.copy(
                            AP(b, v * 256 + 128, [[ZSW, 128], [ESTEP, CH - 1], [1, N]]),
                            AP(b, ESTEP + v * 256, [[ZSW, 128], [ESTEP, CH - 1], [1, N]]),
                        )
                    last_ins = None
                    for v in range(2):
                        dst = AP(b, (CH - 1) * ESTEP + v * 256 + 128, [[ZSW, 128], [1, N]])
                        if g == NGRP - 1:
                            # top chunk: rows >= 16384 are zero (border clamp
                            # gives these corners zero weight; keep finite)
                            last_ins = scalar.memzero(dst)
                        else:
                            src_b = zs[(gcg - 1) % 3]
                            last_ins = scalar.copy(dst, AP(src_b, v * 256, [[ZSW, 128], [1, N]]))
                    last_ins.then_inc(s_a, 1)
                    # stream this chunk's assembled entries to ZPI; the ACT
                    # sequencer runs ahead of the engine pipeline, so fully
                    # drain the copies before the DMA reads the buffer
                    scalar.drain()
                    scalar.wait_ge(s_a, gcg + 1)
                    if gi == 0 and r >= 1:
                        scalar.wait_ge(s_c, NC_ * r)  # WAR zpi vs prev-rep gathers
                    scalar.dma_start(
                        AP(zpi, g * CH * 128 * ESTEP, [[ESTEP, 128], [128 * ESTEP, CH], [1, ESTEP]]),
                        AP(b, 0, [[ZSW, 128], [ESTEP, CH], [1, ESTEP]]),
                    ).then_inc(s_zw[gi], 16)
                for c in range(NC_):
                    gc = r * NC_ + c
                    if gc < 3:
                        continue  # pre-generated
                    scalar.wait_ge(s_c, gc - 2)  # WAR wt_sb vs combine
                    wt_gen(gc)

        @block.gpsimd
        def _(gpsimd):
            nreg = gpsimd.to_reg(NIDX)
            gpsimd.wait_ge(s_idx, 16)
            sv = AP(zpi, 0, [[ESTEP, nrows_ap], [1, ELEM]])
            for r in range(reps):
                for c in range(NC_):
                    gc = r * NC_ + c
                    if gc == 0:
                        gpsimd.wait_ge(s_zz, 16)
                    for pos in range(nch[c]):
                        gpsimd.wait_ge(s_zw[pos], 16 * (r + 1))
                    if gc >= 4:
                        gpsimd.wait_ge(s_c, gc - 3)  # WAR At vs combine
                    gpsimd.dma_gather(
                        AP(At[gc % 4], 0, [[GPC * ELEM, 128], [ELEM, GPC], [1, ELEM]]),
                        sv,
                        AP(idx_t, c * COLS, [[NROWS // 16, 128], [1, COLS]]),
                        NIDX, nreg, ELEM, elem_step=ESTEP,
                    ).then_inc(s_g[gc % 4], 16)

        @block.vector
        def _(vector):
            mult = mybir.AluOpType.mult
            VC = [0]

            def vsync(last_ins):
                # DVE pipeline does not interlock same-engine RAW hazards
                last_ins.then_inc(s_v, 1)
                VC[0] += 1
                vector.wait_ge(s_v, VC[0])

            vector.wait_ge(s_wf, 96)
            vector.memset(AP(mt[0], 0, [[GPC * N, 4], [1, 128]]), 0.0).then_inc(s_mz, 1)

            def zchunk(r, gi, g, v):
                zc = r * 16 + gi * 2 + v
                gcg = r * 8 + gi
                if zc >= 1:
                    vector.wait_ge(s_z, zc)  # WAR ztmp/pipeline drain
                vector.wait_ge(s_l[zc % 4], 16 * (zc // 4 + 1))
                if v == 0 and gcg >= 3:
                    # WAR zs[gcg%3] vs chunk gcg-3's stream + chunk gcg-2's
                    # h1 cross-read
                    vector.wait_ge(s_a, gcg - 1)
                    vector.wait_ge(s_zw[(gcg - 3) % NGRP], 16 * ((gcg - 3) // NGRP + 1))
                s = vt16[zc % 4]
                b = zs[gcg % 3]
                last_ins = None
                for (ks, ln, r0s, r1s, st) in runs:
                    zdst = AP(b, v * 256 + ks, [[ZSW, 128], [ESTEP, CH], [1, ln]])
                    tdst = AP(ztmp, ks, [[CH * N, 128], [N, CH], [1, ln]])
                    v0 = AP(s, r0s, [[CH * N, 128], [N, CH], [st, ln]])
                    v1 = AP(s, r1s, [[CH * N, 128], [N, CH], [st, ln]])
                    f0 = AP(fw_t[0], ks, [[128, 128], [0, CH], [1, ln]])
                    f1 = AP(fw_t[1], ks, [[128, 128], [0, CH], [1, ln]])
                    vector.tensor_tensor(zdst, v0, f0, mult)
                    last_ins = vector.tensor_tensor(tdst, v1, f1, mult)
                vsync(last_ins)
                for (ks, ln, r0s, r1s, st) in runs:
                    zdst = AP(b, v * 256 + ks, [[ZSW, 128], [ESTEP, CH], [1, ln]])
                    tdst = AP(ztmp, ks, [[CH * N, 128], [N, CH], [1, ln]])
                    last_ins = vector.tensor_add(zdst, zdst, tdst)
                last_ins.then_inc(s_z, 1)

            def combine(r, c):
                gc = r * NC_ + c
                if gc >= 1:
                    vector.wait_ge(s_c, gc)  # WAR mt vs prev combine
                vector.wait_ge(s_g[gc % 4], 16 * (gc // 4 + 1))
                vector.wait_ge(s_wl[gc % 3], 16 * (gc // 3 + 1))
                if gc >= 4:
                    vector.wait_ge(s_o[gc % 4], 16 * (gc // 4))  # WAR accb
                A = At[gc % 4]
                W = wt_sb[gc % 3]
                shp = [[GPC * ELEM, 128], [ELEM, GPC], [1, N]]
                oshp = [[GPC * N, 128], [N, GPC], [1, N]]

                def wb(ci):
                    return AP(W, ci * GPC * N, [[WD, 128], [N, GPC], [1, N]])
                maps = [AP(m, 0, oshp) for m in mt]
                # corner offsets within a gathered element (f16 elems):
                #   vol v: (p0,q0)=v*256, (p1,q0)=v*256+128,
                #          (p0,q1)=v*256+512, (p1,q1)=v*256+640
                last_ins = None
                for v in range(2):
                    b = 4 * v
                    vector.tensor_tensor(maps[b + 0], AP(A, v * 256 + 0, shp), wb(0), mult)
                    vector.tensor_tensor(maps[b + 1], AP(A, v * 256 + 128, shp), wb(2), mult)
                    vector.tensor_tensor(maps[b + 2], AP(A, v * 256 + 512, shp), wb(1), mult)
                    last_ins = vector.tensor_tensor(maps[b + 3], AP(A, v * 256 + 640, shp), wb(3), mult)
                vsync(last_ins)
                for v in range(2):
                    b = 4 * v
                    vector.tensor_add(maps[b + 0], maps[b + 0], maps[b + 1])
                    last_ins = vector.tensor_add(maps[b + 2], maps[b + 2], maps[b + 3])
                vsync(last_ins)
                osh2 = [[GPC * 256, 128], [256, GPC], [1, N]]
                vector.tensor_add(AP(accb[gc % 4], 0, osh2), maps[0], maps[2])
                vector.tensor_add(AP(accb[gc % 4], 128, osh2), maps[4], maps[6]) \
                    .then_inc(s_c, 1)

            # interleave the first combines into the z tail: their gathers
            # land while later chunks are still z-interping
            ileave = {(NGRP - 1, 0): [0], (NGRP - 1, 1): [1]}
            for r in range(reps):
                for gi, g in enumerate(chunk_order):
                    for v in range(2):
                        zchunk(r, gi, g, v)
                        for c in ileave.get((gi, v), []):
                            combine(r, c)
                for c in range(2, NC_):
                    combine(r, c)

    nc.compile()
    return nc


def _exact_label_fixup(label_g, theta, lab_f, out_bool):
    """Recompute voxels of |lab_f - 0.5| < FIX_EPS in the reference's exact
    f32 arithmetic order (validated bit-exact against the jax reference)."""
    eps = np.float32(FIX_EPS)
    cand = np.abs(lab_f - np.float32(0.5)) < eps
    if not cand.any():
        return out_bool
    bb, ii, jj, kk = np.nonzero(cand.reshape(-1, N, N, N))
    v = _exact_reference_values(label_g, theta, bb, ii, jj, kk)
    out_bool.reshape(-1, N, N, N)[bb, ii, jj, kk] = v > np.float32(0.5)
    return out_bool


def _exact_reference_values(vol_g, theta, bb, ii, jj, kk):
    """Reference-order f32 trilinear values at selected voxels.

    Replicates: grid einsum (x*t0 + y*t1 + z*t2, left-assoc f32) + t3; unnorm;
    8-corner accumulation in (z,y,x) order with w=(wz*wy)*wx, out += v*w.
    """
    f32 = np.float32
    t = np.arange(N, dtype=f32)
    xn = ((f32(2.0) * t + f32(1.0)) / f32(N) - f32(1.0)).astype(f32)
    th = theta.astype(f32)

    x = xn[ii]; y = xn[jj]; z = xn[kk]

    # f32 fma via f64 (exact up to negligible double-rounding corner cases)
    def fma32(a, b, c):
        return (np.float64(a) * np.float64(b) + c.astype(np.float64)).astype(f32)

    # grid components — XLA CPU lowers the einsum as an FMA chain (verified
    # bit-exact): fma(z, t2, fma(y, t1, x*t0)) + t3
    def comp(r):
        a = fma32(y, th[r, 1], (x * th[r, 0]).astype(f32))
        a = fma32(z, th[r, 2], a)
        return (a + th[r, 3]).astype(f32)
    gx, gy, gz = comp(0), comp(1), comp(2)

    def unnorm(c):
        return np.clip(((c + f32(1.0)) * f32(N) - f32(1.0)) * f32(0.5), f32(0.0), f32(N - 1))
    ux, uy, uz = unnorm(gx), unnorm(gy), unnorm(gz)
    x0 = np.floor(ux); y0 = np.floor(uy); z0 = np.floor(uz)
    fx = (ux - x0).astype(f32); fy = (uy - y0).astype(f32); fz = (uz - z0).astype(f32)
    x0i = x0.astype(np.int64); y0i = y0.astype(np.int64); z0i = z0.astype(np.int64)
    x1i = np.minimum(x0i + 1, N - 1); y1i = np.minimum(y0i + 1, N - 1); z1i = np.minimum(z0i + 1, N - 1)

    vol = vol_g.reshape(-1, N, N, N)
    out = np.zeros(bb.shape, f32)
    one = f32(1.0)
    for zi, wz in ((z0i, (one - fz).astype(f32)), (z1i, fz)):
        for yi, wy in ((y0i, (one - fy).astype(f32)), (y1i, fy)):
            for xi, wx in ((x0i, (one - fx).astype(f32)), (x1i, fx)):
                # inp[b, c, zi, yi, xi] in transposed space == vol[b, xi, yi, zi]
                vals = vol[bb, xi, yi, zi]
                w = ((wz * wy).astype(f32) * wx).astype(f32)
                out = (out + (vals * w).astype(f32)).astype(f32)
    return out


def _host_fallback(input_g, label_g, transform):
    """Arbitrary-transform fallback: full reference computation on host."""
    bb, ii, jj, kk = np.meshgrid(np.arange(8), np.arange(N), np.arange(N), np.arange(N), indexing="ij")
    bb, ii, jj, kk = (a.reshape(-1) for a in (bb, ii, jj, kk))
    theta = transform[:3].astype(np.float32)
    aug_inp = _exact_reference_values(input_g, theta, bb, ii, jj, kk).reshape(8, 1, N, N, N)
    lab = _exact_reference_values(label_g, theta, bb, ii, jj, kk).reshape(8, 1, N, N, N)
    return aug_inp.astype(np.float32), lab > np.float32(0.5)


def _make_inputs(tables, input_g, label_g):
    idx_p = _pack_idxs(tables["idxA"].reshape(-1))
    # per-corner weight tables [j, i]; k-replicated tiles are built on-device
    wts4 = np.stack([tables[nm].T.copy() for nm in ("w00", "w01", "w10", "w11")]).astype(np.float16)
    fwrep = np.stack([np.tile(1.0 - tables["fw"], (128, 1)),
                      np.tile(tables["fw"], (128, 1))]).astype(np.float16)
    in_maps = []
    for b in range(8):
        in_maps.append({
            # partition-major: vol[p, gc*128+k] = volume[row=gc*128+p, k]
            "vol0": np.ascontiguousarray(
                input_g[b, 0].reshape(128, 128, N).astype(np.float16).transpose(1, 0, 2).reshape(128, NROWS)),
            "vol1": np.ascontiguousarray(
                label_g[b, 0].reshape(128, 128, N).astype(np.float16).transpose(1, 0, 2).reshape(128, NROWS)),
            "idxA": idx_p, "wts4": wts4, "fwrep": fwrep,
        })
    return in_maps


def kernel(input_g, label_g, transform):
    input_g = np.ascontiguousarray(input_g, dtype=np.float32)
    label_g = np.ascontiguousarray(label_g, dtype=np.float32)
    transform = np.asarray(transform, dtype=np.float32)
    theta = transform[:3]

    structured = (abs(float(theta[0, 2])) < 1e-12 and abs(float(theta[1, 2])) < 1e-12
                  and abs(float(theta[2, 0])) < 1e-12 and abs(float(theta[2, 1])) < 1e-12)
    if not structured:
        return _host_fallback(input_g, label_g, transform)

    from concourse.bass_utils import run_bass_kernel_spmd

    tables = _host_tables(theta)
    key = transform.tobytes()
    if key not in _CACHE:
        _CACHE[key] = _build_program(tables)
    nc = _CACHE[key]

    in_maps = _make_inputs(tables, input_g, label_g)
    res = run_bass_kernel_spmd(nc, in_maps, core_ids=list(range(8)))

    aug_inp = np.empty((8, 1, N, N, N), np.float32)
    lab_f = np.empty((8, 1, N, N, N), np.float32)
    for b in range(8):
        oi = res.results[b]["outI"]
        aug_inp[b, 0] = oi[:, 0:128].astype(np.float32).reshape(N, N, N)
        lab_f[b, 0] = oi[:, 128:256].astype(np.float32).reshape(N, N, N)

    out_bool = lab_f > np.float32(0.5)
    out_bool = _exact_label_fixup(label_g, theta, lab_f, out_bool)
    return aug_inp, out_bool
